# revision 10
# baseline (speedup 1.0000x reference)
"""Trainium2 Bass kernel for nn_Attention_10771777978404 (sparse_attention).

Sharding over 8 NeuronCores: 2 batch-groups x 4 cores (tensor parallel over
heads within each batch group).
  - core ci handles batch ci//4 and heads [4*(ci%4), 4*(ci%4)+4): it computes
    its q/k/v projections (columns of wq/wk/wv), RoPE, causal attention with
    the low-rank sigmoid gate, and a full-width partial of the output
    projection from its 4 heads (rows of wo).
  - the rank-32 adapter weights are replicated inside each batch group; the
    sigmoid gate is computed as 1/(1+exp(-A)) so the scalar engine only ever
    uses the Exp table (no Sigmoid-table reloads, no DRAM staging).
  - host sums the 4 partial output projections per batch (fp16 partials).

Everything on-device is bf16 with fp32 PSUM accumulation.

Schedule notes (v2):
  - diagonal-band tiles only compute the live query columns [128j:512]
    (causal wedge), cutting PE/ACT/DVE work on the band by ~37%.
  - per-head softmax denominators live in ONE PSUM bank at partition
    offsets 32h, removing the head-boundary WAR on the rowsum accumulator.
  - the wo partial-projection PSUM tiles rotate over 4 banks (pp+po pools)
    and the PSUM->SBUF copies alternate ACT/DVE; the per-chunk output DMAs
    are batched into one 2MB DMA per query block (HWDGE descriptor
    generation is a serial ~630ns/dma resource).
  - input DMAs are coarsened and ordered by first-use so the HWDGE queue
    delivers x/wq/wk chunks at PE pace from the start.

self-contained: hardcodes the problem shapes; only needs `concourse` (on
PYTHONPATH in this container) + jax axon devices.
"""

import math
from contextlib import ExitStack
from dataclasses import dataclass

import numpy as np
import ml_dtypes

import concourse.tile as tile
from concourse import bacc
from concourse import mybir
from concourse import bass_utils

BF16 = mybir.dt.bfloat16
F16 = mybir.dt.float16
F32 = mybir.dt.float32
AF = mybir.ActivationFunctionType


@dataclass(frozen=True)
class Cfg:
    B: int = 2
    S: int = 2048
    DIM: int = 2048
    NH: int = 16
    HD: int = 128
    RANK: int = 32
    NCORES: int = 8
    GROUPS: int = 2     # batch groups of 4 cores
    QT: int = 512       # query block (free dim of score tiles)
    KT: int = 128       # key block (partition dim of score tiles)
    PIPE: int = 5       # score tiles in flight ahead of rowsum/AV drains
    repeat: int = 1     # hardware-loop repetitions of the whole body (timing)
    # ablation flags (profiling on hardware; all True for the real kernel)
    use_gate: bool = True
    use_rowsum: bool = True
    use_attn: bool = True
    use_wo: bool = True

    @property
    def CPG(self):
        return self.NCORES // self.GROUPS  # cores per batch group

    @property
    def HLOC(self):
        return self.NH // self.CPG  # heads per core (4)

    @property
    def DH(self):
        return self.HLOC * self.HD  # per-core head-dim span (512)

    @property
    def KTILES(self):
        return self.DIM // 128  # contraction tiles for projections

    @property
    def QTN(self):
        return self.S // self.QT

    @property
    def DIAG(self):
        return self.QT // self.KT  # k-tiles per diagonal band


FULL = Cfg()


def build_nc(cfg: Cfg = FULL):
    c = cfg
    assert c.HD == 128 and c.KT == 128
    nc = bacc.Bacc("TRN2", target_bir_lowering=False, debug=False,
                   num_devices=c.NCORES)

    # ---- kernel I/O (per core: one batch, HLOC heads) ----
    xT = nc.dram_tensor("xT", [c.DIM, c.S], BF16, kind="ExternalInput")
    wqT = nc.dram_tensor("wqT", [c.DIM, c.DH], BF16, kind="ExternalInput")
    wkT = nc.dram_tensor("wkT", [c.DIM, c.DH], BF16, kind="ExternalInput")
    wvT = nc.dram_tensor("wvT", [c.DIM, c.DH], BF16, kind="ExternalInput")
    wocT = nc.dram_tensor("wocT", [c.DH, c.DIM], BF16, kind="ExternalInput")
    waT = nc.dram_tensor("waT", [c.DIM, 2 * c.RANK], BF16, kind="ExternalInput")
    c2d = nc.dram_tensor("c2d", [c.HD, c.S], BF16, kind="ExternalInput")
    s2d = nc.dram_tensor("s2d", [c.HD, c.S], BF16, kind="ExternalInput")
    pswapd = nc.dram_tensor("pswapd", [c.HD, c.HD], BF16, kind="ExternalInput")
    maskdd = nc.dram_tensor("maskdd", [c.DIAG, c.KT, c.QT], BF16, kind="ExternalInput")

    # partial output projection, transposed: pout[j, t] (fp16; host sums the
    # 4 partials of each batch group in fp32)
    pout = nc.dram_tensor("pout", [c.DIM, c.S], F16, kind="ExternalOutput")

    isqrt = 1.0 / math.sqrt(c.HD)
    sc_score = isqrt
    sc_gate = -1.0
    NKT = c.KTILES

    with ExitStack() as _ctx:
        tc = _ctx.enter_context(tile.TileContext(nc))
        # persistent pools (whole-iteration lifetime)
        cst = _ctx.enter_context(tc.tile_pool(name="const", bufs=1))
        adp = _ctx.enter_context(tc.tile_pool(name="ap", bufs=1))
        qkp = _ctx.enter_context(tc.tile_pool(name="qk", bufs=1))
        vp = _ctx.enter_context(tc.tile_pool(name="vp", bufs=1))
        rtp = _ctx.enter_context(tc.tile_pool(name="rope_t", bufs=1))
        pge = _ctx.enter_context(tc.tile_pool(name="pge", bufs=6))
        gwk = _ctx.enter_context(tc.tile_pool(name="gwk", bufs=2))
        ogp = _ctx.enter_context(tc.tile_pool(name="og", bufs=1))
        # PSUM pools (8 banks total): pp 2 + ps 4 + po 2; the per-head
        # softmax-denominator tiles and the norm-broadcast tiles share the
        # pp rotation so rowsum accumulation never WARs the previous head
        pp = _ctx.enter_context(tc.tile_pool(name="pp", bufs=2, space="PSUM"))
        psp = _ctx.enter_context(tc.tile_pool(name="ps", bufs=4, space="PSUM"))
        pop = _ctx.enter_context(tc.tile_pool(name="po", bufs=2, space="PSUM"))

        def body():
            # ---- constants ----
            c2_sb = cst.tile([128, c.S], BF16, name="c2_sb", tag="c2")
            s2_sb = cst.tile([128, c.S], BF16, name="s2_sb", tag="s2")
            psw_sb = cst.tile([128, 128], BF16, name="psw_sb", tag="psw")
            mask_sb = cst.tile([128, c.DIAG, c.QT], BF16, name="mask_sb", tag="mask")
            ones_sb = cst.tile([128, 1], BF16, name="ones_sb", tag="ones")
            oner_sb = cst.tile([1, 128], F16, name="oner_sb", tag="oner")

            # packed adapter projections: one [64,512] matmul computes both
            # aq and ak rows (halves the adapter matmul count); ak is then
            # moved to partition base 0 with one SBUF->SBUF DMA (engines
            # cannot shift partitions, DMA can)
            aqk_sb = adp.tile([2 * c.RANK, c.S], BF16, name="aqk_sb", tag="aqk")
            ak_sb = adp.tile([c.RANK, c.S], BF16, name="ak_sb", tag="ak")
            q_sb = [qkp.tile([128, c.S], BF16, name=f"q{h}_sb", tag=f"q{h}")
                    for h in range(c.HLOC)]
            k_sb = [qkp.tile([128, c.S], BF16, name=f"k{h}_sb", tag=f"k{h}")
                    for h in range(c.HLOC)]
            v_sb = vp.tile([128, c.S // 128, c.DH], BF16, name="v_sb", tag="v")

            with tc.tile_pool(name="xtp", bufs=1) as xtp:
                wa_sb = xtp.tile([128, c.KTILES, 2 * c.RANK], BF16,
                                 name="wa_sb", tag="wa")
                nc.sync.dma_start(out=wa_sb,
                                  in_=waT.ap().rearrange("(t p) m -> p t m", p=128))
                xt_sb = xtp.tile([128, c.KTILES, c.S], BF16, name="xt_sb", tag="xt")
                xr = xT.ap().rearrange("(t p) n -> p t n", p=128)

                def xsl(j, sl):
                    return xt_sb[:, j, sl]

                nc.vector.memset(ones_sb, 1.0)
                nc.vector.memset(oner_sb, 1.0)

                # ---- adapter + q/k projections, kt-outer, 8 live psum accs ----
                apools = [pp, psp, pop, psp]
                acc = [apools[i].tile([2 * c.RANK, c.QT], F32, name=f"acc_a{i}",
                                      tag=("pp", "ps", "po", "ps")[i])
                       for i in range(c.QTN)]

                with tc.tile_pool(name="wqk", bufs=1) as wp:
                    wq_sb = wp.tile([128, c.KTILES, c.DH], BF16, name="wq_sb", tag="wq")
                    wk_sb = wp.tile([128, c.KTILES, c.DH], BF16, name="wk_sb", tag="wk")
                    # input DMAs ordered by first use; chunk sizes grow so the
                    # HWDGE queue stays ahead of the PE's kt-outer consumption
                    nc.sync.dma_start(out=xt_sb[:, 0, :], in_=xr[:, 0, :])
                    wqr = wqT.ap().rearrange("(t p) m -> p t m", p=128)
                    wkr = wkT.ap().rearrange("(t p) m -> p t m", p=128)
                    nc.scalar.dma_start(out=wq_sb[:, 0:4, :], in_=wqr[:, 0:4, :])
                    nc.scalar.dma_start(out=wk_sb[:, 0:4, :], in_=wkr[:, 0:4, :])
                    nc.sync.dma_start(out=xt_sb[:, 1, :], in_=xr[:, 1, :])
                    nc.sync.dma_start(out=xt_sb[:, 2:4, :], in_=xr[:, 2:4, :])
                    nc.scalar.dma_start(out=wq_sb[:, 4:, :], in_=wqr[:, 4:, :])
                    nc.scalar.dma_start(out=wk_sb[:, 4:, :], in_=wkr[:, 4:, :])
                    nc.sync.dma_start(out=xt_sb[:, 4:8, :], in_=xr[:, 4:8, :])
                    nc.sync.dma_start(out=xt_sb[:, 8:12, :], in_=xr[:, 8:12, :])
                    nc.sync.dma_start(out=xt_sb[:, 12:16, :], in_=xr[:, 12:16, :])

                    allqk = [(dst, w, h, qt)
                             for dst, w in ((q_sb, wq_sb), (k_sb, wk_sb))
                             for h in range(c.HLOC)
                             for qt in range(c.QTN)]
                    # first four groups run kt-outer interleaved with the
                    # adapter accumulation so the PE issues 8 matmuls per
                    # arriving x chunk instead of 4
                    head_pools = [(pp, "pp"), (psp, "ps"), (psp, "ps"),
                                  (pop, "po")]
                    headacc = [head_pools[i][0].tile([128, c.QT], F32,
                                                     name=f"acc_qk{i}",
                                                     tag=head_pools[i][1])
                               for i in range(4)]
                    for kt in range(NKT):
                        for qt in range(c.QTN):
                            nc.tensor.matmul(
                                acc[qt][:, :],
                                wa_sb[:, kt, :],
                                xsl(kt, slice(qt * c.QT, (qt + 1) * c.QT)),
                                start=(kt == 0), stop=(kt == NKT - 1))
                        for i in range(4):
                            dst, w, h, qt = allqk[i]
                            nc.tensor.matmul(
                                headacc[i][:, :],
                                w[:, kt, h * 128:(h + 1) * 128],
                                xsl(kt, slice(qt * c.QT, (qt + 1) * c.QT)),
                                start=(kt == 0), stop=(kt == NKT - 1))
                    for qt in range(c.QTN):
                        nc.vector.tensor_copy(
                            aqk_sb[:, qt * c.QT:(qt + 1) * c.QT], acc[qt][:, :])
                    nc.sync.dma_start(out=ak_sb[:, :],
                                      in_=aqk_sb[c.RANK:2 * c.RANK, :])
                    for i in range(4):
                        dst, w, h, qt = allqk[i]
                        nc.scalar.copy(dst[h][:, qt * c.QT:(qt + 1) * c.QT],
                                       headacc[i][:, :])
                    for dst, w, h, qt in allqk[4:]:
                        psum = pp.tile([128, c.QT], F32, name="psum_qk", tag="pp")
                        for kt in range(NKT):
                            nc.tensor.matmul(
                                psum[:, :],
                                w[:, kt, h * 128:(h + 1) * 128],
                                xsl(kt, slice(qt * c.QT, (qt + 1) * c.QT)),
                                start=(kt == 0), stop=(kt == NKT - 1))
                        nc.scalar.copy(dst[h][:, qt * c.QT:(qt + 1) * c.QT],
                                       psum[:, :])

                # ---- v projection: [tok, d] natural, moving 512 wide ----
                # RoPE tiles are interleaved into the v loop: the rope chain
                # is DVE/GPSIMD-paced, the v matmuls keep the PE busy
                rope_tiles = [(tiles, h, qt)
                              for tiles in (q_sb, k_sb)
                              for h in range(c.HLOC)
                              for qt in range(c.QTN)]

                def rope_step(i):
                    # m1 on GPSIMD (SBUF-only engine), m2 on DVE (PSUM read),
                    # final add alternates so neither engine is the pacer
                    tiles, h, qt = rope_tiles[i]
                    eng = nc.vector if i % 2 == 0 else nc.gpsimd
                    sl = slice(qt * c.QT, (qt + 1) * c.QT)
                    pswp = psp.tile([128, c.QT], F32, name="pswp", tag="ps")
                    nc.tensor.matmul(pswp[:, :], psw_sb[:, :],
                                     tiles[h][:, sl], start=True, stop=True)
                    m1 = rtp.tile([128, c.QT], BF16, name="rope_m1",
                                  tag=f"m1{i % 2}")
                    m2 = rtp.tile([128, c.QT], BF16, name="rope_m2",
                                  tag=f"m2{i % 2}")
                    nc.gpsimd.tensor_mul(m1[:, :], tiles[h][:, sl], c2_sb[:, sl])
                    nc.vector.tensor_mul(m2[:, :], pswp[:, :], s2_sb[:, sl])
                    eng.tensor_add(tiles[h][:, sl], m1[:, :], m2[:, :])

                with tc.tile_pool(name="wvp", bufs=1) as wvpool:
                    wv_sb = wvpool.tile([128, c.KTILES, c.DH], BF16,
                                        name="wv_sb", tag="wv")
                    wr = wvT.ap().rearrange("(t p) m -> p t m", p=128)
                    # fine first chunks: the first token-block's kt loop can
                    # start as soon as wv[0:4] lands
                    nc.sync.dma_start(out=wv_sb[:, 0:4, :], in_=wr[:, 0:4, :])
                    nc.sync.dma_start(out=wv_sb[:, 4:8, :], in_=wr[:, 4:8, :])
                    nc.sync.dma_start(out=wv_sb[:, 8:16, :], in_=wr[:, 8:16, :])
                    # rope tables arrive while the first v token-blocks run
                    nc.sync.dma_start(out=c2_sb, in_=c2d.ap())
                    nc.sync.dma_start(out=s2_sb, in_=s2d.ap())
                    nc.sync.dma_start(out=psw_sb, in_=pswapd.ap())
                    for tt in range(c.S // 128):
                        psum = pp.tile([128, c.DH], F32, name="psum_v", tag="pp")
                        for kt in range(NKT):
                            nc.tensor.matmul(
                                psum[:, :],
                                xsl(kt, slice(tt * 128, (tt + 1) * 128)),
                                wv_sb[:, kt, :],
                                start=(kt == 0), stop=(kt == NKT - 1))
                        nc.scalar.copy(v_sb[:, tt, :], psum[:, :])
                        rope_step(2 * tt)
                        rope_step(2 * tt + 1)

            # ---- attention + output projection, per 512-query block ----
            with tc.tile_pool(name="wog", bufs=1) as wog, \
                    tc.tile_pool(name="wo_out", bufs=2) as wop:
                nc.sync.dma_start(out=mask_sb,
                                  in_=maskdd.ap().rearrange("j p q -> p j q"))
                woc_sb = wog.tile([128, c.HLOC, c.DIM], BF16, name="woc_sb", tag="woc")
                wcr = wocT.ap().rearrange("(h p) j -> p h j", p=128)
                nc.sync.dma_start(out=woc_sb[:, 0:2, :], in_=wcr[:, 0:2, :])
                nc.sync.dma_start(out=woc_sb[:, 2:4, :], in_=wcr[:, 2:4, :])
                # double-buffered per-qt gate tiles r = sigmoid(A)[k, q]
                rg_sb = [wog.tile([128, c.DIAG * c.QTN, c.QT], BF16,
                                  name=f"rg{i}_sb", tag=f"rg{i}") for i in range(2)]

                def gate_step(qt, kt):
                    # r = 1/(1+exp(-A)) via the (shared) Exp table
                    qsl = slice(qt * c.QT, (qt + 1) * c.QT)
                    ksl = slice(kt * c.KT, (kt + 1) * c.KT)
                    pga = psp.tile([128, c.QT], F32, name="pga", tag="ps")
                    nc.tensor.matmul(pga[:, :], ak_sb[:, ksl],
                                     aqk_sb[0:c.RANK, qsl],
                                     start=True, stop=True)
                    ge = pge.tile([128, c.QT], BF16, name="ge", tag="p")
                    nc.scalar.activation(ge[:, :], pga[:, :], AF.Exp, scale=sc_gate)
                    gt = gwk.tile([128, c.QT], F32, name="gt", tag="gt")
                    nc.vector.tensor_scalar_add(gt[:, :], ge[:, :], 1.0)
                    gr = gwk.tile([128, c.QT], F32, name="gr", tag="gr")
                    nc.vector.reciprocal_approx_fast(out=gr[:, :], in_=gt[:, :])
                    nc.gpsimd.tensor_copy(rg_sb[qt % 2][:, kt, :], gr[:, :])

                if c.use_gate and c.use_attn:
                    for kt in range(c.DIAG):
                        gate_step(0, kt)

                for qt in range(c.QTN):
                    qsl = slice(qt * c.QT, (qt + 1) * c.QT)
                    nkt = c.DIAG * (qt + 1)  # causal k tiles
                    rg = rg_sb[qt % 2]

                    og_sb = ogp.tile([128, c.HLOC, c.QT], BF16, name="og_sb", tag="og")
                    if not c.use_attn:
                        nc.vector.memset(og_sb, 0.0)

                    def normalize(h, po, prs):
                        if not c.use_rowsum:
                            nc.vector.tensor_copy(og_sb[:, h, :], po[:, :])
                            return
                        rr = gwk.tile([1, c.QT], F32, name="rr", tag="rr")
                        nc.vector.reciprocal_approx_fast(
                            out=rr[:, :], in_=prs[0:1, :])
                        rrh = gwk.tile([1, c.QT], F16, name="rrh", tag="rrh")
                        nc.vector.tensor_copy(rrh[:, :], rr[:, :])
                        rbc = pp.tile([128, c.QT], F32, name="rbc", tag="pp")
                        nc.tensor.matmul(rbc[:, :], oner_sb[:, :], rrh[:, :],
                                         start=True, stop=True)
                        # DVE can't take two PSUM operands; stage the
                        # broadcast in SBUF first
                        rbs = gwk.tile([128, c.QT], F16, name="rbs", tag="rbs")
                        nc.vector.tensor_copy(rbs[:, :], rbc[:, :])
                        nc.vector.tensor_mul(og_sb[:, h, :], po[:, :],
                                             rbs[:, :])

                    # single software pipeline over all (h, kt) score tiles:
                    # drains lag PIPE steps behind scores ACROSS head
                    # boundaries, so head-end drain tails overlap the next
                    # head's score matmuls; normalize(h) is emitted inline
                    # right after head h's last drain
                    po_t, prs_t = {}, {}
                    stage = []  # (h, kt, col-slice, p_or_pm_tile, pgm_tile)

                    def score_step(h, kt):
                        ksl = slice(kt * c.KT, (kt + 1) * c.KT)
                        j = kt - c.DIAG * qt
                        qoff = 128 * j if j > 0 else 0
                        s = slice(qoff, c.QT)
                        qs = slice(qt * c.QT + qoff, (qt + 1) * c.QT)
                        ps = psp.tile([128, c.QT], F32, name="ps", tag="ps")
                        nc.tensor.matmul(ps[:, s], k_sb[h][:, ksl],
                                         q_sb[h][:, qs], start=True, stop=True)
                        p_sb = pge.tile([128, c.QT], BF16, name="p_sb", tag="p")
                        nc.scalar.activation(p_sb[:, s], ps[:, s], AF.Exp,
                                             scale=sc_score)
                        if j >= 0:
                            # diagonal band: 0/1 causal mask after exp,
                            # in place (rowsum needs the masked pre-gate sum)
                            nc.vector.tensor_mul(p_sb[:, s], p_sb[:, s],
                                                 mask_sb[:, j, s])
                        if c.use_gate:
                            pgm = pge.tile([128, c.QT], BF16, name="pgm",
                                           tag="pgm")
                            nc.vector.tensor_mul(pgm[:, s], p_sb[:, s],
                                                 rg[:, kt, s])
                        else:
                            pgm = p_sb
                        stage.append((h, kt, s, p_sb, pgm))

                    def drain_step():
                        h, kt, s, p_sb, pgm = stage.pop(0)
                        # pre-gate rowsum (softmax denominator)
                        if c.use_rowsum:
                            nc.tensor.matmul(prs_t[h][0:1, s],
                                             ones_sb[:, :], p_sb[:, s],
                                             start=(kt == 0),
                                             stop=(kt == nkt - 1),
                                             skip_group_check=True)
                        # out_h^T[d, q] += v[k, d].T @ p_gated[k, q]
                        nc.tensor.matmul(po_t[h][:, s],
                                         v_sb[:, kt, h * 128:(h + 1) * 128],
                                         pgm[:, s],
                                         start=(kt == 0), stop=(kt == nkt - 1),
                                         skip_group_check=True)
                        if kt == nkt - 1:
                            normalize(h, po_t[h], prs_t[h])

                    for h in range(c.HLOC if c.use_attn else 0):
                        po_t[h] = pop.tile([128, c.QT], F32, name="po", tag="po")
                        prs_t[h] = pp.tile([1, c.QT], F32, name="prs", tag="pp")
                        for kt in range(nkt):
                            score_step(h, kt)
                            if len(stage) > c.PIPE:
                                drain_step()
                    while stage:
                        drain_step()

                    # output-projection partial for this query block,
                    # interleaved with the NEXT block's gate generation so the
                    # PE fills the gate chain's latency with wo matmuls
                    nkt2 = (c.DIAG * (qt + 2)
                            if (qt + 1 < c.QTN and c.use_gate and c.use_attn)
                            else 0)
                    ncha = c.DIM // 128
                    f_sb = wop.tile([128, ncha, c.QT], F16, name="f_sb", tag="f")
                    pfpools = [(pp, "pp"), (pop, "po")]
                    # two gate steps lead the wo loop: their matmuls keep the
                    # PE fed while the last head's normalize chain completes
                    for kt in range(min(2, nkt2)):
                        gate_step(qt + 1, kt)
                    for ch in range(ncha if c.use_wo else 0):
                        pfp, pft = pfpools[ch % 2]
                        pf = pfp.tile([128, c.QT], F32, name="pf", tag=pft)
                        for h in range(c.HLOC):
                            nc.tensor.matmul(
                                pf[:, :],
                                woc_sb[:, h, ch * 128:(ch + 1) * 128],
                                og_sb[:, h, :],
                                start=(h == 0), stop=(h == c.HLOC - 1))
                        if ch % 2 == 0:
                            nc.scalar.copy(f_sb[:, ch, :], pf[:, :])
                        else:
                            nc.vector.tensor_copy(f_sb[:, ch, :], pf[:, :])
                        if ch + 2 < nkt2:
                            gate_step(qt + 1, ch + 2)
                        if ch % 4 == 3:
                            # batched output DMA per 4 chunks (0.5MB each):
                            # early chunks fly while later ones compute
                            nc.sync.dma_start(
                                out=pout.ap().rearrange(
                                    "(ch p) q -> p ch q",
                                    p=128)[:, ch - 3:ch + 1, qsl],
                                in_=f_sb[:, ch - 3:ch + 1, :])
                    if not c.use_wo:
                        for kt in range(nkt2):
                            gate_step(qt + 1, kt)

        if c.repeat > 1:
            with tc.For_i(0, c.repeat, 1,
                          hint_engines=(mybir.EngineType.PE,
                                        mybir.EngineType.DVE,
                                        mybir.EngineType.Activation,
                                        mybir.EngineType.Pool,
                                        mybir.EngineType.SP)):
                body()
        else:
            body()

    nc.compile()
    return nc


def make_core_inputs(inputs: dict, cfg: Cfg = FULL):
    """Host-side sharding: returns in_maps (one dict per core)."""
    c = cfg
    bf16 = ml_dtypes.bfloat16
    x = np.asarray(inputs["x"])
    mask = np.asarray(inputs["mask"])
    fc = np.asarray(inputs["freqs_cos"])
    fs = np.asarray(inputs["freqs_sin"])
    wq, wk, wv, wo = (np.asarray(inputs[k]) for k in ("wq", "wk", "wv", "wo"))
    wa_q, wa_k = np.asarray(inputs["wa_q"]), np.asarray(inputs["wa_k"])

    xTb = [np.ascontiguousarray(x[b].T).astype(bf16) for b in range(c.B)]
    waT = np.ascontiguousarray(
        np.concatenate([wa_q, wa_k], axis=0).T).astype(bf16)

    # rope tables in [d, tok] layout
    c2 = np.empty((c.HD, c.S), np.float32)
    s2 = np.empty((c.HD, c.S), np.float32)
    c2[0::2] = fc.T
    c2[1::2] = fc.T
    s2[0::2] = -fs.T
    s2[1::2] = fs.T
    c2 = c2.astype(bf16)
    s2 = s2.astype(bf16)

    psw = np.zeros((c.HD, c.HD), np.float32)
    idx = np.arange(c.HD)
    psw[idx, idx ^ 1] = 1.0
    psw = psw.astype(bf16)

    # diagonal-band mask patterns [j][k, q], extracted from the input mask
    qt_last = c.QTN - 1
    q0 = qt_last * c.QT
    maskd = np.empty((c.DIAG, c.KT, c.QT), np.float32)
    for j in range(c.DIAG):
        k0 = (c.DIAG * qt_last + j) * c.KT
        maskd[j] = (mask[0, 0, q0:q0 + c.QT, k0:k0 + c.KT].T == 0.0)
    maskd = maskd.astype(bf16)

    wslices = []
    for hs in range(c.CPG):
        rows = slice(hs * c.DH, (hs + 1) * c.DH)
        wslices.append({
            "wqT": np.ascontiguousarray(wq[rows].T).astype(bf16),
            "wkT": np.ascontiguousarray(wk[rows].T).astype(bf16),
            "wvT": np.ascontiguousarray(wv[rows].T).astype(bf16),
            "wocT": np.ascontiguousarray(wo[:, rows].T).astype(bf16),
        })

    in_maps = []
    for ci in range(c.NCORES):
        b = ci // c.CPG
        hs = ci % c.CPG
        in_maps.append({
            "xT": xTb[b],
            **wslices[hs],
            "waT": waT,
            "c2d": c2,
            "s2d": s2,
            "pswapd": psw,
            "maskdd": maskd,
        })
    return in_maps


def assemble_output(results, cfg: Cfg = FULL) -> np.ndarray:
    c = cfg
    out = np.empty((c.B, c.S, c.DIM), np.float32)
    for b in range(c.B):
        total = np.zeros((c.DIM, c.S), np.float32)
        for hs in range(c.CPG):
            total += np.asarray(results[b * c.CPG + hs]["pout"]).astype(np.float32)
        out[b] = total.T
    return out


_NC_CACHE = {}


def run(nc, in_maps, trace=False, cfg: Cfg = FULL, **kw):
    return bass_utils.run_bass_kernel_spmd(
        nc, in_maps, core_ids=list(range(cfg.NCORES)), trace=trace, **kw)


def kernel(**inputs) -> np.ndarray:
    cfg = FULL
    if cfg not in _NC_CACHE:
        _NC_CACHE[cfg] = build_nc(cfg)
    nc = _NC_CACHE[cfg]
    in_maps = make_core_inputs(inputs, cfg)
    res = run(nc, in_maps, cfg=cfg)
    return assemble_output(res.results, cfg)


if __name__ == "__main__":
    nc = build_nc(FULL)
    print("built ok")


# revision 11
# speedup vs baseline: 1.0063x; 1.0063x over previous
"""Trainium2 Bass kernel for nn_Attention_10771777978404 (sparse_attention).

Sharding over 8 NeuronCores: 2 batch-groups x 4 cores (tensor parallel over
heads within each batch group).
  - core ci handles batch ci//4 and heads [4*(ci%4), 4*(ci%4)+4): it computes
    its q/k/v projections (columns of wq/wk/wv), RoPE, causal attention with
    the low-rank sigmoid gate, and a full-width partial of the output
    projection from its 4 heads (rows of wo).
  - the rank-32 adapter weights are replicated inside each batch group; the
    sigmoid gate is computed as 1/(1+exp(-A)) so the scalar engine only ever
    uses the Exp table (no Sigmoid-table reloads, no DRAM staging).
  - host sums the 4 partial output projections per batch (fp16 partials).

Everything on-device is bf16 with fp32 PSUM accumulation.

Schedule notes (v2):
  - diagonal-band tiles only compute the live query columns [128j:512]
    (causal wedge), cutting PE/ACT/DVE work on the band by ~37%.
  - per-head softmax denominators live in ONE PSUM bank at partition
    offsets 32h, removing the head-boundary WAR on the rowsum accumulator.
  - the wo partial-projection PSUM tiles rotate over 4 banks (pp+po pools)
    and the PSUM->SBUF copies alternate ACT/DVE; the per-chunk output DMAs
    are batched into one 2MB DMA per query block (HWDGE descriptor
    generation is a serial ~630ns/dma resource).
  - input DMAs are coarsened and ordered by first-use so the HWDGE queue
    delivers x/wq/wk chunks at PE pace from the start.

self-contained: hardcodes the problem shapes; only needs `concourse` (on
PYTHONPATH in this container) + jax axon devices.
"""

import math
from contextlib import ExitStack
from dataclasses import dataclass

import numpy as np
import ml_dtypes

import concourse.tile as tile
from concourse import bacc
from concourse import mybir
from concourse import bass_utils

BF16 = mybir.dt.bfloat16
F16 = mybir.dt.float16
F32 = mybir.dt.float32
AF = mybir.ActivationFunctionType


@dataclass(frozen=True)
class Cfg:
    B: int = 2
    S: int = 2048
    DIM: int = 2048
    NH: int = 16
    HD: int = 128
    RANK: int = 32
    NCORES: int = 8
    GROUPS: int = 2     # batch groups of 4 cores
    QT: int = 512       # query block (free dim of score tiles)
    KT: int = 128       # key block (partition dim of score tiles)
    PIPE: int = 5       # score tiles in flight ahead of rowsum/AV drains
    repeat: int = 1     # hardware-loop repetitions of the whole body (timing)
    # ablation flags (profiling on hardware; all True for the real kernel)
    use_gate: bool = True
    use_rowsum: bool = True
    use_attn: bool = True
    use_wo: bool = True

    @property
    def CPG(self):
        return self.NCORES // self.GROUPS  # cores per batch group

    @property
    def HLOC(self):
        return self.NH // self.CPG  # heads per core (4)

    @property
    def DH(self):
        return self.HLOC * self.HD  # per-core head-dim span (512)

    @property
    def KTILES(self):
        return self.DIM // 128  # contraction tiles for projections

    @property
    def QTN(self):
        return self.S // self.QT

    @property
    def DIAG(self):
        return self.QT // self.KT  # k-tiles per diagonal band


FULL = Cfg()


def build_nc(cfg: Cfg = FULL):
    c = cfg
    assert c.HD == 128 and c.KT == 128
    nc = bacc.Bacc("TRN2", target_bir_lowering=False, debug=False,
                   num_devices=c.NCORES)

    # ---- kernel I/O (per core: one batch, HLOC heads) ----
    xT = nc.dram_tensor("xT", [c.DIM, c.S], BF16, kind="ExternalInput")
    wqT = nc.dram_tensor("wqT", [c.DIM, c.DH], BF16, kind="ExternalInput")
    wkT = nc.dram_tensor("wkT", [c.DIM, c.DH], BF16, kind="ExternalInput")
    wvT = nc.dram_tensor("wvT", [c.DIM, c.DH], BF16, kind="ExternalInput")
    wocT = nc.dram_tensor("wocT", [c.DH, c.DIM], BF16, kind="ExternalInput")
    waT = nc.dram_tensor("waT", [c.DIM, 2 * c.RANK], BF16, kind="ExternalInput")
    c2d = nc.dram_tensor("c2d", [c.HD, c.S], BF16, kind="ExternalInput")
    s2d = nc.dram_tensor("s2d", [c.HD, c.S], BF16, kind="ExternalInput")
    maskdd = nc.dram_tensor("maskdd", [c.DIAG, c.KT, c.QT], BF16, kind="ExternalInput")

    # partial output projection, transposed: pout[j, t] (fp16; host sums the
    # 4 partials of each batch group in fp32)
    pout = nc.dram_tensor("pout", [c.DIM, c.S], F16, kind="ExternalOutput")

    isqrt = 1.0 / math.sqrt(c.HD)
    sc_score = isqrt
    sc_gate = -1.0
    NKT = c.KTILES

    with ExitStack() as _ctx:
        tc = _ctx.enter_context(tile.TileContext(nc))
        # persistent pools (whole-iteration lifetime)
        cst = _ctx.enter_context(tc.tile_pool(name="const", bufs=1))
        adp = _ctx.enter_context(tc.tile_pool(name="ap", bufs=1))
        qkp = _ctx.enter_context(tc.tile_pool(name="qk", bufs=1))
        vp = _ctx.enter_context(tc.tile_pool(name="vp", bufs=1))
        rtp = _ctx.enter_context(tc.tile_pool(name="rope_t", bufs=1))
        pge = _ctx.enter_context(tc.tile_pool(name="pge", bufs=6))
        gwk = _ctx.enter_context(tc.tile_pool(name="gwk", bufs=2))
        ogp = _ctx.enter_context(tc.tile_pool(name="og", bufs=1))
        # PSUM pools (8 banks total): pp 2 + ps 4 + po 2; the per-head
        # softmax-denominator tiles and the norm-broadcast tiles share the
        # pp rotation so rowsum accumulation never WARs the previous head
        pp = _ctx.enter_context(tc.tile_pool(name="pp", bufs=2, space="PSUM"))
        psp = _ctx.enter_context(tc.tile_pool(name="ps", bufs=4, space="PSUM"))
        pop = _ctx.enter_context(tc.tile_pool(name="po", bufs=2, space="PSUM"))

        def body():
            # ---- constants ----
            c2_sb = cst.tile([128, c.S], BF16, name="c2_sb", tag="c2")
            s2_sb = cst.tile([128, c.S], BF16, name="s2_sb", tag="s2")
            mask_sb = cst.tile([128, c.DIAG, c.QT], BF16, name="mask_sb", tag="mask")
            ones_sb = cst.tile([128, 1], BF16, name="ones_sb", tag="ones")

            # packed adapter projections: one [64,512] matmul computes both
            # aq and ak rows (halves the adapter matmul count); ak is then
            # moved to partition base 0 with one SBUF->SBUF DMA (engines
            # cannot shift partitions, DMA can)
            aqk_sb = adp.tile([2 * c.RANK, c.S], BF16, name="aqk_sb", tag="aqk")
            ak_sb = adp.tile([c.RANK, c.S], BF16, name="ak_sb", tag="ak")
            q_sb = [qkp.tile([128, c.S], BF16, name=f"q{h}_sb", tag=f"q{h}")
                    for h in range(c.HLOC)]
            k_sb = [qkp.tile([128, c.S], BF16, name=f"k{h}_sb", tag=f"k{h}")
                    for h in range(c.HLOC)]
            v_sb = vp.tile([128, c.S // 128, c.DH], BF16, name="v_sb", tag="v")

            with tc.tile_pool(name="xtp", bufs=1) as xtp:
                wa_sb = xtp.tile([128, c.KTILES, 2 * c.RANK], BF16,
                                 name="wa_sb", tag="wa")
                nc.sync.dma_start(out=wa_sb,
                                  in_=waT.ap().rearrange("(t p) m -> p t m", p=128))
                xt_sb = xtp.tile([128, c.KTILES, c.S], BF16, name="xt_sb", tag="xt")
                xr = xT.ap().rearrange("(t p) n -> p t n", p=128)

                def xsl(j, sl):
                    return xt_sb[:, j, sl]

                nc.vector.memset(ones_sb, 1.0)

                # ---- adapter + q/k projections, kt-outer, 8 live psum accs ----
                apools = [pp, psp, pop, psp]
                acc = [apools[i].tile([2 * c.RANK, c.QT], F32, name=f"acc_a{i}",
                                      tag=("pp", "ps", "po", "ps")[i])
                       for i in range(c.QTN)]

                with tc.tile_pool(name="wqk", bufs=1) as wp:
                    wq_sb = wp.tile([128, c.KTILES, c.DH], BF16, name="wq_sb", tag="wq")
                    wk_sb = wp.tile([128, c.KTILES, c.DH], BF16, name="wk_sb", tag="wk")
                    # input DMAs ordered by first use; chunk sizes grow so the
                    # HWDGE queue stays ahead of the PE's kt-outer consumption
                    nc.sync.dma_start(out=xt_sb[:, 0, :], in_=xr[:, 0, :])
                    wqr = wqT.ap().rearrange("(t p) m -> p t m", p=128)
                    wkr = wkT.ap().rearrange("(t p) m -> p t m", p=128)
                    nc.scalar.dma_start(out=wq_sb[:, 0:4, :], in_=wqr[:, 0:4, :])
                    nc.scalar.dma_start(out=wk_sb[:, 0:4, :], in_=wkr[:, 0:4, :])
                    nc.sync.dma_start(out=xt_sb[:, 1, :], in_=xr[:, 1, :])
                    nc.sync.dma_start(out=xt_sb[:, 2:4, :], in_=xr[:, 2:4, :])
                    nc.scalar.dma_start(out=wq_sb[:, 4:, :], in_=wqr[:, 4:, :])
                    nc.scalar.dma_start(out=wk_sb[:, 4:, :], in_=wkr[:, 4:, :])
                    nc.sync.dma_start(out=xt_sb[:, 4:8, :], in_=xr[:, 4:8, :])
                    nc.sync.dma_start(out=xt_sb[:, 8:12, :], in_=xr[:, 8:12, :])
                    nc.sync.dma_start(out=xt_sb[:, 12:16, :], in_=xr[:, 12:16, :])

                    allqk = [(dst, w, h, qt)
                             for dst, w in ((q_sb, wq_sb), (k_sb, wk_sb))
                             for h in range(c.HLOC)
                             for qt in range(c.QTN)]
                    # first four groups run kt-outer interleaved with the
                    # adapter accumulation so the PE issues 8 matmuls per
                    # arriving x chunk instead of 4
                    head_pools = [(pp, "pp"), (psp, "ps"), (psp, "ps"),
                                  (pop, "po")]
                    headacc = [head_pools[i][0].tile([128, c.QT], F32,
                                                     name=f"acc_qk{i}",
                                                     tag=head_pools[i][1])
                               for i in range(4)]
                    for kt in range(NKT):
                        for qt in range(c.QTN):
                            nc.tensor.matmul(
                                acc[qt][:, :],
                                wa_sb[:, kt, :],
                                xsl(kt, slice(qt * c.QT, (qt + 1) * c.QT)),
                                start=(kt == 0), stop=(kt == NKT - 1))
                        for i in range(4):
                            dst, w, h, qt = allqk[i]
                            nc.tensor.matmul(
                                headacc[i][:, :],
                                w[:, kt, h * 128:(h + 1) * 128],
                                xsl(kt, slice(qt * c.QT, (qt + 1) * c.QT)),
                                start=(kt == 0), stop=(kt == NKT - 1))
                    for qt in range(c.QTN):
                        nc.vector.tensor_copy(
                            aqk_sb[:, qt * c.QT:(qt + 1) * c.QT], acc[qt][:, :])
                    nc.sync.dma_start(out=ak_sb[:, :],
                                      in_=aqk_sb[c.RANK:2 * c.RANK, :])
                    for i in range(4):
                        dst, w, h, qt = allqk[i]
                        nc.scalar.copy(dst[h][:, qt * c.QT:(qt + 1) * c.QT],
                                       headacc[i][:, :])
                    for dst, w, h, qt in allqk[4:]:
                        psum = pp.tile([128, c.QT], F32, name="psum_qk", tag="pp")
                        for kt in range(NKT):
                            nc.tensor.matmul(
                                psum[:, :],
                                w[:, kt, h * 128:(h + 1) * 128],
                                xsl(kt, slice(qt * c.QT, (qt + 1) * c.QT)),
                                start=(kt == 0), stop=(kt == NKT - 1))
                        nc.scalar.copy(dst[h][:, qt * c.QT:(qt + 1) * c.QT],
                                       psum[:, :])

                # ---- v projection: [tok, d] natural, moving 512 wide ----
                # RoPE tiles are interleaved into the v loop: the rope chain
                # is DVE/GPSIMD-paced, the v matmuls keep the PE busy
                rope_tiles = [(tiles, h, qt)
                              for tiles in (q_sb, k_sb)
                              for h in range(c.HLOC)
                              for qt in range(c.QTN)]

                def rope_step(i):
                    # q/k head dims use a split re/im layout (host permutes
                    # wq/wk columns and the rope tables; scores are invariant
                    # to a shared d-permutation), so the pair-swap is a
                    # half-swap: two contiguous partition-range DMAs instead
                    # of a PE permute matmul.
                    tiles, h, qt = rope_tiles[i]
                    eng = nc.vector if i % 2 == 0 else nc.gpsimd
                    sl = slice(qt * c.QT, (qt + 1) * c.QT)
                    swp = rtp.tile([128, c.QT], BF16, name="rope_swp",
                                   tag=f"sw{i % 2}")
                    nc.sync.dma_start(out=swp[0:64, :], in_=tiles[h][64:128, sl])
                    nc.sync.dma_start(out=swp[64:128, :], in_=tiles[h][0:64, sl])
                    m1 = rtp.tile([128, c.QT], BF16, name="rope_m1",
                                  tag=f"m1{i % 2}")
                    m2 = rtp.tile([128, c.QT], BF16, name="rope_m2",
                                  tag=f"m2{i % 2}")
                    nc.gpsimd.tensor_mul(m1[:, :], tiles[h][:, sl], c2_sb[:, sl])
                    nc.vector.tensor_mul(m2[:, :], swp[:, :], s2_sb[:, sl])
                    eng.tensor_add(tiles[h][:, sl], m1[:, :], m2[:, :])

                with tc.tile_pool(name="wvp", bufs=1) as wvpool:
                    wv_sb = wvpool.tile([128, c.KTILES, c.DH], BF16,
                                        name="wv_sb", tag="wv")
                    wr = wvT.ap().rearrange("(t p) m -> p t m", p=128)
                    # fine first chunks: the first token-block's kt loop can
                    # start as soon as wv[0:4] lands
                    nc.sync.dma_start(out=wv_sb[:, 0:4, :], in_=wr[:, 0:4, :])
                    nc.sync.dma_start(out=wv_sb[:, 4:8, :], in_=wr[:, 4:8, :])
                    nc.sync.dma_start(out=wv_sb[:, 8:16, :], in_=wr[:, 8:16, :])
                    # rope tables arrive while the first v token-blocks run
                    nc.sync.dma_start(out=c2_sb, in_=c2d.ap())
                    nc.sync.dma_start(out=s2_sb, in_=s2d.ap())
                    for tt in range(c.S // 128):
                        psum = pp.tile([128, c.DH], F32, name="psum_v", tag="pp")
                        for kt in range(NKT):
                            nc.tensor.matmul(
                                psum[:, :],
                                xsl(kt, slice(tt * 128, (tt + 1) * 128)),
                                wv_sb[:, kt, :],
                                start=(kt == 0), stop=(kt == NKT - 1))
                        nc.scalar.copy(v_sb[:, tt, :], psum[:, :])
                        rope_step(2 * tt)
                        rope_step(2 * tt + 1)

            # ---- attention + output projection, per 512-query block ----
            with tc.tile_pool(name="wog", bufs=1) as wog, \
                    tc.tile_pool(name="wo_out", bufs=2) as wop:
                nc.sync.dma_start(out=mask_sb,
                                  in_=maskdd.ap().rearrange("j p q -> p j q"))
                woc_sb = wog.tile([128, c.HLOC, c.DIM], BF16, name="woc_sb", tag="woc")
                wcr = wocT.ap().rearrange("(h p) j -> p h j", p=128)
                nc.sync.dma_start(out=woc_sb[:, 0:2, :], in_=wcr[:, 0:2, :])
                nc.sync.dma_start(out=woc_sb[:, 2:4, :], in_=wcr[:, 2:4, :])
                # double-buffered per-qt gate tiles r = sigmoid(A)[k, q]
                rg_sb = [wog.tile([128, c.DIAG * c.QTN, c.QT], BF16,
                                  name=f"rg{i}_sb", tag=f"rg{i}") for i in range(2)]

                def gate_step(qt, kt):
                    # r = 1/(1+exp(-A)) via the (shared) Exp table
                    qsl = slice(qt * c.QT, (qt + 1) * c.QT)
                    ksl = slice(kt * c.KT, (kt + 1) * c.KT)
                    pga = psp.tile([128, c.QT], F32, name="pga", tag="ps")
                    nc.tensor.matmul(pga[:, :], ak_sb[:, ksl],
                                     aqk_sb[0:c.RANK, qsl],
                                     start=True, stop=True)
                    ge = pge.tile([128, c.QT], BF16, name="ge", tag="p")
                    nc.scalar.activation(ge[:, :], pga[:, :], AF.Exp, scale=sc_gate)
                    gt = gwk.tile([128, c.QT], F32, name="gt", tag="gt")
                    nc.vector.tensor_scalar_add(gt[:, :], ge[:, :], 1.0)
                    gr = gwk.tile([128, c.QT], F32, name="gr", tag="gr")
                    nc.vector.reciprocal_approx_fast(out=gr[:, :], in_=gt[:, :])
                    nc.gpsimd.tensor_copy(rg_sb[qt % 2][:, kt, :], gr[:, :])

                if c.use_gate and c.use_attn:
                    for kt in range(c.DIAG):
                        gate_step(0, kt)

                for qt in range(c.QTN):
                    qsl = slice(qt * c.QT, (qt + 1) * c.QT)
                    nkt = c.DIAG * (qt + 1)  # causal k tiles
                    rg = rg_sb[qt % 2]

                    og_sb = ogp.tile([128, c.HLOC, c.QT], BF16, name="og_sb", tag="og")
                    if not c.use_attn:
                        nc.vector.memset(og_sb, 0.0)

                    def normalize(h, po, prs):
                        if not c.use_rowsum:
                            nc.vector.tensor_copy(og_sb[:, h, :], po[:, :])
                            return
                        rr = gwk.tile([1, c.QT], F32, name="rr", tag="rr")
                        nc.vector.reciprocal_approx_fast(
                            out=rr[:, :], in_=prs[0:1, :])
                        rrh = gwk.tile([1, c.QT], F16, name="rrh", tag="rrh")
                        nc.vector.tensor_copy(rrh[:, :], rr[:, :])
                        # GPSIMD extended-ISA broadcast: partition 0 -> all
                        rbs = gwk.tile([128, c.QT], F16, name="rbs", tag="rbs")
                        nc.gpsimd.partition_broadcast(rbs[:, :], rrh[0:1, :])
                        nc.vector.tensor_mul(og_sb[:, h, :], po[:, :],
                                             rbs[:, :])

                    # single software pipeline over all (h, kt) score tiles:
                    # drains lag PIPE steps behind scores ACROSS head
                    # boundaries, so head-end drain tails overlap the next
                    # head's score matmuls; normalize(h) is emitted inline
                    # right after head h's last drain
                    po_t, prs_t = {}, {}
                    stage = []  # (h, kt, col-slice, p_or_pm_tile, pgm_tile)

                    def score_step(h, kt):
                        ksl = slice(kt * c.KT, (kt + 1) * c.KT)
                        j = kt - c.DIAG * qt
                        qoff = 128 * j if j > 0 else 0
                        s = slice(qoff, c.QT)
                        qs = slice(qt * c.QT + qoff, (qt + 1) * c.QT)
                        ps = psp.tile([128, c.QT], F32, name="ps", tag="ps")
                        nc.tensor.matmul(ps[:, s], k_sb[h][:, ksl],
                                         q_sb[h][:, qs], start=True, stop=True)
                        p_sb = pge.tile([128, c.QT], BF16, name="p_sb", tag="p")
                        nc.scalar.activation(p_sb[:, s], ps[:, s], AF.Exp,
                                             scale=sc_score)
                        if j >= 0:
                            # diagonal band: 0/1 causal mask after exp,
                            # in place (rowsum needs the masked pre-gate sum)
                            nc.vector.tensor_mul(p_sb[:, s], p_sb[:, s],
                                                 mask_sb[:, j, s])
                        if c.use_gate:
                            pgm = pge.tile([128, c.QT], BF16, name="pgm",
                                           tag="pgm")
                            nc.vector.tensor_mul(pgm[:, s], p_sb[:, s],
                                                 rg[:, kt, s])
                        else:
                            pgm = p_sb
                        stage.append((h, kt, s, p_sb, pgm))

                    def drain_step():
                        h, kt, s, p_sb, pgm = stage.pop(0)
                        # pre-gate rowsum (softmax denominator)
                        if c.use_rowsum:
                            nc.tensor.matmul(prs_t[h][0:1, s],
                                             ones_sb[:, :], p_sb[:, s],
                                             start=(kt == 0),
                                             stop=(kt == nkt - 1),
                                             skip_group_check=True)
                        # out_h^T[d, q] += v[k, d].T @ p_gated[k, q]
                        nc.tensor.matmul(po_t[h][:, s],
                                         v_sb[:, kt, h * 128:(h + 1) * 128],
                                         pgm[:, s],
                                         start=(kt == 0), stop=(kt == nkt - 1),
                                         skip_group_check=True)
                        if kt == nkt - 1:
                            normalize(h, po_t[h], prs_t[h])

                    for h in range(c.HLOC if c.use_attn else 0):
                        po_t[h] = pop.tile([128, c.QT], F32, name="po", tag="po")
                        prs_t[h] = pp.tile([1, c.QT], F32, name="prs", tag="pp")
                        for kt in range(nkt):
                            score_step(h, kt)
                            if len(stage) > c.PIPE:
                                drain_step()
                    while stage:
                        drain_step()

                    # output-projection partial for this query block,
                    # interleaved with the NEXT block's gate generation so the
                    # PE fills the gate chain's latency with wo matmuls
                    nkt2 = (c.DIAG * (qt + 2)
                            if (qt + 1 < c.QTN and c.use_gate and c.use_attn)
                            else 0)
                    ncha = c.DIM // 128
                    f_sb = wop.tile([128, ncha, c.QT], F16, name="f_sb", tag="f")
                    pfpools = [(pp, "pp"), (pop, "po")]
                    # two gate steps lead the wo loop: their matmuls keep the
                    # PE fed while the last head's normalize chain completes
                    for kt in range(min(2, nkt2)):
                        gate_step(qt + 1, kt)
                    for ch in range(ncha if c.use_wo else 0):
                        pfp, pft = pfpools[ch % 2]
                        pf = pfp.tile([128, c.QT], F32, name="pf", tag=pft)
                        for h in range(c.HLOC):
                            nc.tensor.matmul(
                                pf[:, :],
                                woc_sb[:, h, ch * 128:(ch + 1) * 128],
                                og_sb[:, h, :],
                                start=(h == 0), stop=(h == c.HLOC - 1))
                        if ch % 2 == 0:
                            nc.scalar.copy(f_sb[:, ch, :], pf[:, :])
                        else:
                            nc.vector.tensor_copy(f_sb[:, ch, :], pf[:, :])
                        if ch + 2 < nkt2:
                            gate_step(qt + 1, ch + 2)
                        if ch % 4 == 3:
                            # batched output DMA per 4 chunks (0.5MB each):
                            # early chunks fly while later ones compute
                            nc.sync.dma_start(
                                out=pout.ap().rearrange(
                                    "(ch p) q -> p ch q",
                                    p=128)[:, ch - 3:ch + 1, qsl],
                                in_=f_sb[:, ch - 3:ch + 1, :])
                    if not c.use_wo:
                        for kt in range(nkt2):
                            gate_step(qt + 1, kt)

        if c.repeat > 1:
            with tc.For_i(0, c.repeat, 1,
                          hint_engines=(mybir.EngineType.PE,
                                        mybir.EngineType.DVE,
                                        mybir.EngineType.Activation,
                                        mybir.EngineType.Pool,
                                        mybir.EngineType.SP)):
                body()
        else:
            body()

    nc.compile()
    return nc


def make_core_inputs(inputs: dict, cfg: Cfg = FULL):
    """Host-side sharding: returns in_maps (one dict per core)."""
    c = cfg
    bf16 = ml_dtypes.bfloat16
    x = np.asarray(inputs["x"])
    mask = np.asarray(inputs["mask"])
    fc = np.asarray(inputs["freqs_cos"])
    fs = np.asarray(inputs["freqs_sin"])
    wq, wk, wv, wo = (np.asarray(inputs[k]) for k in ("wq", "wk", "wv", "wo"))
    wa_q, wa_k = np.asarray(inputs["wa_q"]), np.asarray(inputs["wa_k"])

    xTb = [np.ascontiguousarray(x[b].T).astype(bf16) for b in range(c.B)]
    waT = np.ascontiguousarray(
        np.concatenate([wa_q, wa_k], axis=0).T).astype(bf16)

    # rope tables in [d, tok] layout, split re/im: rows 0:64 = re lanes,
    # 64:128 = im lanes (wq/wk columns are permuted to match; the score
    # dot-product is invariant to a shared head-dim permutation)
    c2 = np.empty((c.HD, c.S), np.float32)
    s2 = np.empty((c.HD, c.S), np.float32)
    c2[0:64] = fc.T
    c2[64:128] = fc.T
    s2[0:64] = -fs.T
    s2[64:128] = fs.T
    c2 = c2.astype(bf16)
    s2 = s2.astype(bf16)

    # per-head column permutation: even (re) dims first, odd (im) second
    dperm = np.concatenate([np.arange(0, c.HD, 2), np.arange(1, c.HD, 2)])
    qkperm = np.concatenate([hb * c.HD + dperm
                             for hb in range(c.DIM // c.HD)])

    # diagonal-band mask patterns [j][k, q], extracted from the input mask
    qt_last = c.QTN - 1
    q0 = qt_last * c.QT
    maskd = np.empty((c.DIAG, c.KT, c.QT), np.float32)
    for j in range(c.DIAG):
        k0 = (c.DIAG * qt_last + j) * c.KT
        maskd[j] = (mask[0, 0, q0:q0 + c.QT, k0:k0 + c.KT].T == 0.0)
    maskd = maskd.astype(bf16)

    wslices = []
    for hs in range(c.CPG):
        rows = slice(hs * c.DH, (hs + 1) * c.DH)
        wslices.append({
            "wqT": np.ascontiguousarray(wq[qkperm][rows].T).astype(bf16),
            "wkT": np.ascontiguousarray(wk[qkperm][rows].T).astype(bf16),
            "wvT": np.ascontiguousarray(wv[rows].T).astype(bf16),
            "wocT": np.ascontiguousarray(wo[:, rows].T).astype(bf16),
        })

    in_maps = []
    for ci in range(c.NCORES):
        b = ci // c.CPG
        hs = ci % c.CPG
        in_maps.append({
            "xT": xTb[b],
            **wslices[hs],
            "waT": waT,
            "c2d": c2,
            "s2d": s2,
            "maskdd": maskd,
        })
    return in_maps


def assemble_output(results, cfg: Cfg = FULL) -> np.ndarray:
    c = cfg
    out = np.empty((c.B, c.S, c.DIM), np.float32)
    for b in range(c.B):
        total = np.zeros((c.DIM, c.S), np.float32)
        for hs in range(c.CPG):
            total += np.asarray(results[b * c.CPG + hs]["pout"]).astype(np.float32)
        out[b] = total.T
    return out


_NC_CACHE = {}


def run(nc, in_maps, trace=False, cfg: Cfg = FULL, **kw):
    return bass_utils.run_bass_kernel_spmd(
        nc, in_maps, core_ids=list(range(cfg.NCORES)), trace=trace, **kw)


def kernel(**inputs) -> np.ndarray:
    cfg = FULL
    if cfg not in _NC_CACHE:
        _NC_CACHE[cfg] = build_nc(cfg)
    nc = _NC_CACHE[cfg]
    in_maps = make_core_inputs(inputs, cfg)
    res = run(nc, in_maps, cfg=cfg)
    return assemble_output(res.results, cfg)


if __name__ == "__main__":
    nc = build_nc(FULL)
    print("built ok")


# revision 18
# speedup vs baseline: 1.0116x; 1.0053x over previous
"""Trainium2 Bass kernel for nn_Attention_10771777978404 (sparse_attention).

Sharding over 8 NeuronCores: 2 batch-groups x 4 cores (tensor parallel over
heads within each batch group).
  - core ci handles batch ci//4 and heads [4*(ci%4), 4*(ci%4)+4): it computes
    its q/k/v projections (columns of wq/wk/wv), RoPE, causal attention with
    the low-rank sigmoid gate, and a full-width partial of the output
    projection from its 4 heads (rows of wo).
  - the rank-32 adapter weights are replicated inside each batch group; the
    sigmoid gate is computed as 1/(1+exp(-A)) so the scalar engine only ever
    uses the Exp table (no Sigmoid-table reloads, no DRAM staging).
  - host sums the 4 partial output projections per batch (fp16 partials).

Everything on-device is bf16 with fp32 PSUM accumulation.

Schedule notes (v2):
  - diagonal-band tiles only compute the live query columns [128j:512]
    (causal wedge), cutting PE/ACT/DVE work on the band by ~37%.
  - per-head softmax denominators live in ONE PSUM bank at partition
    offsets 32h, removing the head-boundary WAR on the rowsum accumulator.
  - the wo partial-projection PSUM tiles rotate over 4 banks (pp+po pools)
    and the PSUM->SBUF copies alternate ACT/DVE; the per-chunk output DMAs
    are batched into one 2MB DMA per query block (HWDGE descriptor
    generation is a serial ~630ns/dma resource).
  - input DMAs are coarsened and ordered by first-use so the HWDGE queue
    delivers x/wq/wk chunks at PE pace from the start.

self-contained: hardcodes the problem shapes; only needs `concourse` (on
PYTHONPATH in this container) + jax axon devices.
"""

import math
from contextlib import ExitStack
from dataclasses import dataclass

import numpy as np
import ml_dtypes

import concourse.tile as tile
from concourse import bacc
from concourse import mybir
from concourse import bass_utils

BF16 = mybir.dt.bfloat16
F16 = mybir.dt.float16
F32 = mybir.dt.float32
AF = mybir.ActivationFunctionType


@dataclass(frozen=True)
class Cfg:
    B: int = 2
    S: int = 2048
    DIM: int = 2048
    NH: int = 16
    HD: int = 128
    RANK: int = 32
    NCORES: int = 8
    GROUPS: int = 2     # batch groups of 4 cores
    QT: int = 512       # query block (free dim of score tiles)
    KT: int = 128       # key block (partition dim of score tiles)
    PIPE: int = 4       # score tiles in flight ahead of rowsum/AV drains
    repeat: int = 1     # hardware-loop repetitions of the whole body (timing)
    # ablation flags (profiling on hardware; all True for the real kernel)
    use_gate: bool = True
    use_rowsum: bool = True
    use_attn: bool = True
    use_wo: bool = True

    @property
    def CPG(self):
        return self.NCORES // self.GROUPS  # cores per batch group

    @property
    def HLOC(self):
        return self.NH // self.CPG  # heads per core (4)

    @property
    def DH(self):
        return self.HLOC * self.HD  # per-core head-dim span (512)

    @property
    def KTILES(self):
        return self.DIM // 128  # contraction tiles for projections

    @property
    def QTN(self):
        return self.S // self.QT

    @property
    def DIAG(self):
        return self.QT // self.KT  # k-tiles per diagonal band


FULL = Cfg()


def build_nc(cfg: Cfg = FULL):
    c = cfg
    assert c.HD == 128 and c.KT == 128
    nc = bacc.Bacc("TRN2", target_bir_lowering=False, debug=False,
                   num_devices=c.NCORES)

    # ---- kernel I/O (per core: one batch, HLOC heads) ----
    xT = nc.dram_tensor("xT", [c.DIM, c.S], BF16, kind="ExternalInput")
    wqT = nc.dram_tensor("wqT", [c.DIM, c.DH], BF16, kind="ExternalInput")
    wkT = nc.dram_tensor("wkT", [c.DIM, c.DH], BF16, kind="ExternalInput")
    wvT = nc.dram_tensor("wvT", [c.DIM, c.DH], BF16, kind="ExternalInput")
    wocT = nc.dram_tensor("wocT", [c.DH, c.DIM], BF16, kind="ExternalInput")
    waT = nc.dram_tensor("waT", [c.DIM, 2 * c.RANK], BF16, kind="ExternalInput")
    c2d = nc.dram_tensor("c2d", [c.HD, c.S], BF16, kind="ExternalInput")
    s2d = nc.dram_tensor("s2d", [c.HD, c.S], BF16, kind="ExternalInput")
    maskdd = nc.dram_tensor("maskdd", [c.DIAG, c.KT, c.QT], BF16, kind="ExternalInput")

    # partial output projection, transposed: pout[j, t] (fp16; host sums the
    # 4 partials of each batch group in fp32)
    pout = nc.dram_tensor("pout", [c.DIM, c.S], F16, kind="ExternalOutput")

    isqrt = 1.0 / math.sqrt(c.HD)
    sc_score = isqrt
    sc_gate = -1.0
    NKT = c.KTILES

    with ExitStack() as _ctx:
        tc = _ctx.enter_context(tile.TileContext(nc))
        # persistent pools (whole-iteration lifetime)
        cst = _ctx.enter_context(tc.tile_pool(name="const", bufs=1))
        adp = _ctx.enter_context(tc.tile_pool(name="ap", bufs=1))
        qkp = _ctx.enter_context(tc.tile_pool(name="qk", bufs=1))
        vp = _ctx.enter_context(tc.tile_pool(name="vp", bufs=1))
        rtp = _ctx.enter_context(tc.tile_pool(name="rope_t", bufs=1))
        pge = _ctx.enter_context(tc.tile_pool(name="pge", bufs=5))
        gwk = _ctx.enter_context(tc.tile_pool(name="gwk", bufs=2))
        ogp = _ctx.enter_context(tc.tile_pool(name="og", bufs=1))
        # PSUM pools (8 banks total): pp 2 + ps 4 + po 2; the per-head
        # softmax-denominator tiles and the norm-broadcast tiles share the
        # pp rotation so rowsum accumulation never WARs the previous head
        pp = _ctx.enter_context(tc.tile_pool(name="pp", bufs=2, space="PSUM"))
        psp = _ctx.enter_context(tc.tile_pool(name="ps", bufs=4, space="PSUM"))
        pop = _ctx.enter_context(tc.tile_pool(name="po", bufs=2, space="PSUM"))

        def body():
            # ---- constants ----
            c2_sb = cst.tile([128, c.S], BF16, name="c2_sb", tag="c2")
            s2_sb = cst.tile([128, c.S], BF16, name="s2_sb", tag="s2")
            ones_sb = cst.tile([128, 1], BF16, name="ones_sb", tag="ones")

            # packed adapter projections: one [64,512] matmul computes both
            # aq and ak rows (halves the adapter matmul count); ak is then
            # moved to partition base 0 with one SBUF->SBUF DMA (engines
            # cannot shift partitions, DMA can)
            aqk_sb = adp.tile([2 * c.RANK, c.S], BF16, name="aqk_sb", tag="aqk")
            ak_sb = adp.tile([c.RANK, c.S], BF16, name="ak_sb", tag="ak")
            q_sb = [qkp.tile([128, c.S], BF16, name=f"q{h}_sb", tag=f"q{h}")
                    for h in range(c.HLOC)]
            k_sb = [qkp.tile([128, c.S], BF16, name=f"k{h}_sb", tag=f"k{h}")
                    for h in range(c.HLOC)]
            v_sb = vp.tile([128, c.S // 128, c.DH], BF16, name="v_sb", tag="v")

            with tc.tile_pool(name="xtp", bufs=1) as xtp:
                wa_sb = xtp.tile([128, c.KTILES, 2 * c.RANK], BF16,
                                 name="wa_sb", tag="wa")
                nc.sync.dma_start(out=wa_sb,
                                  in_=waT.ap().rearrange("(t p) m -> p t m", p=128))
                xt_sb = xtp.tile([128, c.KTILES, c.S], BF16, name="xt_sb", tag="xt")
                xr = xT.ap().rearrange("(t p) n -> p t n", p=128)

                def xsl(j, sl):
                    return xt_sb[:, j, sl]

                nc.vector.memset(ones_sb, 1.0)

                # ---- adapter + q/k projections, kt-outer, 8 live psum accs ----
                apools = [pp, psp, pop, psp]
                acc = [apools[i].tile([2 * c.RANK, c.QT], F32, name=f"acc_a{i}",
                                      tag=("pp", "ps", "po", "ps")[i])
                       for i in range(c.QTN)]

                wvpool = None
                with tc.tile_pool(name="wqk", bufs=1) as wp:
                    wq_sb = wp.tile([128, c.KTILES, c.DH], BF16, name="wq_sb", tag="wq")
                    wk_sb = wp.tile([128, c.KTILES, c.DH], BF16, name="wk_sb", tag="wk")
                    # input DMAs ordered by first use; chunk sizes grow so the
                    # HWDGE queue stays ahead of the PE's kt-outer consumption
                    nc.sync.dma_start(out=xt_sb[:, 0, :], in_=xr[:, 0, :])
                    wqr = wqT.ap().rearrange("(t p) m -> p t m", p=128)
                    wkr = wkT.ap().rearrange("(t p) m -> p t m", p=128)
                    nc.scalar.dma_start(out=wq_sb[:, 0:4, :], in_=wqr[:, 0:4, :])
                    nc.scalar.dma_start(out=wk_sb[:, 0:4, :], in_=wkr[:, 0:4, :])
                    nc.sync.dma_start(out=xt_sb[:, 1, :], in_=xr[:, 1, :])
                    nc.sync.dma_start(out=xt_sb[:, 2:4, :], in_=xr[:, 2:4, :])
                    nc.scalar.dma_start(out=wq_sb[:, 4:, :], in_=wqr[:, 4:, :])
                    nc.scalar.dma_start(out=wk_sb[:, 4:, :], in_=wkr[:, 4:, :])
                    nc.sync.dma_start(out=xt_sb[:, 4:8, :], in_=xr[:, 4:8, :])
                    nc.sync.dma_start(out=xt_sb[:, 8:12, :], in_=xr[:, 8:12, :])
                    nc.sync.dma_start(out=xt_sb[:, 12:16, :], in_=xr[:, 12:16, :])

                    allqk = [(dst, w, h, qt)
                             for dst, w in ((q_sb, wq_sb), (k_sb, wk_sb))
                             for h in range(c.HLOC)
                             for qt in range(c.QTN)]
                    # first four groups run kt-outer interleaved with the
                    # adapter accumulation so the PE issues 8 matmuls per
                    # arriving x chunk instead of 4
                    head_pools = [(pp, "pp"), (psp, "ps"), (psp, "ps"),
                                  (pop, "po")]
                    headacc = [head_pools[i][0].tile([128, c.QT], F32,
                                                     name=f"acc_qk{i}",
                                                     tag=head_pools[i][1])
                               for i in range(4)]
                    for kt in range(NKT):
                        for qt in range(c.QTN):
                            nc.tensor.matmul(
                                acc[qt][:, :],
                                wa_sb[:, kt, :],
                                xsl(kt, slice(qt * c.QT, (qt + 1) * c.QT)),
                                start=(kt == 0), stop=(kt == NKT - 1))
                        for i in range(4):
                            dst, w, h, qt = allqk[i]
                            nc.tensor.matmul(
                                headacc[i][:, :],
                                w[:, kt, h * 128:(h + 1) * 128],
                                xsl(kt, slice(qt * c.QT, (qt + 1) * c.QT)),
                                start=(kt == 0), stop=(kt == NKT - 1))
                    for qt in range(c.QTN):
                        nc.vector.tensor_copy(
                            aqk_sb[:, qt * c.QT:(qt + 1) * c.QT], acc[qt][:, :])
                    nc.sync.dma_start(out=ak_sb[:, :],
                                      in_=aqk_sb[c.RANK:2 * c.RANK, :])
                    for i in range(4):
                        dst, w, h, qt = allqk[i]
                        nc.scalar.copy(dst[h][:, qt * c.QT:(qt + 1) * c.QT],
                                       headacc[i][:, :])
                    for dst, w, h, qt in allqk[4:]:
                        psum = pp.tile([128, c.QT], F32, name="psum_qk", tag="pp")
                        for kt in range(NKT):
                            nc.tensor.matmul(
                                psum[:, :],
                                w[:, kt, h * 128:(h + 1) * 128],
                                xsl(kt, slice(qt * c.QT, (qt + 1) * c.QT)),
                                start=(kt == 0), stop=(kt == NKT - 1))
                        nc.scalar.copy(dst[h][:, qt * c.QT:(qt + 1) * c.QT],
                                       psum[:, :])

                # ---- v projection: [tok, d] natural, moving 512 wide ----
                # RoPE tiles are interleaved into the v loop: the rope chain
                # is DVE/GPSIMD-paced, the v matmuls keep the PE busy
                rope_tiles = [(tiles, h, qt)
                              for tiles in (q_sb, k_sb)
                              for h in range(c.HLOC)
                              for qt in range(c.QTN)]

                def rope_step(i):
                    # q/k head dims use a split re/im layout (host permutes
                    # wq/wk columns and the rope tables; scores are invariant
                    # to a shared d-permutation), so the pair-swap is a
                    # half-swap: two contiguous partition-range DMAs instead
                    # of a PE permute matmul.
                    tiles, h, qt = rope_tiles[i]
                    eng = nc.vector if i % 2 == 0 else nc.gpsimd
                    sl = slice(qt * c.QT, (qt + 1) * c.QT)
                    swp = rtp.tile([128, c.QT], BF16, name="rope_swp",
                                   tag=f"sw{i % 2}")
                    nc.sync.dma_start(out=swp[0:64, :], in_=tiles[h][64:128, sl])
                    nc.sync.dma_start(out=swp[64:128, :], in_=tiles[h][0:64, sl])
                    m1 = rtp.tile([128, c.QT], BF16, name="rope_m1",
                                  tag=f"m1{i % 2}")
                    nc.gpsimd.tensor_mul(m1[:, :], tiles[h][:, sl], c2_sb[:, sl])
                    # sin term lands in place over the swapped copy
                    nc.vector.tensor_mul(swp[:, :], swp[:, :], s2_sb[:, sl])
                    eng.tensor_add(tiles[h][:, sl], m1[:, :], swp[:, :])

                with tc.tile_pool(name="wvp", bufs=1) as wvpool:
                    wv_sb = wvpool.tile([128, c.KTILES, c.DH], BF16,
                                        name="wv_sb", tag="wv")
                    wr = wvT.ap().rearrange("(t p) m -> p t m", p=128)
                    nc.sync.dma_start(out=wv_sb[:, 0:4, :], in_=wr[:, 0:4, :])
                    nc.sync.dma_start(out=wv_sb[:, 4:8, :], in_=wr[:, 4:8, :])
                    nc.sync.dma_start(out=wv_sb[:, 8:16, :], in_=wr[:, 8:16, :])
                    # rope tables arrive while the first v token-blocks run
                    nc.sync.dma_start(out=c2_sb, in_=c2d.ap())
                    nc.sync.dma_start(out=s2_sb, in_=s2d.ap())
                    for tt in range(c.S // 128):
                        psum = pp.tile([128, c.DH], F32, name="psum_v", tag="pp")
                        for kt in range(NKT):
                            nc.tensor.matmul(
                                psum[:, :],
                                xsl(kt, slice(tt * 128, (tt + 1) * 128)),
                                wv_sb[:, kt, :],
                                start=(kt == 0), stop=(kt == NKT - 1))
                        nc.scalar.copy(v_sb[:, tt, :], psum[:, :])
                        rope_step(2 * tt)
                        rope_step(2 * tt + 1)

            # ---- attention + output projection, per 512-query block ----
            with tc.tile_pool(name="wog", bufs=1) as wog, \
                    tc.tile_pool(name="wo_out", bufs=2) as wop:
                mask_sb = wog.tile([128, c.DIAG, c.QT], BF16, name="mask_sb",
                                   tag="mask")
                nc.sync.dma_start(out=mask_sb,
                                  in_=maskdd.ap().rearrange("j p q -> p j q"))
                woc_sb = wog.tile([128, c.HLOC, c.DIM], BF16, name="woc_sb", tag="woc")
                wcr = wocT.ap().rearrange("(h p) j -> p h j", p=128)
                nc.sync.dma_start(out=woc_sb[:, 0:2, :], in_=wcr[:, 0:2, :])
                nc.sync.dma_start(out=woc_sb[:, 2:4, :], in_=wcr[:, 2:4, :])
                # double-buffered per-qt gate tiles r = sigmoid(A)[k, q]
                rg_sb = [wog.tile([128, c.DIAG * c.QTN, c.QT], BF16,
                                  name=f"rg{i}_sb", tag=f"rg{i}") for i in range(2)]

                def gate_step(qt, kt):
                    # r = 1/(1+exp(-A)) via the (shared) Exp table
                    qsl = slice(qt * c.QT, (qt + 1) * c.QT)
                    ksl = slice(kt * c.KT, (kt + 1) * c.KT)
                    pga = psp.tile([128, c.QT], F32, name="pga", tag="ps")
                    nc.tensor.matmul(pga[:, :], ak_sb[:, ksl],
                                     aqk_sb[0:c.RANK, qsl],
                                     start=True, stop=True)
                    ge = pge.tile([128, c.QT], BF16, name="ge", tag="p")
                    nc.scalar.activation(ge[:, :], pga[:, :], AF.Exp, scale=sc_gate)
                    gt = gwk.tile([128, c.QT], F32, name="gt", tag="gt")
                    nc.vector.tensor_scalar_add(gt[:, :], ge[:, :], 1.0)
                    gr = gwk.tile([128, c.QT], F32, name="gr", tag="gr")
                    nc.vector.reciprocal_approx_fast(out=gr[:, :], in_=gt[:, :])
                    nc.gpsimd.tensor_copy(rg_sb[qt % 2][:, kt, :], gr[:, :])

                if c.use_gate and c.use_attn:
                    for kt in range(c.DIAG):
                        gate_step(0, kt)

                for qt in range(c.QTN):
                    qsl = slice(qt * c.QT, (qt + 1) * c.QT)
                    nkt = c.DIAG * (qt + 1)  # causal k tiles
                    rg = rg_sb[qt % 2]

                    og_sb = ogp.tile([128, c.HLOC, c.QT], BF16, name="og_sb", tag="og")
                    if not c.use_attn:
                        nc.vector.memset(og_sb, 0.0)

                    def normalize(h, po, prs):
                        if not c.use_rowsum:
                            nc.vector.tensor_copy(og_sb[:, h, :], po[:, :])
                            return
                        rr = gwk.tile([1, c.QT], F32, name="rr", tag="rr", bufs=1)
                        nc.vector.reciprocal_approx_fast(
                            out=rr[:, :], in_=prs[0:1, :])
                        rrh = gwk.tile([1, c.QT], F16, name="rrh", tag="rrh", bufs=1)
                        nc.vector.tensor_copy(rrh[:, :], rr[:, :])
                        # GPSIMD extended-ISA broadcast: partition 0 -> all
                        rbs = gwk.tile([128, c.QT], F16, name="rbs", tag="rbs", bufs=1)
                        nc.gpsimd.partition_broadcast(rbs[:, :], rrh[0:1, :])
                        nc.vector.tensor_mul(og_sb[:, h, :], po[:, :],
                                             rbs[:, :])

                    # single software pipeline over all (h, kt) score tiles:
                    # drains lag PIPE steps behind scores ACROSS head
                    # boundaries, so head-end drain tails overlap the next
                    # head's score matmuls; normalize(h) is emitted inline
                    # right after head h's last drain
                    po_t, prs_t = {}, {}
                    stage = []  # (h, kt, col-slice, p_or_pm_tile, pgm_tile)

                    def score_step(h, kt):
                        ksl = slice(kt * c.KT, (kt + 1) * c.KT)
                        j = kt - c.DIAG * qt
                        qoff = 128 * j if j > 0 else 0
                        s = slice(qoff, c.QT)
                        qs = slice(qt * c.QT + qoff, (qt + 1) * c.QT)
                        ps = psp.tile([128, c.QT], F32, name="ps", tag="ps")
                        nc.tensor.matmul(ps[:, s], k_sb[h][:, ksl],
                                         q_sb[h][:, qs], start=True, stop=True)
                        p_sb = pge.tile([128, c.QT], BF16, name="p_sb", tag="p")
                        nc.scalar.activation(p_sb[:, s], ps[:, s], AF.Exp,
                                             scale=sc_score)
                        if j >= 0:
                            # diagonal band: 0/1 causal mask after exp,
                            # in place (rowsum needs the masked pre-gate sum)
                            nc.vector.tensor_mul(p_sb[:, s], p_sb[:, s],
                                                 mask_sb[:, j, s])
                        if c.use_gate:
                            pgm = pge.tile([128, c.QT], BF16, name="pgm",
                                           tag="pgm")
                            nc.vector.tensor_mul(pgm[:, s], p_sb[:, s],
                                                 rg[:, kt, s])
                        else:
                            pgm = p_sb
                        stage.append((h, kt, s, p_sb, pgm))

                    def drain_step():
                        h, kt, s, p_sb, pgm = stage.pop(0)
                        # pre-gate rowsum (softmax denominator)
                        if c.use_rowsum:
                            nc.tensor.matmul(prs_t[h][0:1, s],
                                             ones_sb[:, :], p_sb[:, s],
                                             start=(kt == 0),
                                             stop=(kt == nkt - 1),
                                             skip_group_check=True)
                        # out_h^T[d, q] += v[k, d].T @ p_gated[k, q]
                        nc.tensor.matmul(po_t[h][:, s],
                                         v_sb[:, kt, h * 128:(h + 1) * 128],
                                         pgm[:, s],
                                         start=(kt == 0), stop=(kt == nkt - 1),
                                         skip_group_check=True)
                        if kt == nkt - 1:
                            normalize(h, po_t[h], prs_t[h])

                    for h in range(c.HLOC if c.use_attn else 0):
                        po_t[h] = pop.tile([128, c.QT], F32, name="po", tag="po")
                        prs_t[h] = pp.tile([1, c.QT], F32, name="prs", tag="pp")
                        for kt in range(nkt):
                            score_step(h, kt)
                            if len(stage) > c.PIPE:
                                drain_step()
                    while stage:
                        drain_step()

                    # output-projection partial for this query block,
                    # interleaved with the NEXT block's gate generation so the
                    # PE fills the gate chain's latency with wo matmuls
                    nkt2 = (c.DIAG * (qt + 2)
                            if (qt + 1 < c.QTN and c.use_gate and c.use_attn)
                            else 0)
                    ncha = c.DIM // 128
                    f_sb = wop.tile([128, ncha, c.QT], F16, name="f_sb", tag="f")
                    pfpools = [(pp, "pp"), (pop, "po")]
                    # two gate steps lead the wo loop: their matmuls keep the
                    # PE fed while the last head's normalize chain completes
                    for kt in range(min(2, nkt2)):
                        gate_step(qt + 1, kt)
                    for ch in range(ncha if c.use_wo else 0):
                        pfp, pft = pfpools[ch % 2]
                        pf = pfp.tile([128, c.QT], F32, name="pf", tag=pft)
                        for h in range(c.HLOC):
                            nc.tensor.matmul(
                                pf[:, :],
                                woc_sb[:, h, ch * 128:(ch + 1) * 128],
                                og_sb[:, h, :],
                                start=(h == 0), stop=(h == c.HLOC - 1))
                        if ch % 2 == 0:
                            nc.scalar.copy(f_sb[:, ch, :], pf[:, :])
                        else:
                            nc.vector.tensor_copy(f_sb[:, ch, :], pf[:, :])
                        if ch + 2 < nkt2:
                            gate_step(qt + 1, ch + 2)
                        if ch % 4 == 3:
                            # batched output DMA per 4 chunks (0.5MB each):
                            # early chunks fly while later ones compute
                            nc.sync.dma_start(
                                out=pout.ap().rearrange(
                                    "(ch p) q -> p ch q",
                                    p=128)[:, ch - 3:ch + 1, qsl],
                                in_=f_sb[:, ch - 3:ch + 1, :])
                    if not c.use_wo:
                        for kt in range(nkt2):
                            gate_step(qt + 1, kt)

        if c.repeat > 1:
            with tc.For_i(0, c.repeat, 1,
                          hint_engines=(mybir.EngineType.PE,
                                        mybir.EngineType.DVE,
                                        mybir.EngineType.Activation,
                                        mybir.EngineType.Pool,
                                        mybir.EngineType.SP)):
                body()
        else:
            body()

    nc.compile()
    return nc


def make_core_inputs(inputs: dict, cfg: Cfg = FULL):
    """Host-side sharding: returns in_maps (one dict per core)."""
    c = cfg
    bf16 = ml_dtypes.bfloat16
    x = np.asarray(inputs["x"])
    mask = np.asarray(inputs["mask"])
    fc = np.asarray(inputs["freqs_cos"])
    fs = np.asarray(inputs["freqs_sin"])
    wq, wk, wv, wo = (np.asarray(inputs[k]) for k in ("wq", "wk", "wv", "wo"))
    wa_q, wa_k = np.asarray(inputs["wa_q"]), np.asarray(inputs["wa_k"])

    xTb = [np.ascontiguousarray(x[b].T).astype(bf16) for b in range(c.B)]
    waT = np.ascontiguousarray(
        np.concatenate([wa_q, wa_k], axis=0).T).astype(bf16)

    # rope tables in [d, tok] layout, split re/im: rows 0:64 = re lanes,
    # 64:128 = im lanes (wq/wk columns are permuted to match; the score
    # dot-product is invariant to a shared head-dim permutation)
    c2 = np.empty((c.HD, c.S), np.float32)
    s2 = np.empty((c.HD, c.S), np.float32)
    c2[0:64] = fc.T
    c2[64:128] = fc.T
    s2[0:64] = -fs.T
    s2[64:128] = fs.T
    c2 = c2.astype(bf16)
    s2 = s2.astype(bf16)

    # per-head column permutation: even (re) dims first, odd (im) second
    dperm = np.concatenate([np.arange(0, c.HD, 2), np.arange(1, c.HD, 2)])
    qkperm = np.concatenate([hb * c.HD + dperm
                             for hb in range(c.DIM // c.HD)])

    # diagonal-band mask patterns [j][k, q], extracted from the input mask
    qt_last = c.QTN - 1
    q0 = qt_last * c.QT
    maskd = np.empty((c.DIAG, c.KT, c.QT), np.float32)
    for j in range(c.DIAG):
        k0 = (c.DIAG * qt_last + j) * c.KT
        maskd[j] = (mask[0, 0, q0:q0 + c.QT, k0:k0 + c.KT].T == 0.0)
    maskd = maskd.astype(bf16)

    wslices = []
    for hs in range(c.CPG):
        rows = slice(hs * c.DH, (hs + 1) * c.DH)
        wslices.append({
            "wqT": np.ascontiguousarray(wq[qkperm][rows].T).astype(bf16),
            "wkT": np.ascontiguousarray(wk[qkperm][rows].T).astype(bf16),
            "wvT": np.ascontiguousarray(wv[rows].T).astype(bf16),
            "wocT": np.ascontiguousarray(wo[:, rows].T).astype(bf16),
        })

    in_maps = []
    for ci in range(c.NCORES):
        b = ci // c.CPG
        hs = ci % c.CPG
        in_maps.append({
            "xT": xTb[b],
            **wslices[hs],
            "waT": waT,
            "c2d": c2,
            "s2d": s2,
            "maskdd": maskd,
        })
    return in_maps


def assemble_output(results, cfg: Cfg = FULL) -> np.ndarray:
    c = cfg
    out = np.empty((c.B, c.S, c.DIM), np.float32)
    for b in range(c.B):
        total = np.zeros((c.DIM, c.S), np.float32)
        for hs in range(c.CPG):
            total += np.asarray(results[b * c.CPG + hs]["pout"]).astype(np.float32)
        out[b] = total.T
    return out


_NC_CACHE = {}


def run(nc, in_maps, trace=False, cfg: Cfg = FULL, **kw):
    return bass_utils.run_bass_kernel_spmd(
        nc, in_maps, core_ids=list(range(cfg.NCORES)), trace=trace, **kw)


def kernel(**inputs) -> np.ndarray:
    cfg = FULL
    if cfg not in _NC_CACHE:
        _NC_CACHE[cfg] = build_nc(cfg)
    nc = _NC_CACHE[cfg]
    in_maps = make_core_inputs(inputs, cfg)
    res = run(nc, in_maps, cfg=cfg)
    return assemble_output(res.results, cfg)


if __name__ == "__main__":
    nc = build_nc(FULL)
    print("built ok")


# revision 19
# speedup vs baseline: 1.0141x; 1.0024x over previous
"""Trainium2 Bass kernel for nn_Attention_10771777978404 (sparse_attention).

Sharding over 8 NeuronCores: 2 batch-groups x 4 cores (tensor parallel over
heads within each batch group).
  - core ci handles batch ci//4 and heads [4*(ci%4), 4*(ci%4)+4): it computes
    its q/k/v projections (columns of wq/wk/wv), RoPE, causal attention with
    the low-rank sigmoid gate, and a full-width partial of the output
    projection from its 4 heads (rows of wo).
  - the rank-32 adapter weights are replicated inside each batch group; the
    sigmoid gate is computed as 1/(1+exp(-A)) so the scalar engine only ever
    uses the Exp table (no Sigmoid-table reloads, no DRAM staging).
  - host sums the 4 partial output projections per batch (fp16 partials).

Everything on-device is bf16 with fp32 PSUM accumulation.

Schedule notes (v2):
  - diagonal-band tiles only compute the live query columns [128j:512]
    (causal wedge), cutting PE/ACT/DVE work on the band by ~37%.
  - per-head softmax denominators live in ONE PSUM bank at partition
    offsets 32h, removing the head-boundary WAR on the rowsum accumulator.
  - the wo partial-projection PSUM tiles rotate over 4 banks (pp+po pools)
    and the PSUM->SBUF copies alternate ACT/DVE; the per-chunk output DMAs
    are batched into one 2MB DMA per query block (HWDGE descriptor
    generation is a serial ~630ns/dma resource).
  - input DMAs are coarsened and ordered by first-use so the HWDGE queue
    delivers x/wq/wk chunks at PE pace from the start.

self-contained: hardcodes the problem shapes; only needs `concourse` (on
PYTHONPATH in this container) + jax axon devices.
"""

import math
from contextlib import ExitStack
from dataclasses import dataclass

import numpy as np
import ml_dtypes

import concourse.tile as tile
from concourse import bacc
from concourse import mybir
from concourse import bass_utils

BF16 = mybir.dt.bfloat16
F16 = mybir.dt.float16
F32 = mybir.dt.float32
AF = mybir.ActivationFunctionType


@dataclass(frozen=True)
class Cfg:
    B: int = 2
    S: int = 2048
    DIM: int = 2048
    NH: int = 16
    HD: int = 128
    RANK: int = 32
    NCORES: int = 8
    GROUPS: int = 2     # batch groups of 4 cores
    QT: int = 512       # query block (free dim of score tiles)
    KT: int = 128       # key block (partition dim of score tiles)
    PIPE: int = 4       # score tiles in flight ahead of rowsum/AV drains
    repeat: int = 1     # hardware-loop repetitions of the whole body (timing)
    # ablation flags (profiling on hardware; all True for the real kernel)
    use_gate: bool = True
    use_rowsum: bool = True
    use_attn: bool = True
    use_wo: bool = True

    @property
    def CPG(self):
        return self.NCORES // self.GROUPS  # cores per batch group

    @property
    def HLOC(self):
        return self.NH // self.CPG  # heads per core (4)

    @property
    def DH(self):
        return self.HLOC * self.HD  # per-core head-dim span (512)

    @property
    def KTILES(self):
        return self.DIM // 128  # contraction tiles for projections

    @property
    def QTN(self):
        return self.S // self.QT

    @property
    def DIAG(self):
        return self.QT // self.KT  # k-tiles per diagonal band


FULL = Cfg()


def build_nc(cfg: Cfg = FULL):
    c = cfg
    assert c.HD == 128 and c.KT == 128
    nc = bacc.Bacc("TRN2", target_bir_lowering=False, debug=False,
                   num_devices=c.NCORES)

    # ---- kernel I/O (per core: one batch, HLOC heads) ----
    xT = nc.dram_tensor("xT", [c.DIM, c.S], BF16, kind="ExternalInput")
    wqT = nc.dram_tensor("wqT", [c.DIM, c.DH], BF16, kind="ExternalInput")
    wkT = nc.dram_tensor("wkT", [c.DIM, c.DH], BF16, kind="ExternalInput")
    wvT = nc.dram_tensor("wvT", [c.DIM, c.DH], BF16, kind="ExternalInput")
    wocT = nc.dram_tensor("wocT", [c.DH, c.DIM], BF16, kind="ExternalInput")
    waT = nc.dram_tensor("waT", [c.DIM, 2 * c.RANK], BF16, kind="ExternalInput")
    c2d = nc.dram_tensor("c2d", [c.HD, c.S], BF16, kind="ExternalInput")
    s2d = nc.dram_tensor("s2d", [c.HD, c.S], BF16, kind="ExternalInput")
    maskdd = nc.dram_tensor("maskdd", [c.DIAG, c.KT, c.QT], BF16, kind="ExternalInput")

    # partial output projection, transposed: pout[j, t] (fp16; host sums the
    # 4 partials of each batch group in fp32)
    pout = nc.dram_tensor("pout", [c.DIM, c.S], F16, kind="ExternalOutput")

    isqrt = 1.0 / math.sqrt(c.HD)
    sc_score = isqrt
    sc_gate = -1.0
    NKT = c.KTILES

    with ExitStack() as _ctx:
        tc = _ctx.enter_context(tile.TileContext(nc))
        # persistent pools (whole-iteration lifetime)
        cst = _ctx.enter_context(tc.tile_pool(name="const", bufs=1))
        adp = _ctx.enter_context(tc.tile_pool(name="ap", bufs=1))
        qkp = _ctx.enter_context(tc.tile_pool(name="qk", bufs=1))
        vp = _ctx.enter_context(tc.tile_pool(name="vp", bufs=1))
        rtp = _ctx.enter_context(tc.tile_pool(name="rope_t", bufs=1))
        pge = _ctx.enter_context(tc.tile_pool(name="pge", bufs=5))
        gwk = _ctx.enter_context(tc.tile_pool(name="gwk", bufs=2))
        ogp = _ctx.enter_context(tc.tile_pool(name="og", bufs=1))
        # PSUM pools (8 banks total): pp 2 + ps 4 + po 2; the per-head
        # softmax-denominator tiles and the norm-broadcast tiles share the
        # pp rotation so rowsum accumulation never WARs the previous head
        pp = _ctx.enter_context(tc.tile_pool(name="pp", bufs=2, space="PSUM"))
        psp = _ctx.enter_context(tc.tile_pool(name="ps", bufs=4, space="PSUM"))
        pop = _ctx.enter_context(tc.tile_pool(name="po", bufs=2, space="PSUM"))

        def body():
            # ---- constants ----
            c2_sb = cst.tile([128, c.S], BF16, name="c2_sb", tag="c2")
            s2_sb = cst.tile([128, c.S], BF16, name="s2_sb", tag="s2")
            ones_sb = cst.tile([128, 1], BF16, name="ones_sb", tag="ones")

            # packed adapter projections: one [64,512] matmul computes both
            # aq and ak rows (halves the adapter matmul count); ak is then
            # moved to partition base 0 with one SBUF->SBUF DMA (engines
            # cannot shift partitions, DMA can)
            aqk_sb = adp.tile([2 * c.RANK, c.S], BF16, name="aqk_sb", tag="aqk")
            ak_sb = adp.tile([c.RANK, c.S], BF16, name="ak_sb", tag="ak")
            q_sb = [qkp.tile([128, c.S], BF16, name=f"q{h}_sb", tag=f"q{h}")
                    for h in range(c.HLOC)]
            k_sb = [qkp.tile([128, c.S], BF16, name=f"k{h}_sb", tag=f"k{h}")
                    for h in range(c.HLOC)]
            v_sb = vp.tile([128, c.S // 128, c.DH], BF16, name="v_sb", tag="v")

            with tc.tile_pool(name="xtp", bufs=1) as xtp:
                wa_sb = xtp.tile([128, c.KTILES, 2 * c.RANK], BF16,
                                 name="wa_sb", tag="wa")
                nc.sync.dma_start(out=wa_sb,
                                  in_=waT.ap().rearrange("(t p) m -> p t m", p=128))
                xt_sb = xtp.tile([128, c.KTILES, c.S], BF16, name="xt_sb", tag="xt")
                xr = xT.ap().rearrange("(t p) n -> p t n", p=128)

                def xsl(j, sl):
                    return xt_sb[:, j, sl]

                nc.vector.memset(ones_sb, 1.0)

                # ---- adapter + q/k projections, kt-outer, 8 live psum accs ----
                apools = [pp, psp, pop, psp]
                acc = [apools[i].tile([2 * c.RANK, c.QT], F32, name=f"acc_a{i}",
                                      tag=("pp", "ps", "po", "ps")[i])
                       for i in range(c.QTN)]

                wvpool = None
                with tc.tile_pool(name="wqk", bufs=1) as wp:
                    wq_sb = wp.tile([128, c.KTILES, c.DH], BF16, name="wq_sb", tag="wq")
                    wk_sb = wp.tile([128, c.KTILES, c.DH], BF16, name="wk_sb", tag="wk")
                    # input DMAs ordered by first use; the adapter matmuls
                    # lead the head matmuls by 2 kt steps, so the sweep can
                    # start on x alone while wq/wk stream in behind
                    nc.sync.dma_start(out=xt_sb[:, 0, :], in_=xr[:, 0, :])
                    nc.sync.dma_start(out=xt_sb[:, 1, :], in_=xr[:, 1, :])
                    wqr = wqT.ap().rearrange("(t p) m -> p t m", p=128)
                    wkr = wkT.ap().rearrange("(t p) m -> p t m", p=128)
                    nc.scalar.dma_start(out=wq_sb[:, 0:2, :], in_=wqr[:, 0:2, :])
                    nc.scalar.dma_start(out=wk_sb[:, 0:2, :], in_=wkr[:, 0:2, :])
                    nc.sync.dma_start(out=xt_sb[:, 2:4, :], in_=xr[:, 2:4, :])
                    nc.scalar.dma_start(out=wq_sb[:, 2:6, :], in_=wqr[:, 2:6, :])
                    nc.scalar.dma_start(out=wk_sb[:, 2:6, :], in_=wkr[:, 2:6, :])
                    nc.sync.dma_start(out=xt_sb[:, 4:8, :], in_=xr[:, 4:8, :])
                    nc.scalar.dma_start(out=wq_sb[:, 6:, :], in_=wqr[:, 6:, :])
                    nc.scalar.dma_start(out=wk_sb[:, 6:, :], in_=wkr[:, 6:, :])
                    nc.sync.dma_start(out=xt_sb[:, 8:12, :], in_=xr[:, 8:12, :])
                    nc.sync.dma_start(out=xt_sb[:, 12:16, :], in_=xr[:, 12:16, :])

                    allqk = [(dst, w, h, qt)
                             for dst, w in ((q_sb, wq_sb), (k_sb, wk_sb))
                             for h in range(c.HLOC)
                             for qt in range(c.QTN)]
                    # first four groups run kt-outer interleaved with the
                    # adapter accumulation so the PE issues 8 matmuls per
                    # arriving x chunk instead of 4
                    head_pools = [(pp, "pp"), (psp, "ps"), (psp, "ps"),
                                  (pop, "po")]
                    headacc = [head_pools[i][0].tile([128, c.QT], F32,
                                                     name=f"acc_qk{i}",
                                                     tag=head_pools[i][1])
                               for i in range(4)]
                    LAG = 2
                    for s in range(NKT + LAG):
                        if s < NKT:
                            for qt in range(c.QTN):
                                nc.tensor.matmul(
                                    acc[qt][:, :],
                                    wa_sb[:, s, :],
                                    xsl(s, slice(qt * c.QT, (qt + 1) * c.QT)),
                                    start=(s == 0), stop=(s == NKT - 1))
                        if s >= LAG:
                            kt = s - LAG
                            for i in range(4):
                                dst, w, h, qt = allqk[i]
                                nc.tensor.matmul(
                                    headacc[i][:, :],
                                    w[:, kt, h * 128:(h + 1) * 128],
                                    xsl(kt, slice(qt * c.QT, (qt + 1) * c.QT)),
                                    start=(kt == 0), stop=(kt == NKT - 1))
                    for qt in range(c.QTN):
                        nc.vector.tensor_copy(
                            aqk_sb[:, qt * c.QT:(qt + 1) * c.QT], acc[qt][:, :])
                    nc.sync.dma_start(out=ak_sb[:, :],
                                      in_=aqk_sb[c.RANK:2 * c.RANK, :])
                    for i in range(4):
                        dst, w, h, qt = allqk[i]
                        nc.scalar.copy(dst[h][:, qt * c.QT:(qt + 1) * c.QT],
                                       headacc[i][:, :])
                    for dst, w, h, qt in allqk[4:]:
                        psum = pp.tile([128, c.QT], F32, name="psum_qk", tag="pp")
                        for kt in range(NKT):
                            nc.tensor.matmul(
                                psum[:, :],
                                w[:, kt, h * 128:(h + 1) * 128],
                                xsl(kt, slice(qt * c.QT, (qt + 1) * c.QT)),
                                start=(kt == 0), stop=(kt == NKT - 1))
                        nc.scalar.copy(dst[h][:, qt * c.QT:(qt + 1) * c.QT],
                                       psum[:, :])

                # ---- v projection: [tok, d] natural, moving 512 wide ----
                # RoPE tiles are interleaved into the v loop: the rope chain
                # is DVE/GPSIMD-paced, the v matmuls keep the PE busy
                rope_tiles = [(tiles, h, qt)
                              for tiles in (q_sb, k_sb)
                              for h in range(c.HLOC)
                              for qt in range(c.QTN)]

                def rope_step(i):
                    # q/k head dims use a split re/im layout (host permutes
                    # wq/wk columns and the rope tables; scores are invariant
                    # to a shared d-permutation), so the pair-swap is a
                    # half-swap: two contiguous partition-range DMAs instead
                    # of a PE permute matmul.
                    tiles, h, qt = rope_tiles[i]
                    eng = nc.vector if i % 2 == 0 else nc.gpsimd
                    sl = slice(qt * c.QT, (qt + 1) * c.QT)
                    swp = rtp.tile([128, c.QT], BF16, name="rope_swp",
                                   tag=f"sw{i % 2}")
                    nc.sync.dma_start(out=swp[0:64, :], in_=tiles[h][64:128, sl])
                    nc.sync.dma_start(out=swp[64:128, :], in_=tiles[h][0:64, sl])
                    m1 = rtp.tile([128, c.QT], BF16, name="rope_m1",
                                  tag=f"m1{i % 2}")
                    nc.gpsimd.tensor_mul(m1[:, :], tiles[h][:, sl], c2_sb[:, sl])
                    # sin term lands in place over the swapped copy
                    nc.vector.tensor_mul(swp[:, :], swp[:, :], s2_sb[:, sl])
                    eng.tensor_add(tiles[h][:, sl], m1[:, :], swp[:, :])

                with tc.tile_pool(name="wvp", bufs=1) as wvpool:
                    wv_sb = wvpool.tile([128, c.KTILES, c.DH], BF16,
                                        name="wv_sb", tag="wv")
                    wr = wvT.ap().rearrange("(t p) m -> p t m", p=128)
                    nc.sync.dma_start(out=wv_sb[:, 0:4, :], in_=wr[:, 0:4, :])
                    nc.sync.dma_start(out=wv_sb[:, 4:8, :], in_=wr[:, 4:8, :])
                    nc.sync.dma_start(out=wv_sb[:, 8:16, :], in_=wr[:, 8:16, :])
                    # rope tables arrive while the first v token-blocks run
                    nc.sync.dma_start(out=c2_sb, in_=c2d.ap())
                    nc.sync.dma_start(out=s2_sb, in_=s2d.ap())
                    for tt in range(c.S // 128):
                        psum = pp.tile([128, c.DH], F32, name="psum_v", tag="pp")
                        for kt in range(NKT):
                            nc.tensor.matmul(
                                psum[:, :],
                                xsl(kt, slice(tt * 128, (tt + 1) * 128)),
                                wv_sb[:, kt, :],
                                start=(kt == 0), stop=(kt == NKT - 1))
                        nc.scalar.copy(v_sb[:, tt, :], psum[:, :])
                        rope_step(2 * tt)
                        rope_step(2 * tt + 1)

            # ---- attention + output projection, per 512-query block ----
            with tc.tile_pool(name="wog", bufs=1) as wog, \
                    tc.tile_pool(name="wo_out", bufs=2) as wop:
                mask_sb = wog.tile([128, c.DIAG, c.QT], BF16, name="mask_sb",
                                   tag="mask")
                nc.sync.dma_start(out=mask_sb,
                                  in_=maskdd.ap().rearrange("j p q -> p j q"))
                woc_sb = wog.tile([128, c.HLOC, c.DIM], BF16, name="woc_sb", tag="woc")
                wcr = wocT.ap().rearrange("(h p) j -> p h j", p=128)
                nc.sync.dma_start(out=woc_sb[:, 0:2, :], in_=wcr[:, 0:2, :])
                nc.sync.dma_start(out=woc_sb[:, 2:4, :], in_=wcr[:, 2:4, :])
                # double-buffered per-qt gate tiles r = sigmoid(A)[k, q]
                rg_sb = [wog.tile([128, c.DIAG * c.QTN, c.QT], BF16,
                                  name=f"rg{i}_sb", tag=f"rg{i}") for i in range(2)]

                def gate_step(qt, kt):
                    # r = 1/(1+exp(-A)) via the (shared) Exp table
                    qsl = slice(qt * c.QT, (qt + 1) * c.QT)
                    ksl = slice(kt * c.KT, (kt + 1) * c.KT)
                    pga = psp.tile([128, c.QT], F32, name="pga", tag="ps")
                    nc.tensor.matmul(pga[:, :], ak_sb[:, ksl],
                                     aqk_sb[0:c.RANK, qsl],
                                     start=True, stop=True)
                    ge = pge.tile([128, c.QT], BF16, name="ge", tag="p")
                    nc.scalar.activation(ge[:, :], pga[:, :], AF.Exp, scale=sc_gate)
                    gt = gwk.tile([128, c.QT], F32, name="gt", tag="gt")
                    nc.vector.tensor_scalar_add(gt[:, :], ge[:, :], 1.0)
                    gr = gwk.tile([128, c.QT], F32, name="gr", tag="gr")
                    nc.vector.reciprocal_approx_fast(out=gr[:, :], in_=gt[:, :])
                    nc.gpsimd.tensor_copy(rg_sb[qt % 2][:, kt, :], gr[:, :])

                if c.use_gate and c.use_attn:
                    for kt in range(c.DIAG):
                        gate_step(0, kt)

                for qt in range(c.QTN):
                    qsl = slice(qt * c.QT, (qt + 1) * c.QT)
                    nkt = c.DIAG * (qt + 1)  # causal k tiles
                    rg = rg_sb[qt % 2]

                    og_sb = ogp.tile([128, c.HLOC, c.QT], BF16, name="og_sb", tag="og")
                    if not c.use_attn:
                        nc.vector.memset(og_sb, 0.0)

                    def normalize(h, po, prs):
                        if not c.use_rowsum:
                            nc.vector.tensor_copy(og_sb[:, h, :], po[:, :])
                            return
                        rr = gwk.tile([1, c.QT], F32, name="rr", tag="rr", bufs=1)
                        nc.vector.reciprocal_approx_fast(
                            out=rr[:, :], in_=prs[0:1, :])
                        rrh = gwk.tile([1, c.QT], F16, name="rrh", tag="rrh", bufs=1)
                        nc.vector.tensor_copy(rrh[:, :], rr[:, :])
                        # GPSIMD extended-ISA broadcast: partition 0 -> all
                        rbs = gwk.tile([128, c.QT], F16, name="rbs", tag="rbs", bufs=1)
                        nc.gpsimd.partition_broadcast(rbs[:, :], rrh[0:1, :])
                        nc.vector.tensor_mul(og_sb[:, h, :], po[:, :],
                                             rbs[:, :])

                    # single software pipeline over all (h, kt) score tiles:
                    # drains lag PIPE steps behind scores ACROSS head
                    # boundaries, so head-end drain tails overlap the next
                    # head's score matmuls; normalize(h) is emitted inline
                    # right after head h's last drain
                    po_t, prs_t = {}, {}
                    stage = []  # (h, kt, col-slice, p_or_pm_tile, pgm_tile)

                    def score_step(h, kt):
                        ksl = slice(kt * c.KT, (kt + 1) * c.KT)
                        j = kt - c.DIAG * qt
                        qoff = 128 * j if j > 0 else 0
                        s = slice(qoff, c.QT)
                        qs = slice(qt * c.QT + qoff, (qt + 1) * c.QT)
                        ps = psp.tile([128, c.QT], F32, name="ps", tag="ps")
                        nc.tensor.matmul(ps[:, s], k_sb[h][:, ksl],
                                         q_sb[h][:, qs], start=True, stop=True)
                        p_sb = pge.tile([128, c.QT], BF16, name="p_sb", tag="p")
                        nc.scalar.activation(p_sb[:, s], ps[:, s], AF.Exp,
                                             scale=sc_score)
                        if j >= 0:
                            # diagonal band: 0/1 causal mask after exp,
                            # in place (rowsum needs the masked pre-gate sum)
                            nc.vector.tensor_mul(p_sb[:, s], p_sb[:, s],
                                                 mask_sb[:, j, s])
                        if c.use_gate:
                            pgm = pge.tile([128, c.QT], BF16, name="pgm",
                                           tag="pgm")
                            nc.vector.tensor_mul(pgm[:, s], p_sb[:, s],
                                                 rg[:, kt, s])
                        else:
                            pgm = p_sb
                        stage.append((h, kt, s, p_sb, pgm))

                    def drain_step():
                        h, kt, s, p_sb, pgm = stage.pop(0)
                        # pre-gate rowsum (softmax denominator)
                        if c.use_rowsum:
                            nc.tensor.matmul(prs_t[h][0:1, s],
                                             ones_sb[:, :], p_sb[:, s],
                                             start=(kt == 0),
                                             stop=(kt == nkt - 1),
                                             skip_group_check=True)
                        # out_h^T[d, q] += v[k, d].T @ p_gated[k, q]
                        nc.tensor.matmul(po_t[h][:, s],
                                         v_sb[:, kt, h * 128:(h + 1) * 128],
                                         pgm[:, s],
                                         start=(kt == 0), stop=(kt == nkt - 1),
                                         skip_group_check=True)
                        if kt == nkt - 1:
                            normalize(h, po_t[h], prs_t[h])

                    for h in range(c.HLOC if c.use_attn else 0):
                        po_t[h] = pop.tile([128, c.QT], F32, name="po", tag="po")
                        prs_t[h] = pp.tile([1, c.QT], F32, name="prs", tag="pp")
                        for kt in range(nkt):
                            score_step(h, kt)
                            if len(stage) > c.PIPE:
                                drain_step()
                    while stage:
                        drain_step()

                    # output-projection partial for this query block,
                    # interleaved with the NEXT block's gate generation so the
                    # PE fills the gate chain's latency with wo matmuls
                    nkt2 = (c.DIAG * (qt + 2)
                            if (qt + 1 < c.QTN and c.use_gate and c.use_attn)
                            else 0)
                    ncha = c.DIM // 128
                    f_sb = wop.tile([128, ncha, c.QT], F16, name="f_sb", tag="f")
                    pfpools = [(pp, "pp"), (pop, "po")]
                    # two gate steps lead the wo loop: their matmuls keep the
                    # PE fed while the last head's normalize chain completes
                    for kt in range(min(2, nkt2)):
                        gate_step(qt + 1, kt)
                    for ch in range(ncha if c.use_wo else 0):
                        pfp, pft = pfpools[ch % 2]
                        pf = pfp.tile([128, c.QT], F32, name="pf", tag=pft)
                        for h in range(c.HLOC):
                            nc.tensor.matmul(
                                pf[:, :],
                                woc_sb[:, h, ch * 128:(ch + 1) * 128],
                                og_sb[:, h, :],
                                start=(h == 0), stop=(h == c.HLOC - 1))
                        if ch % 2 == 0:
                            nc.scalar.copy(f_sb[:, ch, :], pf[:, :])
                        else:
                            nc.vector.tensor_copy(f_sb[:, ch, :], pf[:, :])
                        if ch + 2 < nkt2:
                            gate_step(qt + 1, ch + 2)
                        if qt == c.QTN - 1 and ch % 2 == 1:
                            nc.sync.dma_start(
                                out=pout.ap().rearrange(
                                    "(ch p) q -> p ch q",
                                    p=128)[:, ch - 1:ch + 1, qsl],
                                in_=f_sb[:, ch - 1:ch + 1, :])
                        elif qt < c.QTN - 1 and ch % 4 == 3:
                            # batched output DMA per 4 chunks (0.5MB each):
                            # early chunks fly while later ones compute
                            nc.sync.dma_start(
                                out=pout.ap().rearrange(
                                    "(ch p) q -> p ch q",
                                    p=128)[:, ch - 3:ch + 1, qsl],
                                in_=f_sb[:, ch - 3:ch + 1, :])
                    if not c.use_wo:
                        for kt in range(nkt2):
                            gate_step(qt + 1, kt)

        if c.repeat > 1:
            with tc.For_i(0, c.repeat, 1,
                          hint_engines=(mybir.EngineType.PE,
                                        mybir.EngineType.DVE,
                                        mybir.EngineType.Activation,
                                        mybir.EngineType.Pool,
                                        mybir.EngineType.SP)):
                body()
        else:
            body()

    nc.compile()
    return nc


def make_core_inputs(inputs: dict, cfg: Cfg = FULL):
    """Host-side sharding: returns in_maps (one dict per core)."""
    c = cfg
    bf16 = ml_dtypes.bfloat16
    x = np.asarray(inputs["x"])
    mask = np.asarray(inputs["mask"])
    fc = np.asarray(inputs["freqs_cos"])
    fs = np.asarray(inputs["freqs_sin"])
    wq, wk, wv, wo = (np.asarray(inputs[k]) for k in ("wq", "wk", "wv", "wo"))
    wa_q, wa_k = np.asarray(inputs["wa_q"]), np.asarray(inputs["wa_k"])

    xTb = [np.ascontiguousarray(x[b].T).astype(bf16) for b in range(c.B)]
    waT = np.ascontiguousarray(
        np.concatenate([wa_q, wa_k], axis=0).T).astype(bf16)

    # rope tables in [d, tok] layout, split re/im: rows 0:64 = re lanes,
    # 64:128 = im lanes (wq/wk columns are permuted to match; the score
    # dot-product is invariant to a shared head-dim permutation)
    c2 = np.empty((c.HD, c.S), np.float32)
    s2 = np.empty((c.HD, c.S), np.float32)
    c2[0:64] = fc.T
    c2[64:128] = fc.T
    s2[0:64] = -fs.T
    s2[64:128] = fs.T
    c2 = c2.astype(bf16)
    s2 = s2.astype(bf16)

    # per-head column permutation: even (re) dims first, odd (im) second
    dperm = np.concatenate([np.arange(0, c.HD, 2), np.arange(1, c.HD, 2)])
    qkperm = np.concatenate([hb * c.HD + dperm
                             for hb in range(c.DIM // c.HD)])

    # diagonal-band mask patterns [j][k, q], extracted from the input mask
    qt_last = c.QTN - 1
    q0 = qt_last * c.QT
    maskd = np.empty((c.DIAG, c.KT, c.QT), np.float32)
    for j in range(c.DIAG):
        k0 = (c.DIAG * qt_last + j) * c.KT
        maskd[j] = (mask[0, 0, q0:q0 + c.QT, k0:k0 + c.KT].T == 0.0)
    maskd = maskd.astype(bf16)

    wslices = []
    for hs in range(c.CPG):
        rows = slice(hs * c.DH, (hs + 1) * c.DH)
        wslices.append({
            "wqT": np.ascontiguousarray(wq[qkperm][rows].T).astype(bf16),
            "wkT": np.ascontiguousarray(wk[qkperm][rows].T).astype(bf16),
            "wvT": np.ascontiguousarray(wv[rows].T).astype(bf16),
            "wocT": np.ascontiguousarray(wo[:, rows].T).astype(bf16),
        })

    in_maps = []
    for ci in range(c.NCORES):
        b = ci // c.CPG
        hs = ci % c.CPG
        in_maps.append({
            "xT": xTb[b],
            **wslices[hs],
            "waT": waT,
            "c2d": c2,
            "s2d": s2,
            "maskdd": maskd,
        })
    return in_maps


def assemble_output(results, cfg: Cfg = FULL) -> np.ndarray:
    c = cfg
    out = np.empty((c.B, c.S, c.DIM), np.float32)
    for b in range(c.B):
        total = np.zeros((c.DIM, c.S), np.float32)
        for hs in range(c.CPG):
            total += np.asarray(results[b * c.CPG + hs]["pout"]).astype(np.float32)
        out[b] = total.T
    return out


_NC_CACHE = {}


def run(nc, in_maps, trace=False, cfg: Cfg = FULL, **kw):
    return bass_utils.run_bass_kernel_spmd(
        nc, in_maps, core_ids=list(range(cfg.NCORES)), trace=trace, **kw)


def kernel(**inputs) -> np.ndarray:
    cfg = FULL
    if cfg not in _NC_CACHE:
        _NC_CACHE[cfg] = build_nc(cfg)
    nc = _NC_CACHE[cfg]
    in_maps = make_core_inputs(inputs, cfg)
    res = run(nc, in_maps, cfg=cfg)
    return assemble_output(res.results, cfg)


if __name__ == "__main__":
    nc = build_nc(FULL)
    print("built ok")


# revision 22
# speedup vs baseline: 1.0202x; 1.0061x over previous
"""Trainium2 Bass kernel for nn_Attention_10771777978404 (sparse_attention).

Sharding over 8 NeuronCores: 2 batch-groups x 4 cores (tensor parallel over
heads within each batch group).
  - core ci handles batch ci//4 and heads [4*(ci%4), 4*(ci%4)+4): it computes
    its q/k/v projections (columns of wq/wk/wv), RoPE, causal attention with
    the low-rank sigmoid gate, and a full-width partial of the output
    projection from its 4 heads (rows of wo).
  - the rank-32 adapter weights are replicated inside each batch group; the
    sigmoid gate is computed as 1/(1+exp(-A)) so the scalar engine only ever
    uses the Exp table (no Sigmoid-table reloads, no DRAM staging).
  - host sums the 4 partial output projections per batch (fp16 partials).

Everything on-device is bf16 with fp32 PSUM accumulation.

Schedule notes (v2):
  - diagonal-band tiles only compute the live query columns [128j:512]
    (causal wedge), cutting PE/ACT/DVE work on the band by ~37%.
  - per-head softmax denominators live in ONE PSUM bank at partition
    offsets 32h, removing the head-boundary WAR on the rowsum accumulator.
  - the wo partial-projection PSUM tiles rotate over 4 banks (pp+po pools)
    and the PSUM->SBUF copies alternate ACT/DVE; the per-chunk output DMAs
    are batched into one 2MB DMA per query block (HWDGE descriptor
    generation is a serial ~630ns/dma resource).
  - input DMAs are coarsened and ordered by first-use so the HWDGE queue
    delivers x/wq/wk chunks at PE pace from the start.

self-contained: hardcodes the problem shapes; only needs `concourse` (on
PYTHONPATH in this container) + jax axon devices.
"""

import math
from contextlib import ExitStack
from dataclasses import dataclass

import numpy as np
import ml_dtypes

import concourse.tile as tile
from concourse import bacc
from concourse import mybir
from concourse import bass_utils

BF16 = mybir.dt.bfloat16
F16 = mybir.dt.float16
F32 = mybir.dt.float32
AF = mybir.ActivationFunctionType


@dataclass(frozen=True)
class Cfg:
    B: int = 2
    S: int = 2048
    DIM: int = 2048
    NH: int = 16
    HD: int = 128
    RANK: int = 32
    NCORES: int = 8
    GROUPS: int = 2     # batch groups of 4 cores
    QT: int = 512       # query block (free dim of score tiles)
    KT: int = 128       # key block (partition dim of score tiles)
    PIPE: int = 6       # score tiles in flight ahead of rowsum/AV drains
    repeat: int = 1     # hardware-loop repetitions of the whole body (timing)
    # ablation flags (profiling on hardware; all True for the real kernel)
    use_gate: bool = True
    use_rowsum: bool = True
    use_attn: bool = True
    use_wo: bool = True

    @property
    def CPG(self):
        return self.NCORES // self.GROUPS  # cores per batch group

    @property
    def HLOC(self):
        return self.NH // self.CPG  # heads per core (4)

    @property
    def DH(self):
        return self.HLOC * self.HD  # per-core head-dim span (512)

    @property
    def KTILES(self):
        return self.DIM // 128  # contraction tiles for projections

    @property
    def QTN(self):
        return self.S // self.QT

    @property
    def DIAG(self):
        return self.QT // self.KT  # k-tiles per diagonal band


FULL = Cfg()


def build_nc(cfg: Cfg = FULL):
    c = cfg
    assert c.HD == 128 and c.KT == 128
    nc = bacc.Bacc("TRN2", target_bir_lowering=False, debug=False,
                   num_devices=c.NCORES)

    # ---- kernel I/O (per core: one batch, HLOC heads) ----
    xT = nc.dram_tensor("xT", [c.DIM, c.S], BF16, kind="ExternalInput")
    wqT = nc.dram_tensor("wqT", [c.DIM, c.DH], BF16, kind="ExternalInput")
    wkT = nc.dram_tensor("wkT", [c.DIM, c.DH], BF16, kind="ExternalInput")
    wvT = nc.dram_tensor("wvT", [c.DIM, c.DH], BF16, kind="ExternalInput")
    wocT = nc.dram_tensor("wocT", [c.DH, c.DIM], BF16, kind="ExternalInput")
    waT = nc.dram_tensor("waT", [c.DIM, 2 * c.RANK], BF16, kind="ExternalInput")
    c2d = nc.dram_tensor("c2d", [c.HD, c.S], BF16, kind="ExternalInput")
    s2d = nc.dram_tensor("s2d", [c.HD, c.S], BF16, kind="ExternalInput")
    maskdd = nc.dram_tensor("maskdd", [c.DIAG, c.KT, c.QT], BF16, kind="ExternalInput")

    # partial output projection, transposed: pout[j, t] (fp16; host sums the
    # 4 partials of each batch group in fp32)
    pout = nc.dram_tensor("pout", [c.DIM, c.S], F16, kind="ExternalOutput")

    isqrt = 1.0 / math.sqrt(c.HD)
    sc_score = isqrt
    sc_gate = -1.0
    NKT = c.KTILES

    with ExitStack() as _ctx:
        tc = _ctx.enter_context(tile.TileContext(nc))
        # persistent pools (whole-iteration lifetime)
        cst = _ctx.enter_context(tc.tile_pool(name="const", bufs=1))
        adp = _ctx.enter_context(tc.tile_pool(name="ap", bufs=1))
        qkp = _ctx.enter_context(tc.tile_pool(name="qk", bufs=1))
        vp = _ctx.enter_context(tc.tile_pool(name="vp", bufs=1))
        rtp = _ctx.enter_context(tc.tile_pool(name="rope_t", bufs=1))
        pge = _ctx.enter_context(tc.tile_pool(name="pge", bufs=7))
        gwk = _ctx.enter_context(tc.tile_pool(name="gwk", bufs=2))
        ogp = _ctx.enter_context(tc.tile_pool(name="og", bufs=1))
        # PSUM pools (8 banks total): pp 2 + ps 4 + po 2; the per-head
        # softmax-denominator tiles and the norm-broadcast tiles share the
        # pp rotation so rowsum accumulation never WARs the previous head
        pp = _ctx.enter_context(tc.tile_pool(name="pp", bufs=2, space="PSUM"))
        psp = _ctx.enter_context(tc.tile_pool(name="ps", bufs=4, space="PSUM"))
        pop = _ctx.enter_context(tc.tile_pool(name="po", bufs=2, space="PSUM"))

        def body():
            # ---- constants ----
            c2_sb = cst.tile([128, c.S], BF16, name="c2_sb", tag="c2")
            s2_sb = cst.tile([128, c.S], BF16, name="s2_sb", tag="s2")
            ones_sb = cst.tile([128, 1], BF16, name="ones_sb", tag="ones")

            # packed adapter projections: one [64,512] matmul computes both
            # aq and ak rows (halves the adapter matmul count); ak is then
            # moved to partition base 0 with one SBUF->SBUF DMA (engines
            # cannot shift partitions, DMA can)
            aqk_sb = adp.tile([2 * c.RANK, c.S], BF16, name="aqk_sb", tag="aqk")
            ak_sb = adp.tile([c.RANK, c.S], BF16, name="ak_sb", tag="ak")
            q_sb = [qkp.tile([128, c.S], BF16, name=f"q{h}_sb", tag=f"q{h}")
                    for h in range(c.HLOC)]
            k_sb = [qkp.tile([128, c.S], BF16, name=f"k{h}_sb", tag=f"k{h}")
                    for h in range(c.HLOC)]
            v_sb = vp.tile([128, c.S // 128, c.DH], BF16, name="v_sb", tag="v")

            with tc.tile_pool(name="xtp", bufs=1) as xtp:
                wa_sb = xtp.tile([128, c.KTILES, 2 * c.RANK], BF16,
                                 name="wa_sb", tag="wa")
                nc.sync.dma_start(out=wa_sb,
                                  in_=waT.ap().rearrange("(t p) m -> p t m", p=128))
                xt_sb = xtp.tile([128, c.KTILES, c.S], BF16, name="xt_sb", tag="xt")
                xr = xT.ap().rearrange("(t p) n -> p t n", p=128)

                def xsl(j, sl):
                    return xt_sb[:, j, sl]

                nc.vector.memset(ones_sb, 1.0)

                # ---- adapter + q/k projections, kt-outer, 8 live psum accs ----
                apools = [pp, psp, pop, psp]
                acc = [apools[i].tile([2 * c.RANK, c.QT], F32, name=f"acc_a{i}",
                                      tag=("pp", "ps", "po", "ps")[i])
                       for i in range(c.QTN)]

                wvpool = None
                with tc.tile_pool(name="wqk", bufs=1) as wp:
                    wq_sb = wp.tile([128, c.KTILES, c.DH], BF16, name="wq_sb", tag="wq")
                    wk_sb = wp.tile([128, c.KTILES, c.DH], BF16, name="wk_sb", tag="wk")
                    # input DMAs ordered by first use; the adapter matmuls
                    # lead the head matmuls by 2 kt steps, so the sweep can
                    # start on x alone while wq/wk stream in behind
                    nc.sync.dma_start(out=xt_sb[:, 0, :], in_=xr[:, 0, :])
                    nc.sync.dma_start(out=xt_sb[:, 1, :], in_=xr[:, 1, :])
                    wqr = wqT.ap().rearrange("(t p) m -> p t m", p=128)
                    wkr = wkT.ap().rearrange("(t p) m -> p t m", p=128)
                    nc.scalar.dma_start(out=wq_sb[:, 0:2, :], in_=wqr[:, 0:2, :])
                    nc.scalar.dma_start(out=wk_sb[:, 0:2, :], in_=wkr[:, 0:2, :])
                    nc.sync.dma_start(out=xt_sb[:, 2:4, :], in_=xr[:, 2:4, :])
                    nc.scalar.dma_start(out=wq_sb[:, 2:6, :], in_=wqr[:, 2:6, :])
                    nc.scalar.dma_start(out=wk_sb[:, 2:6, :], in_=wkr[:, 2:6, :])
                    nc.sync.dma_start(out=xt_sb[:, 4:8, :], in_=xr[:, 4:8, :])
                    nc.scalar.dma_start(out=wq_sb[:, 6:, :], in_=wqr[:, 6:, :])
                    nc.scalar.dma_start(out=wk_sb[:, 6:, :], in_=wkr[:, 6:, :])
                    nc.sync.dma_start(out=xt_sb[:, 8:12, :], in_=xr[:, 8:12, :])
                    nc.sync.dma_start(out=xt_sb[:, 12:16, :], in_=xr[:, 12:16, :])

                    allqk = [(dst, w, h, qt)
                             for dst, w in ((q_sb, wq_sb), (k_sb, wk_sb))
                             for h in range(c.HLOC)
                             for qt in range(c.QTN)]
                    # first four groups run kt-outer interleaved with the
                    # adapter accumulation so the PE issues 8 matmuls per
                    # arriving x chunk instead of 4
                    head_pools = [(pp, "pp"), (psp, "ps"), (psp, "ps"),
                                  (pop, "po")]
                    headacc = [head_pools[i][0].tile([128, c.QT], F32,
                                                     name=f"acc_qk{i}",
                                                     tag=head_pools[i][1])
                               for i in range(4)]
                    LAG = 2
                    for s in range(NKT + LAG):
                        if s < NKT:
                            for qt in range(c.QTN):
                                nc.tensor.matmul(
                                    acc[qt][:, :],
                                    wa_sb[:, s, :],
                                    xsl(s, slice(qt * c.QT, (qt + 1) * c.QT)),
                                    start=(s == 0), stop=(s == NKT - 1))
                        if s >= LAG:
                            kt = s - LAG
                            for i in range(4):
                                dst, w, h, qt = allqk[i]
                                nc.tensor.matmul(
                                    headacc[i][:, :],
                                    w[:, kt, h * 128:(h + 1) * 128],
                                    xsl(kt, slice(qt * c.QT, (qt + 1) * c.QT)),
                                    start=(kt == 0), stop=(kt == NKT - 1))
                    for qt in range(c.QTN):
                        nc.vector.tensor_copy(
                            aqk_sb[:, qt * c.QT:(qt + 1) * c.QT], acc[qt][:, :])
                    nc.sync.dma_start(out=ak_sb[:, :],
                                      in_=aqk_sb[c.RANK:2 * c.RANK, :])
                    for i in range(4):
                        dst, w, h, qt = allqk[i]
                        nc.scalar.copy(dst[h][:, qt * c.QT:(qt + 1) * c.QT],
                                       headacc[i][:, :])
                    for dst, w, h, qt in allqk[4:]:
                        psum = pp.tile([128, c.QT], F32, name="psum_qk", tag="pp")
                        for kt in range(NKT):
                            nc.tensor.matmul(
                                psum[:, :],
                                w[:, kt, h * 128:(h + 1) * 128],
                                xsl(kt, slice(qt * c.QT, (qt + 1) * c.QT)),
                                start=(kt == 0), stop=(kt == NKT - 1))
                        nc.scalar.copy(dst[h][:, qt * c.QT:(qt + 1) * c.QT],
                                       psum[:, :])

                # ---- v projection: [tok, d] natural, moving 512 wide ----
                # RoPE tiles are interleaved into the v loop: the rope chain
                # is DVE/GPSIMD-paced, the v matmuls keep the PE busy
                rope_tiles = [(tiles, h, qt)
                              for tiles in (q_sb, k_sb)
                              for h in range(c.HLOC)
                              for qt in range(c.QTN)]

                def rope_step(i):
                    # q/k head dims use a split re/im layout (host permutes
                    # wq/wk columns and the rope tables; scores are invariant
                    # to a shared d-permutation), so the pair-swap is a
                    # half-swap: two contiguous partition-range DMAs instead
                    # of a PE permute matmul.
                    tiles, h, qt = rope_tiles[i]
                    eng = nc.vector if i % 2 == 0 else nc.gpsimd
                    sl = slice(qt * c.QT, (qt + 1) * c.QT)
                    swp = rtp.tile([128, c.QT], BF16, name="rope_swp",
                                   tag=f"sw{i % 2}")
                    nc.sync.dma_start(out=swp[0:64, :], in_=tiles[h][64:128, sl])
                    nc.sync.dma_start(out=swp[64:128, :], in_=tiles[h][0:64, sl])
                    m1 = rtp.tile([128, c.QT], BF16, name="rope_m1",
                                  tag=f"m1{i % 2}")
                    nc.gpsimd.tensor_mul(m1[:, :], tiles[h][:, sl], c2_sb[:, sl])
                    # sin term lands in place over the swapped copy
                    nc.vector.tensor_mul(swp[:, :], swp[:, :], s2_sb[:, sl])
                    eng.tensor_add(tiles[h][:, sl], m1[:, :], swp[:, :])

                with tc.tile_pool(name="wvp", bufs=1) as wvpool:
                    wv_sb = wvpool.tile([128, c.KTILES, c.DH], BF16,
                                        name="wv_sb", tag="wv")
                    wr = wvT.ap().rearrange("(t p) m -> p t m", p=128)
                    nc.sync.dma_start(out=wv_sb[:, 0:4, :], in_=wr[:, 0:4, :])
                    nc.sync.dma_start(out=wv_sb[:, 4:8, :], in_=wr[:, 4:8, :])
                    nc.sync.dma_start(out=wv_sb[:, 8:16, :], in_=wr[:, 8:16, :])
                    # rope tables arrive while the first v token-blocks run
                    nc.sync.dma_start(out=c2_sb, in_=c2d.ap())
                    nc.sync.dma_start(out=s2_sb, in_=s2d.ap())
                    for tt in range(c.S // 128):
                        psum = pp.tile([128, c.DH], F32, name="psum_v", tag="pp")
                        for kt in range(NKT):
                            nc.tensor.matmul(
                                psum[:, :],
                                xsl(kt, slice(tt * 128, (tt + 1) * 128)),
                                wv_sb[:, kt, :],
                                start=(kt == 0), stop=(kt == NKT - 1))
                        nc.scalar.copy(v_sb[:, tt, :], psum[:, :])
                        rope_step(2 * tt)
                        rope_step(2 * tt + 1)

            # ---- attention + output projection, per 512-query block ----
            with tc.tile_pool(name="wog", bufs=1) as wog, \
                    tc.tile_pool(name="wo_out", bufs=2) as wop:
                mask_sb = wog.tile([128, c.DIAG, c.QT], BF16, name="mask_sb",
                                   tag="mask")
                nc.sync.dma_start(out=mask_sb,
                                  in_=maskdd.ap().rearrange("j p q -> p j q"))
                woc_sb = wog.tile([128, c.HLOC, c.DIM], BF16, name="woc_sb", tag="woc")
                wcr = wocT.ap().rearrange("(h p) j -> p h j", p=128)
                nc.sync.dma_start(out=woc_sb[:, 0:2, :], in_=wcr[:, 0:2, :])
                nc.sync.dma_start(out=woc_sb[:, 2:4, :], in_=wcr[:, 2:4, :])
                # double-buffered per-qt gate tiles r = sigmoid(A)[k, q]
                rg_sb = [wog.tile([128, c.DIAG * c.QTN, c.QT], BF16,
                                  name=f"rg{i}_sb", tag=f"rg{i}") for i in range(2)]

                def gate_step(qt, kt):
                    # r = 1/(1+exp(-A)) via the (shared) Exp table
                    qsl = slice(qt * c.QT, (qt + 1) * c.QT)
                    ksl = slice(kt * c.KT, (kt + 1) * c.KT)
                    pga = psp.tile([128, c.QT], F32, name="pga", tag="ps")
                    nc.tensor.matmul(pga[:, :], ak_sb[:, ksl],
                                     aqk_sb[0:c.RANK, qsl],
                                     start=True, stop=True)
                    ge = pge.tile([128, c.QT], BF16, name="ge", tag="p")
                    nc.scalar.activation(ge[:, :], pga[:, :], AF.Exp, scale=sc_gate)
                    gt = gwk.tile([128, c.QT], F32, name="gt", tag="gt")
                    nc.vector.tensor_scalar_add(gt[:, :], ge[:, :], 1.0)
                    gr = gwk.tile([128, c.QT], F32, name="gr", tag="gr")
                    nc.vector.reciprocal_approx_fast(out=gr[:, :], in_=gt[:, :])
                    nc.gpsimd.tensor_copy(rg_sb[qt % 2][:, kt, :], gr[:, :])

                if c.use_gate and c.use_attn:
                    for kt in range(c.DIAG):
                        gate_step(0, kt)

                for qt in range(c.QTN):
                    qsl = slice(qt * c.QT, (qt + 1) * c.QT)
                    nkt = c.DIAG * (qt + 1)  # causal k tiles
                    rg = rg_sb[qt % 2]

                    og_sb = ogp.tile([128, c.HLOC, c.QT], BF16, name="og_sb", tag="og")
                    if not c.use_attn:
                        nc.vector.memset(og_sb, 0.0)

                    def normalize(h, po, prs):
                        if not c.use_rowsum:
                            nc.vector.tensor_copy(og_sb[:, h, :], po[:, :])
                            return
                        rr = gwk.tile([1, c.QT], F32, name="rr", tag="rr", bufs=1)
                        nc.vector.reciprocal_approx_fast(
                            out=rr[:, :], in_=prs[0:1, :])
                        rrh = gwk.tile([1, c.QT], F16, name="rrh", tag="rrh", bufs=1)
                        nc.vector.tensor_copy(rrh[:, :], rr[:, :])
                        # GPSIMD extended-ISA broadcast: partition 0 -> all
                        rbs = gwk.tile([128, c.QT], F16, name="rbs", tag="rbs", bufs=1)
                        nc.gpsimd.partition_broadcast(rbs[:, :], rrh[0:1, :])
                        nc.vector.tensor_mul(og_sb[:, h, :], po[:, :],
                                             rbs[:, :])

                    # single software pipeline over all (h, kt) score tiles:
                    # drains lag PIPE steps behind scores ACROSS head
                    # boundaries, so head-end drain tails overlap the next
                    # head's score matmuls; normalize(h) is emitted inline
                    # right after head h's last drain
                    po_t, prs_t = {}, {}
                    stage = []  # (h, kt, col-slice, p_or_pm_tile, pgm_tile)

                    def score_step(h, kt):
                        ksl = slice(kt * c.KT, (kt + 1) * c.KT)
                        j = kt - c.DIAG * qt
                        qoff = 128 * j if j > 0 else 0
                        s = slice(qoff, c.QT)
                        qs = slice(qt * c.QT + qoff, (qt + 1) * c.QT)
                        ps = psp.tile([128, c.QT], F32, name="ps", tag="ps")
                        nc.tensor.matmul(ps[:, s], k_sb[h][:, ksl],
                                         q_sb[h][:, qs], start=True, stop=True)
                        p_sb = pge.tile([128, c.QT], BF16, name="p_sb", tag="p")
                        nc.scalar.activation(p_sb[:, s], ps[:, s], AF.Exp,
                                             scale=sc_score)
                        if j >= 0:
                            # diagonal band: 0/1 causal mask after exp,
                            # in place (rowsum needs the masked pre-gate sum)
                            nc.vector.tensor_mul(p_sb[:, s], p_sb[:, s],
                                                 mask_sb[:, j, s])
                        if c.use_gate:
                            pgm = pge.tile([128, c.QT], BF16, name="pgm",
                                           tag="pgm")
                            nc.vector.tensor_mul(pgm[:, s], p_sb[:, s],
                                                 rg[:, kt, s])
                        else:
                            pgm = p_sb
                        stage.append((h, kt, s, p_sb, pgm))

                    def drain_step():
                        h, kt, s, p_sb, pgm = stage.pop(0)
                        # pre-gate rowsum (softmax denominator)
                        if c.use_rowsum:
                            nc.tensor.matmul(prs_t[h][0:1, s],
                                             ones_sb[:, :], p_sb[:, s],
                                             start=(kt == 0),
                                             stop=(kt == nkt - 1),
                                             skip_group_check=True)
                        # out_h^T[d, q] += v[k, d].T @ p_gated[k, q]
                        nc.tensor.matmul(po_t[h][:, s],
                                         v_sb[:, kt, h * 128:(h + 1) * 128],
                                         pgm[:, s],
                                         start=(kt == 0), stop=(kt == nkt - 1),
                                         skip_group_check=True)
                        if kt == nkt - 1:
                            normalize(h, po_t[h], prs_t[h])

                    for h in range(c.HLOC if c.use_attn else 0):
                        po_t[h] = pop.tile([128, c.QT], F32, name="po", tag="po")
                        prs_t[h] = pp.tile([1, c.QT], F32, name="prs", tag="pp")
                        for kt in range(nkt):
                            score_step(h, kt)
                            if len(stage) > c.PIPE:
                                drain_step()
                    while stage:
                        drain_step()

                    # output-projection partial for this query block,
                    # interleaved with the NEXT block's gate generation so the
                    # PE fills the gate chain's latency with wo matmuls
                    nkt2 = (c.DIAG * (qt + 2)
                            if (qt + 1 < c.QTN and c.use_gate and c.use_attn)
                            else 0)
                    ncha = c.DIM // 128
                    f_sb = wop.tile([128, ncha, c.QT], F16, name="f_sb", tag="f")
                    pfpools = [(pp, "pp"), (pop, "po")]
                    # two gate steps lead the wo loop: their matmuls keep the
                    # PE fed while the last head's normalize chain completes
                    for kt in range(min(2, nkt2)):
                        gate_step(qt + 1, kt)
                    for ch in range(ncha if c.use_wo else 0):
                        pfp, pft = pfpools[ch % 2]
                        pf = pfp.tile([128, c.QT], F32, name="pf", tag=pft)
                        for h in range(c.HLOC):
                            nc.tensor.matmul(
                                pf[:, :],
                                woc_sb[:, h, ch * 128:(ch + 1) * 128],
                                og_sb[:, h, :],
                                start=(h == 0), stop=(h == c.HLOC - 1))
                        if ch % 2 == 0:
                            nc.scalar.copy(f_sb[:, ch, :], pf[:, :])
                        else:
                            nc.vector.tensor_copy(f_sb[:, ch, :], pf[:, :])
                        if ch + 2 < nkt2:
                            gate_step(qt + 1, ch + 2)
                        if qt == c.QTN - 1 and ch % 2 == 1:
                            nc.sync.dma_start(
                                out=pout.ap().rearrange(
                                    "(ch p) q -> p ch q",
                                    p=128)[:, ch - 1:ch + 1, qsl],
                                in_=f_sb[:, ch - 1:ch + 1, :])
                        elif qt < c.QTN - 1 and ch % 4 == 3:
                            # batched output DMA per 4 chunks (0.5MB each):
                            # early chunks fly while later ones compute
                            nc.sync.dma_start(
                                out=pout.ap().rearrange(
                                    "(ch p) q -> p ch q",
                                    p=128)[:, ch - 3:ch + 1, qsl],
                                in_=f_sb[:, ch - 3:ch + 1, :])
                    if not c.use_wo:
                        for kt in range(nkt2):
                            gate_step(qt + 1, kt)

        if c.repeat > 1:
            with tc.For_i(0, c.repeat, 1,
                          hint_engines=(mybir.EngineType.PE,
                                        mybir.EngineType.DVE,
                                        mybir.EngineType.Activation,
                                        mybir.EngineType.Pool,
                                        mybir.EngineType.SP)):
                body()
        else:
            body()

    nc.compile()
    return nc


def make_core_inputs(inputs: dict, cfg: Cfg = FULL):
    """Host-side sharding: returns in_maps (one dict per core)."""
    c = cfg
    bf16 = ml_dtypes.bfloat16
    x = np.asarray(inputs["x"])
    mask = np.asarray(inputs["mask"])
    fc = np.asarray(inputs["freqs_cos"])
    fs = np.asarray(inputs["freqs_sin"])
    wq, wk, wv, wo = (np.asarray(inputs[k]) for k in ("wq", "wk", "wv", "wo"))
    wa_q, wa_k = np.asarray(inputs["wa_q"]), np.asarray(inputs["wa_k"])

    xTb = [np.ascontiguousarray(x[b].T).astype(bf16) for b in range(c.B)]
    waT = np.ascontiguousarray(
        np.concatenate([wa_q, wa_k], axis=0).T).astype(bf16)

    # rope tables in [d, tok] layout, split re/im: rows 0:64 = re lanes,
    # 64:128 = im lanes (wq/wk columns are permuted to match; the score
    # dot-product is invariant to a shared head-dim permutation)
    c2 = np.empty((c.HD, c.S), np.float32)
    s2 = np.empty((c.HD, c.S), np.float32)
    c2[0:64] = fc.T
    c2[64:128] = fc.T
    s2[0:64] = -fs.T
    s2[64:128] = fs.T
    c2 = c2.astype(bf16)
    s2 = s2.astype(bf16)

    # per-head column permutation: even (re) dims first, odd (im) second
    dperm = np.concatenate([np.arange(0, c.HD, 2), np.arange(1, c.HD, 2)])
    qkperm = np.concatenate([hb * c.HD + dperm
                             for hb in range(c.DIM // c.HD)])

    # diagonal-band mask patterns [j][k, q], extracted from the input mask
    qt_last = c.QTN - 1
    q0 = qt_last * c.QT
    maskd = np.empty((c.DIAG, c.KT, c.QT), np.float32)
    for j in range(c.DIAG):
        k0 = (c.DIAG * qt_last + j) * c.KT
        maskd[j] = (mask[0, 0, q0:q0 + c.QT, k0:k0 + c.KT].T == 0.0)
    maskd = maskd.astype(bf16)

    wslices = []
    for hs in range(c.CPG):
        rows = slice(hs * c.DH, (hs + 1) * c.DH)
        wslices.append({
            "wqT": np.ascontiguousarray(wq[qkperm][rows].T).astype(bf16),
            "wkT": np.ascontiguousarray(wk[qkperm][rows].T).astype(bf16),
            "wvT": np.ascontiguousarray(wv[rows].T).astype(bf16),
            "wocT": np.ascontiguousarray(wo[:, rows].T).astype(bf16),
        })

    in_maps = []
    for ci in range(c.NCORES):
        b = ci // c.CPG
        hs = ci % c.CPG
        in_maps.append({
            "xT": xTb[b],
            **wslices[hs],
            "waT": waT,
            "c2d": c2,
            "s2d": s2,
            "maskdd": maskd,
        })
    return in_maps


def assemble_output(results, cfg: Cfg = FULL) -> np.ndarray:
    c = cfg
    out = np.empty((c.B, c.S, c.DIM), np.float32)
    for b in range(c.B):
        total = np.zeros((c.DIM, c.S), np.float32)
        for hs in range(c.CPG):
            total += np.asarray(results[b * c.CPG + hs]["pout"]).astype(np.float32)
        out[b] = total.T
    return out


_NC_CACHE = {}


def run(nc, in_maps, trace=False, cfg: Cfg = FULL, **kw):
    return bass_utils.run_bass_kernel_spmd(
        nc, in_maps, core_ids=list(range(cfg.NCORES)), trace=trace, **kw)


def kernel(**inputs) -> np.ndarray:
    cfg = FULL
    if cfg not in _NC_CACHE:
        _NC_CACHE[cfg] = build_nc(cfg)
    nc = _NC_CACHE[cfg]
    in_maps = make_core_inputs(inputs, cfg)
    res = run(nc, in_maps, cfg=cfg)
    return assemble_output(res.results, cfg)


if __name__ == "__main__":
    nc = build_nc(FULL)
    print("built ok")


# revision 24
# speedup vs baseline: 1.0270x; 1.0067x over previous
"""Trainium2 Bass kernel for nn_Attention_10771777978404 (sparse_attention).

Sharding over 8 NeuronCores: 2 batch-groups x 4 cores (tensor parallel over
heads within each batch group).
  - core ci handles batch ci//4 and heads [4*(ci%4), 4*(ci%4)+4): it computes
    its q/k/v projections (columns of wq/wk/wv), RoPE, causal attention with
    the low-rank sigmoid gate, and a full-width partial of the output
    projection from its 4 heads (rows of wo).
  - the rank-32 adapter weights are replicated inside each batch group; the
    sigmoid gate is computed as 1/(1+exp(-A)) so the scalar engine only ever
    uses the Exp table (no Sigmoid-table reloads, no DRAM staging).
  - host sums the 4 partial output projections per batch (fp16 partials).

Everything on-device is bf16 with fp32 PSUM accumulation.

Schedule notes (v5, 510us on HW vs 603us baseline):
  - diagonal-band tiles only compute the live query columns [128j:512]
    (causal wedge), cutting PE/ACT/DVE work on the band by ~37%; the causal
    mask is applied in place on the exp tile.
  - attention runs as ONE flat software pipeline over all (h, kt) tiles per
    query block: rowsum/AV drains lag the score matmuls by PIPE=6 steps
    ACROSS head boundaries, and normalize(h) is emitted inline after head
    h's last drain. Per-head denominators rotate through the pp PSUM pool
    (2 banks) so rowsum accumulation never WARs the previous head.
  - RoPE uses a split re/im head-dim layout (host permutes wq/wk columns
    and the rope tables; scores are invariant to a shared d-permutation),
    so the pair-swap is two contiguous partition-range DMAs instead of a
    PE permute matmul; the softmax 1/sum broadcast runs on the GPSIMD
    extended-ISA partition_broadcast instead of a ones-matmul. Both cuts
    matter because each matmul carries a ~55ns serial Ldweights cost.
  - the wo partial-projection PSUM tiles rotate over 4 banks (pp+po pools),
    PSUM->SBUF copies alternate ACT/DVE, gate generation for the next block
    interleaves into the wo loop, and output DMAs are batched 4 chunks per
    dma_start (HWDGE descriptor generation is a serial ~630ns/dma).
  - input DMAs are ordered by first use with the adapter matmuls leading
    the head matmuls by 2 kt steps, so the projection sweep starts on x
    alone while wq/wk stream in behind.

self-contained: hardcodes the problem shapes; only needs `concourse` (on
PYTHONPATH in this container) + jax axon devices.
"""

import math
from contextlib import ExitStack
from dataclasses import dataclass

import numpy as np
import ml_dtypes

import concourse.tile as tile
from concourse import bacc
from concourse import mybir
from concourse import bass_utils

BF16 = mybir.dt.bfloat16
F16 = mybir.dt.float16
F32 = mybir.dt.float32
AF = mybir.ActivationFunctionType


@dataclass(frozen=True)
class Cfg:
    B: int = 2
    S: int = 2048
    DIM: int = 2048
    NH: int = 16
    HD: int = 128
    RANK: int = 32
    NCORES: int = 8
    GROUPS: int = 2     # batch groups of 4 cores
    QT: int = 512       # query block (free dim of score tiles)
    KT: int = 128       # key block (partition dim of score tiles)
    PIPE: int = 8       # score tiles in flight ahead of rowsum/AV drains
    repeat: int = 1     # hardware-loop repetitions of the whole body (timing)
    # ablation flags (profiling on hardware; all True for the real kernel)
    use_gate: bool = True
    use_rowsum: bool = True
    use_attn: bool = True
    use_wo: bool = True

    @property
    def CPG(self):
        return self.NCORES // self.GROUPS  # cores per batch group

    @property
    def HLOC(self):
        return self.NH // self.CPG  # heads per core (4)

    @property
    def DH(self):
        return self.HLOC * self.HD  # per-core head-dim span (512)

    @property
    def KTILES(self):
        return self.DIM // 128  # contraction tiles for projections

    @property
    def QTN(self):
        return self.S // self.QT

    @property
    def DIAG(self):
        return self.QT // self.KT  # k-tiles per diagonal band


FULL = Cfg()


def build_nc(cfg: Cfg = FULL):
    c = cfg
    assert c.HD == 128 and c.KT == 128
    nc = bacc.Bacc("TRN2", target_bir_lowering=False, debug=False,
                   num_devices=c.NCORES)

    # ---- kernel I/O (per core: one batch, HLOC heads) ----
    xT = nc.dram_tensor("xT", [c.DIM, c.S], BF16, kind="ExternalInput")
    wqT = nc.dram_tensor("wqT", [c.DIM, c.DH], BF16, kind="ExternalInput")
    wkT = nc.dram_tensor("wkT", [c.DIM, c.DH], BF16, kind="ExternalInput")
    wvT = nc.dram_tensor("wvT", [c.DIM, c.DH], BF16, kind="ExternalInput")
    wocT = nc.dram_tensor("wocT", [c.DH, c.DIM], BF16, kind="ExternalInput")
    waT = nc.dram_tensor("waT", [c.DIM, 2 * c.RANK], BF16, kind="ExternalInput")
    c2d = nc.dram_tensor("c2d", [c.HD, c.S], BF16, kind="ExternalInput")
    s2d = nc.dram_tensor("s2d", [c.HD, c.S], BF16, kind="ExternalInput")
    maskdd = nc.dram_tensor("maskdd", [c.DIAG, c.KT, c.QT], BF16, kind="ExternalInput")

    # partial output projection, transposed: pout[j, t] (fp16; host sums the
    # 4 partials of each batch group in fp32)
    pout = nc.dram_tensor("pout", [c.DIM, c.S], F16, kind="ExternalOutput")

    isqrt = 1.0 / math.sqrt(c.HD)
    sc_score = isqrt
    sc_gate = -1.0
    NKT = c.KTILES

    with ExitStack() as _ctx:
        tc = _ctx.enter_context(tile.TileContext(nc))
        # persistent pools (whole-iteration lifetime)
        cst = _ctx.enter_context(tc.tile_pool(name="const", bufs=1))
        adp = _ctx.enter_context(tc.tile_pool(name="ap", bufs=1))
        qkp = _ctx.enter_context(tc.tile_pool(name="qk", bufs=1))
        vp = _ctx.enter_context(tc.tile_pool(name="vp", bufs=1))
        rtp = _ctx.enter_context(tc.tile_pool(name="rope_t", bufs=1))
        pge = _ctx.enter_context(tc.tile_pool(name="pge", bufs=9))
        gwk = _ctx.enter_context(tc.tile_pool(name="gwk", bufs=2))
        ogp = _ctx.enter_context(tc.tile_pool(name="og", bufs=1))
        # PSUM pools (8 banks total): pp 2 + ps 4 + po 2; the per-head
        # softmax-denominator tiles and the norm-broadcast tiles share the
        # pp rotation so rowsum accumulation never WARs the previous head
        pp = _ctx.enter_context(tc.tile_pool(name="pp", bufs=2, space="PSUM"))
        psp = _ctx.enter_context(tc.tile_pool(name="ps", bufs=4, space="PSUM"))
        pop = _ctx.enter_context(tc.tile_pool(name="po", bufs=2, space="PSUM"))

        def body():
            # ---- constants ----
            c2_sb = cst.tile([128, c.S], BF16, name="c2_sb", tag="c2")
            s2_sb = cst.tile([128, c.S], BF16, name="s2_sb", tag="s2")
            ones_sb = cst.tile([128, 1], BF16, name="ones_sb", tag="ones")

            # packed adapter projections: one [64,512] matmul computes both
            # aq and ak rows (halves the adapter matmul count); ak is then
            # moved to partition base 0 with one SBUF->SBUF DMA (engines
            # cannot shift partitions, DMA can)
            aqk_sb = adp.tile([2 * c.RANK, c.S], BF16, name="aqk_sb", tag="aqk")
            ak_sb = adp.tile([c.RANK, c.S], BF16, name="ak_sb", tag="ak")
            q_sb = [qkp.tile([128, c.S], BF16, name=f"q{h}_sb", tag=f"q{h}")
                    for h in range(c.HLOC)]
            k_sb = [qkp.tile([128, c.S], BF16, name=f"k{h}_sb", tag=f"k{h}")
                    for h in range(c.HLOC)]
            v_sb = vp.tile([128, c.S // 128, c.DH], BF16, name="v_sb", tag="v")

            with tc.tile_pool(name="xtp", bufs=1) as xtp:
                wa_sb = xtp.tile([128, c.KTILES, 2 * c.RANK], BF16,
                                 name="wa_sb", tag="wa")
                nc.sync.dma_start(out=wa_sb,
                                  in_=waT.ap().rearrange("(t p) m -> p t m", p=128))
                xt_sb = xtp.tile([128, c.KTILES, c.S], BF16, name="xt_sb", tag="xt")
                xr = xT.ap().rearrange("(t p) n -> p t n", p=128)

                def xsl(j, sl):
                    return xt_sb[:, j, sl]

                nc.vector.memset(ones_sb, 1.0)

                # ---- adapter + q/k projections, kt-outer, 8 live psum accs ----
                apools = [pp, psp, pop, psp]
                acc = [apools[i].tile([2 * c.RANK, c.QT], F32, name=f"acc_a{i}",
                                      tag=("pp", "ps", "po", "ps")[i])
                       for i in range(c.QTN)]

                wvpool = None
                with tc.tile_pool(name="wqk", bufs=1) as wp:
                    wq_sb = wp.tile([128, c.KTILES, c.DH], BF16, name="wq_sb", tag="wq")
                    wk_sb = wp.tile([128, c.KTILES, c.DH], BF16, name="wk_sb", tag="wk")
                    # input DMAs ordered by first use; the adapter matmuls
                    # lead the head matmuls by 2 kt steps, so the sweep can
                    # start on x alone while wq/wk stream in behind
                    nc.sync.dma_start(out=xt_sb[:, 0, :], in_=xr[:, 0, :])
                    nc.sync.dma_start(out=xt_sb[:, 1, :], in_=xr[:, 1, :])
                    wqr = wqT.ap().rearrange("(t p) m -> p t m", p=128)
                    wkr = wkT.ap().rearrange("(t p) m -> p t m", p=128)
                    nc.scalar.dma_start(out=wq_sb[:, 0:2, :], in_=wqr[:, 0:2, :])
                    nc.scalar.dma_start(out=wk_sb[:, 0:2, :], in_=wkr[:, 0:2, :])
                    nc.sync.dma_start(out=xt_sb[:, 2:4, :], in_=xr[:, 2:4, :])
                    nc.scalar.dma_start(out=wq_sb[:, 2:6, :], in_=wqr[:, 2:6, :])
                    nc.scalar.dma_start(out=wk_sb[:, 2:6, :], in_=wkr[:, 2:6, :])
                    nc.sync.dma_start(out=xt_sb[:, 4:8, :], in_=xr[:, 4:8, :])
                    nc.scalar.dma_start(out=wq_sb[:, 6:, :], in_=wqr[:, 6:, :])
                    nc.scalar.dma_start(out=wk_sb[:, 6:, :], in_=wkr[:, 6:, :])
                    nc.sync.dma_start(out=xt_sb[:, 8:12, :], in_=xr[:, 8:12, :])
                    nc.sync.dma_start(out=xt_sb[:, 12:16, :], in_=xr[:, 12:16, :])

                    allqk = [(dst, w, h, qt)
                             for dst, w in ((q_sb, wq_sb), (k_sb, wk_sb))
                             for h in range(c.HLOC)
                             for qt in range(c.QTN)]
                    # first four groups run kt-outer interleaved with the
                    # adapter accumulation so the PE issues 8 matmuls per
                    # arriving x chunk instead of 4
                    head_pools = [(pp, "pp"), (psp, "ps"), (psp, "ps"),
                                  (pop, "po")]
                    headacc = [head_pools[i][0].tile([128, c.QT], F32,
                                                     name=f"acc_qk{i}",
                                                     tag=head_pools[i][1])
                               for i in range(4)]
                    LAG = 2
                    for s in range(NKT + LAG):
                        if s < NKT:
                            for qt in range(c.QTN):
                                nc.tensor.matmul(
                                    acc[qt][:, :],
                                    wa_sb[:, s, :],
                                    xsl(s, slice(qt * c.QT, (qt + 1) * c.QT)),
                                    start=(s == 0), stop=(s == NKT - 1))
                        if s >= LAG:
                            kt = s - LAG
                            for i in range(4):
                                dst, w, h, qt = allqk[i]
                                nc.tensor.matmul(
                                    headacc[i][:, :],
                                    w[:, kt, h * 128:(h + 1) * 128],
                                    xsl(kt, slice(qt * c.QT, (qt + 1) * c.QT)),
                                    start=(kt == 0), stop=(kt == NKT - 1))
                    for qt in range(c.QTN):
                        nc.vector.tensor_copy(
                            aqk_sb[:, qt * c.QT:(qt + 1) * c.QT], acc[qt][:, :])
                    nc.sync.dma_start(out=ak_sb[:, :],
                                      in_=aqk_sb[c.RANK:2 * c.RANK, :])
                    for i in range(4):
                        dst, w, h, qt = allqk[i]
                        nc.scalar.copy(dst[h][:, qt * c.QT:(qt + 1) * c.QT],
                                       headacc[i][:, :])
                    for dst, w, h, qt in allqk[4:]:
                        psum = pp.tile([128, c.QT], F32, name="psum_qk", tag="pp")
                        for kt in range(NKT):
                            nc.tensor.matmul(
                                psum[:, :],
                                w[:, kt, h * 128:(h + 1) * 128],
                                xsl(kt, slice(qt * c.QT, (qt + 1) * c.QT)),
                                start=(kt == 0), stop=(kt == NKT - 1))
                        nc.scalar.copy(dst[h][:, qt * c.QT:(qt + 1) * c.QT],
                                       psum[:, :])

                # ---- v projection: [tok, d] natural, moving 512 wide ----
                # RoPE tiles are interleaved into the v loop: the rope chain
                # is DVE/GPSIMD-paced, the v matmuls keep the PE busy
                rope_tiles = [(tiles, h, qt)
                              for tiles in (q_sb, k_sb)
                              for h in range(c.HLOC)
                              for qt in range(c.QTN)]

                def rope_step(i):
                    # q/k head dims use a split re/im layout (host permutes
                    # wq/wk columns and the rope tables; scores are invariant
                    # to a shared d-permutation), so the pair-swap is a
                    # half-swap: two contiguous partition-range DMAs instead
                    # of a PE permute matmul.
                    tiles, h, qt = rope_tiles[i]
                    eng = nc.vector if i % 2 == 0 else nc.gpsimd
                    sl = slice(qt * c.QT, (qt + 1) * c.QT)
                    swp = rtp.tile([128, c.QT], BF16, name="rope_swp",
                                   tag=f"sw{i % 2}")
                    nc.sync.dma_start(out=swp[0:64, :], in_=tiles[h][64:128, sl])
                    nc.sync.dma_start(out=swp[64:128, :], in_=tiles[h][0:64, sl])
                    m1 = rtp.tile([128, c.QT], BF16, name="rope_m1",
                                  tag=f"m1{i % 2}")
                    nc.gpsimd.tensor_mul(m1[:, :], tiles[h][:, sl], c2_sb[:, sl])
                    # sin term lands in place over the swapped copy
                    nc.vector.tensor_mul(swp[:, :], swp[:, :], s2_sb[:, sl])
                    eng.tensor_add(tiles[h][:, sl], m1[:, :], swp[:, :])

                with tc.tile_pool(name="wvp", bufs=1) as wvpool:
                    wv_sb = wvpool.tile([128, c.KTILES, c.DH], BF16,
                                        name="wv_sb", tag="wv")
                    wr = wvT.ap().rearrange("(t p) m -> p t m", p=128)
                    nc.sync.dma_start(out=wv_sb[:, 0:4, :], in_=wr[:, 0:4, :])
                    nc.sync.dma_start(out=wv_sb[:, 4:8, :], in_=wr[:, 4:8, :])
                    nc.sync.dma_start(out=wv_sb[:, 8:16, :], in_=wr[:, 8:16, :])
                    # rope tables arrive while the first v token-blocks run
                    nc.sync.dma_start(out=c2_sb, in_=c2d.ap())
                    nc.sync.dma_start(out=s2_sb, in_=s2d.ap())
                    for tt in range(c.S // 128):
                        psum = pp.tile([128, c.DH], F32, name="psum_v", tag="pp")
                        for kt in range(NKT):
                            nc.tensor.matmul(
                                psum[:, :],
                                xsl(kt, slice(tt * 128, (tt + 1) * 128)),
                                wv_sb[:, kt, :],
                                start=(kt == 0), stop=(kt == NKT - 1))
                        nc.scalar.copy(v_sb[:, tt, :], psum[:, :])
                        rope_step(2 * tt)
                        rope_step(2 * tt + 1)

            # ---- attention + output projection, per 512-query block ----
            with tc.tile_pool(name="wog", bufs=1) as wog, \
                    tc.tile_pool(name="wo_out", bufs=2) as wop:
                mask_sb = wog.tile([128, c.DIAG, c.QT], BF16, name="mask_sb",
                                   tag="mask")
                nc.sync.dma_start(out=mask_sb,
                                  in_=maskdd.ap().rearrange("j p q -> p j q"))
                woc_sb = wog.tile([128, c.HLOC, c.DIM], BF16, name="woc_sb", tag="woc")
                wcr = wocT.ap().rearrange("(h p) j -> p h j", p=128)
                nc.sync.dma_start(out=woc_sb[:, 0:2, :], in_=wcr[:, 0:2, :])
                nc.sync.dma_start(out=woc_sb[:, 2:4, :], in_=wcr[:, 2:4, :])
                # double-buffered per-qt gate tiles r = sigmoid(A)[k, q]
                rg_sb = [wog.tile([128, c.DIAG * c.QTN, c.QT], BF16,
                                  name=f"rg{i}_sb", tag=f"rg{i}") for i in range(2)]

                def gate_step(qt, kt):
                    # r = 1/(1+exp(-A)) via the (shared) Exp table
                    qsl = slice(qt * c.QT, (qt + 1) * c.QT)
                    ksl = slice(kt * c.KT, (kt + 1) * c.KT)
                    pga = psp.tile([128, c.QT], F32, name="pga", tag="ps")
                    nc.tensor.matmul(pga[:, :], ak_sb[:, ksl],
                                     aqk_sb[0:c.RANK, qsl],
                                     start=True, stop=True)
                    ge = pge.tile([128, c.QT], BF16, name="ge", tag="p")
                    nc.scalar.activation(ge[:, :], pga[:, :], AF.Exp, scale=sc_gate)
                    gt = gwk.tile([128, c.QT], F32, name="gt", tag="gt")
                    nc.vector.tensor_scalar_add(gt[:, :], ge[:, :], 1.0)
                    gr = gwk.tile([128, c.QT], F32, name="gr", tag="gr")
                    nc.vector.reciprocal_approx_fast(out=gr[:, :], in_=gt[:, :])
                    nc.gpsimd.tensor_copy(rg_sb[qt % 2][:, kt, :], gr[:, :])

                if c.use_gate and c.use_attn:
                    for kt in range(c.DIAG):
                        gate_step(0, kt)

                for qt in range(c.QTN):
                    qsl = slice(qt * c.QT, (qt + 1) * c.QT)
                    nkt = c.DIAG * (qt + 1)  # causal k tiles
                    rg = rg_sb[qt % 2]

                    og_sb = ogp.tile([128, c.HLOC, c.QT], BF16, name="og_sb", tag="og")
                    if not c.use_attn:
                        nc.vector.memset(og_sb, 0.0)

                    def normalize(h, po, prs):
                        if not c.use_rowsum:
                            nc.vector.tensor_copy(og_sb[:, h, :], po[:, :])
                            return
                        rr = gwk.tile([1, c.QT], F32, name="rr", tag="rr", bufs=1)
                        nc.vector.reciprocal_approx_fast(
                            out=rr[:, :], in_=prs[0:1, :])
                        rrh = gwk.tile([1, c.QT], F16, name="rrh", tag="rrh", bufs=1)
                        nc.vector.tensor_copy(rrh[:, :], rr[:, :])
                        # GPSIMD extended-ISA broadcast: partition 0 -> all
                        rbs = gwk.tile([128, c.QT], F16, name="rbs", tag="rbs", bufs=1)
                        nc.gpsimd.partition_broadcast(rbs[:, :], rrh[0:1, :])
                        nc.vector.tensor_mul(og_sb[:, h, :], po[:, :],
                                             rbs[:, :])

                    # single software pipeline over all (h, kt) score tiles:
                    # drains lag PIPE steps behind scores ACROSS head
                    # boundaries, so head-end drain tails overlap the next
                    # head's score matmuls; normalize(h) is emitted inline
                    # right after head h's last drain
                    po_t, prs_t = {}, {}
                    stage = []  # (h, kt, col-slice, p_or_pm_tile, pgm_tile)

                    def score_step(h, kt):
                        ksl = slice(kt * c.KT, (kt + 1) * c.KT)
                        j = kt - c.DIAG * qt
                        qoff = 128 * j if j > 0 else 0
                        s = slice(qoff, c.QT)
                        qs = slice(qt * c.QT + qoff, (qt + 1) * c.QT)
                        ps = psp.tile([128, c.QT], F32, name="ps", tag="ps")
                        nc.tensor.matmul(ps[:, s], k_sb[h][:, ksl],
                                         q_sb[h][:, qs], start=True, stop=True)
                        p_sb = pge.tile([128, c.QT], BF16, name="p_sb", tag="p")
                        nc.scalar.activation(p_sb[:, s], ps[:, s], AF.Exp,
                                             scale=sc_score)
                        if j >= 0:
                            # diagonal band: 0/1 causal mask after exp,
                            # in place (rowsum needs the masked pre-gate sum)
                            nc.vector.tensor_mul(p_sb[:, s], p_sb[:, s],
                                                 mask_sb[:, j, s])
                        if c.use_gate:
                            pgm = pge.tile([128, c.QT], BF16, name="pgm",
                                           tag="pgm")
                            nc.vector.tensor_mul(pgm[:, s], p_sb[:, s],
                                                 rg[:, kt, s])
                        else:
                            pgm = p_sb
                        stage.append((h, kt, s, p_sb, pgm))

                    def drain_step():
                        h, kt, s, p_sb, pgm = stage.pop(0)
                        # pre-gate rowsum (softmax denominator)
                        if c.use_rowsum:
                            nc.tensor.matmul(prs_t[h][0:1, s],
                                             ones_sb[:, :], p_sb[:, s],
                                             start=(kt == 0),
                                             stop=(kt == nkt - 1),
                                             skip_group_check=True)
                        # out_h^T[d, q] += v[k, d].T @ p_gated[k, q]
                        nc.tensor.matmul(po_t[h][:, s],
                                         v_sb[:, kt, h * 128:(h + 1) * 128],
                                         pgm[:, s],
                                         start=(kt == 0), stop=(kt == nkt - 1),
                                         skip_group_check=True)
                        if kt == nkt - 1:
                            normalize(h, po_t[h], prs_t[h])

                    for h in range(c.HLOC if c.use_attn else 0):
                        po_t[h] = pop.tile([128, c.QT], F32, name="po", tag="po")
                        prs_t[h] = pp.tile([1, c.QT], F32, name="prs", tag="pp")
                        for kt in range(nkt):
                            score_step(h, kt)
                            if len(stage) > c.PIPE:
                                drain_step()
                    while stage:
                        drain_step()

                    # output-projection partial for this query block,
                    # interleaved with the NEXT block's gate generation so the
                    # PE fills the gate chain's latency with wo matmuls
                    nkt2 = (c.DIAG * (qt + 2)
                            if (qt + 1 < c.QTN and c.use_gate and c.use_attn)
                            else 0)
                    ncha = c.DIM // 128
                    f_sb = wop.tile([128, ncha, c.QT], F16, name="f_sb", tag="f")
                    pfpools = [(pp, "pp"), (pop, "po")]
                    # four gate steps lead the wo loop: their matmuls keep
                    # the PE fed while the last head's normalize completes,
                    # and no gate exp lands in the loop's tail where it would
                    # delay the next block's first score exps on ACT
                    for kt in range(min(4, nkt2)):
                        gate_step(qt + 1, kt)
                    for ch in range(ncha if c.use_wo else 0):
                        pfp, pft = pfpools[ch % 2]
                        pf = pfp.tile([128, c.QT], F32, name="pf", tag=pft)
                        for h in range(c.HLOC):
                            nc.tensor.matmul(
                                pf[:, :],
                                woc_sb[:, h, ch * 128:(ch + 1) * 128],
                                og_sb[:, h, :],
                                start=(h == 0), stop=(h == c.HLOC - 1))
                        if ch % 2 == 0 and ch < 12:
                            nc.scalar.copy(f_sb[:, ch, :], pf[:, :])
                        else:
                            nc.vector.tensor_copy(f_sb[:, ch, :], pf[:, :])
                        if ch + 4 < nkt2:
                            gate_step(qt + 1, ch + 4)
                        if qt == c.QTN - 1 and ch % 2 == 1:
                            nc.sync.dma_start(
                                out=pout.ap().rearrange(
                                    "(ch p) q -> p ch q",
                                    p=128)[:, ch - 1:ch + 1, qsl],
                                in_=f_sb[:, ch - 1:ch + 1, :])
                        elif qt < c.QTN - 1 and ch % 4 == 3:
                            # batched output DMA per 4 chunks (0.5MB each):
                            # early chunks fly while later ones compute
                            nc.sync.dma_start(
                                out=pout.ap().rearrange(
                                    "(ch p) q -> p ch q",
                                    p=128)[:, ch - 3:ch + 1, qsl],
                                in_=f_sb[:, ch - 3:ch + 1, :])
                    if not c.use_wo:
                        for kt in range(nkt2):
                            gate_step(qt + 1, kt)

        if c.repeat > 1:
            with tc.For_i(0, c.repeat, 1,
                          hint_engines=(mybir.EngineType.PE,
                                        mybir.EngineType.DVE,
                                        mybir.EngineType.Activation,
                                        mybir.EngineType.Pool,
                                        mybir.EngineType.SP)):
                body()
        else:
            body()

    nc.compile()
    return nc


def make_core_inputs(inputs: dict, cfg: Cfg = FULL):
    """Host-side sharding: returns in_maps (one dict per core)."""
    c = cfg
    bf16 = ml_dtypes.bfloat16
    x = np.asarray(inputs["x"])
    mask = np.asarray(inputs["mask"])
    fc = np.asarray(inputs["freqs_cos"])
    fs = np.asarray(inputs["freqs_sin"])
    wq, wk, wv, wo = (np.asarray(inputs[k]) for k in ("wq", "wk", "wv", "wo"))
    wa_q, wa_k = np.asarray(inputs["wa_q"]), np.asarray(inputs["wa_k"])

    xTb = [np.ascontiguousarray(x[b].T).astype(bf16) for b in range(c.B)]
    waT = np.ascontiguousarray(
        np.concatenate([wa_q, wa_k], axis=0).T).astype(bf16)

    # rope tables in [d, tok] layout, split re/im: rows 0:64 = re lanes,
    # 64:128 = im lanes (wq/wk columns are permuted to match; the score
    # dot-product is invariant to a shared head-dim permutation)
    c2 = np.empty((c.HD, c.S), np.float32)
    s2 = np.empty((c.HD, c.S), np.float32)
    c2[0:64] = fc.T
    c2[64:128] = fc.T
    s2[0:64] = -fs.T
    s2[64:128] = fs.T
    c2 = c2.astype(bf16)
    s2 = s2.astype(bf16)

    # per-head column permutation: even (re) dims first, odd (im) second
    dperm = np.concatenate([np.arange(0, c.HD, 2), np.arange(1, c.HD, 2)])
    qkperm = np.concatenate([hb * c.HD + dperm
                             for hb in range(c.DIM // c.HD)])

    # diagonal-band mask patterns [j][k, q], extracted from the input mask
    qt_last = c.QTN - 1
    q0 = qt_last * c.QT
    maskd = np.empty((c.DIAG, c.KT, c.QT), np.float32)
    for j in range(c.DIAG):
        k0 = (c.DIAG * qt_last + j) * c.KT
        maskd[j] = (mask[0, 0, q0:q0 + c.QT, k0:k0 + c.KT].T == 0.0)
    maskd = maskd.astype(bf16)

    wslices = []
    for hs in range(c.CPG):
        rows = slice(hs * c.DH, (hs + 1) * c.DH)
        wslices.append({
            "wqT": np.ascontiguousarray(wq[qkperm][rows].T).astype(bf16),
            "wkT": np.ascontiguousarray(wk[qkperm][rows].T).astype(bf16),
            "wvT": np.ascontiguousarray(wv[rows].T).astype(bf16),
            "wocT": np.ascontiguousarray(wo[:, rows].T).astype(bf16),
        })

    in_maps = []
    for ci in range(c.NCORES):
        b = ci // c.CPG
        hs = ci % c.CPG
        in_maps.append({
            "xT": xTb[b],
            **wslices[hs],
            "waT": waT,
            "c2d": c2,
            "s2d": s2,
            "maskdd": maskd,
        })
    return in_maps


def assemble_output(results, cfg: Cfg = FULL) -> np.ndarray:
    c = cfg
    out = np.empty((c.B, c.S, c.DIM), np.float32)
    for b in range(c.B):
        total = np.zeros((c.DIM, c.S), np.float32)
        for hs in range(c.CPG):
            total += np.asarray(results[b * c.CPG + hs]["pout"]).astype(np.float32)
        out[b] = total.T
    return out


_NC_CACHE = {}


def run(nc, in_maps, trace=False, cfg: Cfg = FULL, **kw):
    return bass_utils.run_bass_kernel_spmd(
        nc, in_maps, core_ids=list(range(cfg.NCORES)), trace=trace, **kw)


def kernel(**inputs) -> np.ndarray:
    cfg = FULL
    if cfg not in _NC_CACHE:
        _NC_CACHE[cfg] = build_nc(cfg)
    nc = _NC_CACHE[cfg]
    in_maps = make_core_inputs(inputs, cfg)
    res = run(nc, in_maps, cfg=cfg)
    return assemble_output(res.results, cfg)


if __name__ == "__main__":
    nc = build_nc(FULL)
    print("built ok")


# revision 26
# speedup vs baseline: 1.0350x; 1.0077x over previous
"""Trainium2 Bass kernel for nn_Attention_10771777978404 (sparse_attention).

Sharding over 8 NeuronCores: 2 batch-groups x 4 cores (tensor parallel over
heads within each batch group).
  - core ci handles batch ci//4 and heads [4*(ci%4), 4*(ci%4)+4): it computes
    its q/k/v projections (columns of wq/wk/wv), RoPE, causal attention with
    the low-rank sigmoid gate, and a full-width partial of the output
    projection from its 4 heads (rows of wo).
  - the rank-32 adapter weights are replicated inside each batch group; the
    sigmoid gate is computed as 1/(1+exp(-A)) so the scalar engine only ever
    uses the Exp table (no Sigmoid-table reloads, no DRAM staging).
  - host sums the 4 partial output projections per batch (fp16 partials).

Everything on-device is bf16 with fp32 PSUM accumulation.

Schedule notes (v6, ~507us on HW vs 603us baseline):
  - diagonal-band tiles only compute the live query columns [128j:512]
    (causal wedge), cutting PE/ACT/DVE work on the band by ~37%; the causal
    mask is applied in place on the exp tile.
  - attention runs as ONE flat software pipeline over all (h, kt) tiles per
    query block: rowsum/AV drains lag the score matmuls by PIPE=8 steps
    ACROSS head boundaries, and normalize(h) is emitted inline after head
    h's last drain. Per-head denominators rotate through the pp PSUM pool
    (2 banks) so rowsum accumulation never WARs the previous head.
  - RoPE uses a split re/im head-dim layout (host permutes wq/wk columns
    and the rope tables; scores are invariant to a shared d-permutation),
    so the pair-swap is two contiguous partition-range DMAs instead of a
    PE permute matmul; the softmax 1/sum broadcast runs on the GPSIMD
    extended-ISA partition_broadcast instead of a ones-matmul. Both cuts
    matter because each matmul carries a ~55ns serial Ldweights cost.
  - the wo partial-projection PSUM tiles rotate over 4 banks (pp+po pools);
    PSUM->SBUF copies alternate ACT/DVE but the loop tail is DVE-only and
    gate generation leads by 4 chunks, so the ACT queue is drained when the
    next block's score exps arrive; output DMAs are batched 4 chunks per
    dma_start (HWDGE descriptor generation is a serial ~630ns/dma).
  - input DMAs are ordered by first use with the adapter matmuls leading
    the head matmuls by 2 kt steps, so the projection sweep starts on x
    alone while wq/wk stream in behind.

self-contained: hardcodes the problem shapes; only needs `concourse` (on
PYTHONPATH in this container) + jax axon devices.
"""

import math
from contextlib import ExitStack
from dataclasses import dataclass

import numpy as np
import ml_dtypes

import concourse.tile as tile
from concourse import bacc
from concourse import mybir
from concourse import bass_utils

BF16 = mybir.dt.bfloat16
F16 = mybir.dt.float16
F32 = mybir.dt.float32
AF = mybir.ActivationFunctionType


@dataclass(frozen=True)
class Cfg:
    B: int = 2
    S: int = 2048
    DIM: int = 2048
    NH: int = 16
    HD: int = 128
    RANK: int = 32
    NCORES: int = 8
    GROUPS: int = 2     # batch groups of 4 cores
    QT: int = 512       # query block (free dim of score tiles)
    KT: int = 128       # key block (partition dim of score tiles)
    PIPE: int = 8       # score tiles in flight ahead of rowsum/AV drains
    repeat: int = 1     # hardware-loop repetitions of the whole body (timing)
    # ablation flags (profiling on hardware; all True for the real kernel)
    use_gate: bool = True
    use_rowsum: bool = True
    use_attn: bool = True
    use_wo: bool = True

    @property
    def CPG(self):
        return self.NCORES // self.GROUPS  # cores per batch group

    @property
    def HLOC(self):
        return self.NH // self.CPG  # heads per core (4)

    @property
    def DH(self):
        return self.HLOC * self.HD  # per-core head-dim span (512)

    @property
    def KTILES(self):
        return self.DIM // 128  # contraction tiles for projections

    @property
    def QTN(self):
        return self.S // self.QT

    @property
    def DIAG(self):
        return self.QT // self.KT  # k-tiles per diagonal band


FULL = Cfg()


def build_nc(cfg: Cfg = FULL):
    c = cfg
    assert c.HD == 128 and c.KT == 128
    nc = bacc.Bacc("TRN2", target_bir_lowering=False, debug=False,
                   num_devices=c.NCORES)

    # ---- kernel I/O (per core: one batch, HLOC heads) ----
    xT = nc.dram_tensor("xT", [c.DIM, c.S], BF16, kind="ExternalInput")
    wqT = nc.dram_tensor("wqT", [c.DIM, c.DH], BF16, kind="ExternalInput")
    wkT = nc.dram_tensor("wkT", [c.DIM, c.DH], BF16, kind="ExternalInput")
    wvT = nc.dram_tensor("wvT", [c.DIM, c.DH], BF16, kind="ExternalInput")
    wocT = nc.dram_tensor("wocT", [c.DH, c.DIM], BF16, kind="ExternalInput")
    waT = nc.dram_tensor("waT", [c.DIM, 2 * c.RANK], BF16, kind="ExternalInput")
    c2d = nc.dram_tensor("c2d", [c.HD, c.S], BF16, kind="ExternalInput")
    s2d = nc.dram_tensor("s2d", [c.HD, c.S], BF16, kind="ExternalInput")
    maskdd = nc.dram_tensor("maskdd", [c.DIAG, c.KT, c.QT], BF16, kind="ExternalInput")

    # partial output projection, transposed: pout[j, t] (fp16; host sums the
    # 4 partials of each batch group in fp32)
    pout = nc.dram_tensor("pout", [c.DIM, c.S], F16, kind="ExternalOutput")

    isqrt = 1.0 / math.sqrt(c.HD)
    sc_score = isqrt
    sc_gate = -1.0
    NKT = c.KTILES

    with ExitStack() as _ctx:
        tc = _ctx.enter_context(tile.TileContext(nc))
        # persistent pools (whole-iteration lifetime)
        cst = _ctx.enter_context(tc.tile_pool(name="const", bufs=1))
        adp = _ctx.enter_context(tc.tile_pool(name="ap", bufs=1))
        qkp = _ctx.enter_context(tc.tile_pool(name="qk", bufs=1))
        vp = _ctx.enter_context(tc.tile_pool(name="vp", bufs=1))
        rtp = _ctx.enter_context(tc.tile_pool(name="rope_t", bufs=1))
        pge = _ctx.enter_context(tc.tile_pool(name="pge", bufs=9))
        gwk = _ctx.enter_context(tc.tile_pool(name="gwk", bufs=2))
        ogp = _ctx.enter_context(tc.tile_pool(name="og", bufs=1))
        # PSUM pools (8 banks total): pp 2 + ps 4 + po 2; the per-head
        # softmax-denominator tiles and the norm-broadcast tiles share the
        # pp rotation so rowsum accumulation never WARs the previous head
        pp = _ctx.enter_context(tc.tile_pool(name="pp", bufs=2, space="PSUM"))
        psp = _ctx.enter_context(tc.tile_pool(name="ps", bufs=4, space="PSUM"))
        pop = _ctx.enter_context(tc.tile_pool(name="po", bufs=2, space="PSUM"))

        def body():
            # ---- constants ----
            c2_sb = cst.tile([128, c.S], BF16, name="c2_sb", tag="c2")
            s2_sb = cst.tile([128, c.S], BF16, name="s2_sb", tag="s2")
            ones_sb = cst.tile([128, 1], BF16, name="ones_sb", tag="ones")

            # packed adapter projections: one [64,512] matmul computes both
            # aq and ak rows (halves the adapter matmul count); ak is then
            # moved to partition base 0 with one SBUF->SBUF DMA (engines
            # cannot shift partitions, DMA can)
            aqk_sb = adp.tile([2 * c.RANK, c.S], BF16, name="aqk_sb", tag="aqk")
            ak_sb = adp.tile([c.RANK, c.S], BF16, name="ak_sb", tag="ak")
            q_sb = [qkp.tile([128, c.S], BF16, name=f"q{h}_sb", tag=f"q{h}")
                    for h in range(c.HLOC)]
            k_sb = [qkp.tile([128, c.S], BF16, name=f"k{h}_sb", tag=f"k{h}")
                    for h in range(c.HLOC)]
            v_sb = vp.tile([128, c.S // 128, c.DH], BF16, name="v_sb", tag="v")
            wv0_sb = vp.tile([128, 4, c.DH], BF16, name="wv0_sb", tag="wv0")

            with tc.tile_pool(name="xtp", bufs=1) as xtp:
                wa_sb = xtp.tile([128, c.KTILES, 2 * c.RANK], BF16,
                                 name="wa_sb", tag="wa")
                nc.sync.dma_start(out=wa_sb,
                                  in_=waT.ap().rearrange("(t p) m -> p t m", p=128))
                xt_sb = xtp.tile([128, c.KTILES, c.S], BF16, name="xt_sb", tag="xt")
                xr = xT.ap().rearrange("(t p) n -> p t n", p=128)

                def xsl(j, sl):
                    return xt_sb[:, j, sl]

                nc.vector.memset(ones_sb, 1.0)

                # ---- adapter + q/k projections, kt-outer, 8 live psum accs ----
                apools = [pp, psp, pop, psp]
                acc = [apools[i].tile([2 * c.RANK, c.QT], F32, name=f"acc_a{i}",
                                      tag=("pp", "ps", "po", "ps")[i])
                       for i in range(c.QTN)]

                wvpool = None
                with tc.tile_pool(name="wqk", bufs=1) as wp:
                    wq_sb = wp.tile([128, c.KTILES, c.DH], BF16, name="wq_sb", tag="wq")
                    wk_sb = wp.tile([128, c.KTILES, c.DH], BF16, name="wk_sb", tag="wk")
                    # input DMAs ordered by first use; the adapter matmuls
                    # lead the head matmuls by 2 kt steps, so the sweep can
                    # start on x alone while wq/wk stream in behind
                    nc.sync.dma_start(out=xt_sb[:, 0, :], in_=xr[:, 0, :])
                    nc.sync.dma_start(out=xt_sb[:, 1, :], in_=xr[:, 1, :])
                    wqr = wqT.ap().rearrange("(t p) m -> p t m", p=128)
                    wkr = wkT.ap().rearrange("(t p) m -> p t m", p=128)
                    nc.scalar.dma_start(out=wq_sb[:, 0:2, :], in_=wqr[:, 0:2, :])
                    nc.scalar.dma_start(out=wk_sb[:, 0:2, :], in_=wkr[:, 0:2, :])
                    nc.sync.dma_start(out=xt_sb[:, 2:4, :], in_=xr[:, 2:4, :])
                    nc.scalar.dma_start(out=wq_sb[:, 2:6, :], in_=wqr[:, 2:6, :])
                    nc.scalar.dma_start(out=wk_sb[:, 2:6, :], in_=wkr[:, 2:6, :])
                    nc.sync.dma_start(out=xt_sb[:, 4:8, :], in_=xr[:, 4:8, :])
                    nc.scalar.dma_start(out=wq_sb[:, 6:, :], in_=wqr[:, 6:, :])
                    nc.scalar.dma_start(out=wk_sb[:, 6:, :], in_=wkr[:, 6:, :])
                    nc.sync.dma_start(out=xt_sb[:, 8:12, :], in_=xr[:, 8:12, :])
                    nc.sync.dma_start(out=xt_sb[:, 12:16, :], in_=xr[:, 12:16, :])

                    allqk = [(dst, w, h, qt)
                             for dst, w in ((q_sb, wq_sb), (k_sb, wk_sb))
                             for h in range(c.HLOC)
                             for qt in range(c.QTN)]
                    # first four groups run kt-outer interleaved with the
                    # adapter accumulation so the PE issues 8 matmuls per
                    # arriving x chunk instead of 4
                    head_pools = [(pp, "pp"), (psp, "ps"), (psp, "ps"),
                                  (pop, "po")]
                    headacc = [head_pools[i][0].tile([128, c.QT], F32,
                                                     name=f"acc_qk{i}",
                                                     tag=head_pools[i][1])
                               for i in range(4)]
                    LAG = 2
                    for s in range(NKT + LAG):
                        if s < NKT:
                            for qt in range(c.QTN):
                                nc.tensor.matmul(
                                    acc[qt][:, :],
                                    wa_sb[:, s, :],
                                    xsl(s, slice(qt * c.QT, (qt + 1) * c.QT)),
                                    start=(s == 0), stop=(s == NKT - 1))
                        if s >= LAG:
                            kt = s - LAG
                            for i in range(4):
                                dst, w, h, qt = allqk[i]
                                nc.tensor.matmul(
                                    headacc[i][:, :],
                                    w[:, kt, h * 128:(h + 1) * 128],
                                    xsl(kt, slice(qt * c.QT, (qt + 1) * c.QT)),
                                    start=(kt == 0), stop=(kt == NKT - 1))
                    for qt in range(c.QTN):
                        nc.vector.tensor_copy(
                            aqk_sb[:, qt * c.QT:(qt + 1) * c.QT], acc[qt][:, :])
                    nc.sync.dma_start(out=ak_sb[:, :],
                                      in_=aqk_sb[c.RANK:2 * c.RANK, :])
                    for i in range(4):
                        dst, w, h, qt = allqk[i]
                        nc.scalar.copy(dst[h][:, qt * c.QT:(qt + 1) * c.QT],
                                       headacc[i][:, :])
                    wr = wvT.ap().rearrange("(t p) m -> p t m", p=128)
                    for gi, (dst, w, h, qt) in enumerate(allqk[4:]):
                        if gi == 24:
                            # prefetch the first wv chunks so the v loop's
                            # first token-block starts without a DMA wait
                            nc.sync.dma_start(out=wv0_sb, in_=wr[:, 0:4, :])
                        psum = pp.tile([128, c.QT], F32, name="psum_qk", tag="pp")
                        for kt in range(NKT):
                            nc.tensor.matmul(
                                psum[:, :],
                                w[:, kt, h * 128:(h + 1) * 128],
                                xsl(kt, slice(qt * c.QT, (qt + 1) * c.QT)),
                                start=(kt == 0), stop=(kt == NKT - 1))
                        nc.scalar.copy(dst[h][:, qt * c.QT:(qt + 1) * c.QT],
                                       psum[:, :])

                # ---- v projection: [tok, d] natural, moving 512 wide ----
                # RoPE tiles are interleaved into the v loop: the rope chain
                # is DVE/GPSIMD-paced, the v matmuls keep the PE busy
                rope_tiles = [(tiles, h, qt)
                              for tiles in (q_sb, k_sb)
                              for h in range(c.HLOC)
                              for qt in range(c.QTN)]

                def rope_step(i):
                    # q/k head dims use a split re/im layout (host permutes
                    # wq/wk columns and the rope tables; scores are invariant
                    # to a shared d-permutation), so the pair-swap is a
                    # half-swap: two contiguous partition-range DMAs instead
                    # of a PE permute matmul.
                    tiles, h, qt = rope_tiles[i]
                    eng = nc.vector if i % 2 == 0 else nc.gpsimd
                    sl = slice(qt * c.QT, (qt + 1) * c.QT)
                    swp = rtp.tile([128, c.QT], BF16, name="rope_swp",
                                   tag=f"sw{i % 2}")
                    nc.sync.dma_start(out=swp[0:64, :], in_=tiles[h][64:128, sl])
                    nc.sync.dma_start(out=swp[64:128, :], in_=tiles[h][0:64, sl])
                    m1 = rtp.tile([128, c.QT], BF16, name="rope_m1",
                                  tag=f"m1{i % 2}")
                    nc.gpsimd.tensor_mul(m1[:, :], tiles[h][:, sl], c2_sb[:, sl])
                    # sin term lands in place over the swapped copy
                    nc.vector.tensor_mul(swp[:, :], swp[:, :], s2_sb[:, sl])
                    eng.tensor_add(tiles[h][:, sl], m1[:, :], swp[:, :])

                with tc.tile_pool(name="wvp", bufs=1) as wvpool:
                    wv_sb = wvpool.tile([128, c.KTILES - 4, c.DH], BF16,
                                        name="wv_sb", tag="wv")
                    nc.sync.dma_start(out=wv_sb[:, 0:4, :], in_=wr[:, 4:8, :])
                    nc.sync.dma_start(out=wv_sb[:, 4:12, :], in_=wr[:, 8:16, :])
                    # rope tables arrive while the first v token-blocks run
                    nc.sync.dma_start(out=c2_sb, in_=c2d.ap())
                    nc.sync.dma_start(out=s2_sb, in_=s2d.ap())
                    for tt in range(c.S // 128):
                        psum = pp.tile([128, c.DH], F32, name="psum_v", tag="pp")
                        for kt in range(NKT):
                            wsrc = (wv0_sb[:, kt, :] if kt < 4
                                    else wv_sb[:, kt - 4, :])
                            nc.tensor.matmul(
                                psum[:, :],
                                xsl(kt, slice(tt * 128, (tt + 1) * 128)),
                                wsrc,
                                start=(kt == 0), stop=(kt == NKT - 1))
                        nc.scalar.copy(v_sb[:, tt, :], psum[:, :])
                        rope_step(2 * tt)
                        rope_step(2 * tt + 1)

            # ---- attention + output projection, per 512-query block ----
            with tc.tile_pool(name="wog", bufs=1) as wog, \
                    tc.tile_pool(name="wo_out", bufs=2) as wop:
                mask_sb = wog.tile([128, c.DIAG, c.QT], BF16, name="mask_sb",
                                   tag="mask")
                nc.sync.dma_start(out=mask_sb,
                                  in_=maskdd.ap().rearrange("j p q -> p j q"))
                woc_sb = wog.tile([128, c.HLOC, c.DIM], BF16, name="woc_sb", tag="woc")
                wcr = wocT.ap().rearrange("(h p) j -> p h j", p=128)
                nc.sync.dma_start(out=woc_sb[:, 0:2, :], in_=wcr[:, 0:2, :])
                nc.sync.dma_start(out=woc_sb[:, 2:4, :], in_=wcr[:, 2:4, :])
                # double-buffered per-qt gate tiles r = sigmoid(A)[k, q]
                rg_sb = [wog.tile([128, c.DIAG * c.QTN, c.QT], BF16,
                                  name=f"rg{i}_sb", tag=f"rg{i}") for i in range(2)]

                def gate_step(qt, kt):
                    # r = 1/(1+exp(-A)) via the (shared) Exp table
                    qsl = slice(qt * c.QT, (qt + 1) * c.QT)
                    ksl = slice(kt * c.KT, (kt + 1) * c.KT)
                    pga = psp.tile([128, c.QT], F32, name="pga", tag="ps")
                    nc.tensor.matmul(pga[:, :], ak_sb[:, ksl],
                                     aqk_sb[0:c.RANK, qsl],
                                     start=True, stop=True)
                    ge = pge.tile([128, c.QT], BF16, name="ge", tag="p")
                    nc.scalar.activation(ge[:, :], pga[:, :], AF.Exp, scale=sc_gate)
                    gt = gwk.tile([128, c.QT], F32, name="gt", tag="gt")
                    nc.vector.tensor_scalar_add(gt[:, :], ge[:, :], 1.0)
                    gr = gwk.tile([128, c.QT], F32, name="gr", tag="gr")
                    nc.vector.reciprocal_approx_fast(out=gr[:, :], in_=gt[:, :])
                    nc.gpsimd.tensor_copy(rg_sb[qt % 2][:, kt, :], gr[:, :])

                if c.use_gate and c.use_attn:
                    for kt in range(c.DIAG):
                        gate_step(0, kt)

                for qt in range(c.QTN):
                    qsl = slice(qt * c.QT, (qt + 1) * c.QT)
                    nkt = c.DIAG * (qt + 1)  # causal k tiles
                    rg = rg_sb[qt % 2]

                    og_sb = ogp.tile([128, c.HLOC, c.QT], BF16, name="og_sb", tag="og")
                    if not c.use_attn:
                        nc.vector.memset(og_sb, 0.0)

                    def normalize(h, po, prs):
                        if not c.use_rowsum:
                            nc.vector.tensor_copy(og_sb[:, h, :], po[:, :])
                            return
                        rr = gwk.tile([1, c.QT], F32, name="rr", tag="rr", bufs=1)
                        nc.vector.reciprocal_approx_fast(
                            out=rr[:, :], in_=prs[0:1, :])
                        rrh = gwk.tile([1, c.QT], F16, name="rrh", tag="rrh", bufs=1)
                        nc.vector.tensor_copy(rrh[:, :], rr[:, :])
                        # GPSIMD extended-ISA broadcast: partition 0 -> all
                        rbs = gwk.tile([128, c.QT], F16, name="rbs", tag="rbs", bufs=1)
                        nc.gpsimd.partition_broadcast(rbs[:, :], rrh[0:1, :])
                        nc.vector.tensor_mul(og_sb[:, h, :], po[:, :],
                                             rbs[:, :])

                    # single software pipeline over all (h, kt) score tiles:
                    # drains lag PIPE steps behind scores ACROSS head
                    # boundaries, so head-end drain tails overlap the next
                    # head's score matmuls; normalize(h) is emitted inline
                    # right after head h's last drain
                    po_t, prs_t = {}, {}
                    stage = []  # (h, kt, col-slice, p_or_pm_tile, pgm_tile)

                    def score_step(h, kt):
                        ksl = slice(kt * c.KT, (kt + 1) * c.KT)
                        j = kt - c.DIAG * qt
                        qoff = 128 * j if j > 0 else 0
                        s = slice(qoff, c.QT)
                        qs = slice(qt * c.QT + qoff, (qt + 1) * c.QT)
                        ps = psp.tile([128, c.QT], F32, name="ps", tag="ps")
                        nc.tensor.matmul(ps[:, s], k_sb[h][:, ksl],
                                         q_sb[h][:, qs], start=True, stop=True)
                        p_sb = pge.tile([128, c.QT], BF16, name="p_sb", tag="p")
                        nc.scalar.activation(p_sb[:, s], ps[:, s], AF.Exp,
                                             scale=sc_score)
                        if j >= 0:
                            # diagonal band: 0/1 causal mask after exp,
                            # in place (rowsum needs the masked pre-gate sum)
                            nc.vector.tensor_mul(p_sb[:, s], p_sb[:, s],
                                                 mask_sb[:, j, s])
                        if c.use_gate:
                            pgm = pge.tile([128, c.QT], BF16, name="pgm",
                                           tag="pgm")
                            nc.vector.tensor_mul(pgm[:, s], p_sb[:, s],
                                                 rg[:, kt, s])
                        else:
                            pgm = p_sb
                        stage.append((h, kt, s, p_sb, pgm))

                    def drain_step():
                        h, kt, s, p_sb, pgm = stage.pop(0)
                        # pre-gate rowsum (softmax denominator)
                        if c.use_rowsum:
                            nc.tensor.matmul(prs_t[h][0:1, s],
                                             ones_sb[:, :], p_sb[:, s],
                                             start=(kt == 0),
                                             stop=(kt == nkt - 1),
                                             skip_group_check=True)
                        # out_h^T[d, q] += v[k, d].T @ p_gated[k, q]
                        nc.tensor.matmul(po_t[h][:, s],
                                         v_sb[:, kt, h * 128:(h + 1) * 128],
                                         pgm[:, s],
                                         start=(kt == 0), stop=(kt == nkt - 1),
                                         skip_group_check=True)
                        if kt == nkt - 1:
                            normalize(h, po_t[h], prs_t[h])

                    for h in range(c.HLOC if c.use_attn else 0):
                        po_t[h] = pop.tile([128, c.QT], F32, name="po", tag="po")
                        prs_t[h] = pp.tile([1, c.QT], F32, name="prs", tag="pp")
                        for kt in range(nkt):
                            score_step(h, kt)
                            if len(stage) > c.PIPE:
                                drain_step()
                    while stage:
                        drain_step()

                    # output-projection partial for this query block,
                    # interleaved with the NEXT block's gate generation so the
                    # PE fills the gate chain's latency with wo matmuls
                    nkt2 = (c.DIAG * (qt + 2)
                            if (qt + 1 < c.QTN and c.use_gate and c.use_attn)
                            else 0)
                    ncha = c.DIM // 128
                    f_sb = wop.tile([128, ncha, c.QT], F16, name="f_sb", tag="f")
                    pfpools = [(pp, "pp"), (pop, "po")]
                    # four gate steps lead the wo loop: their matmuls keep
                    # the PE fed while the last head's normalize completes,
                    # and no gate exp lands in the loop's tail where it would
                    # delay the next block's first score exps on ACT
                    for kt in range(min(4, nkt2)):
                        gate_step(qt + 1, kt)
                    for ch in range(ncha if c.use_wo else 0):
                        pfp, pft = pfpools[ch % 2]
                        pf = pfp.tile([128, c.QT], F32, name="pf", tag=pft)
                        for h in range(c.HLOC):
                            nc.tensor.matmul(
                                pf[:, :],
                                woc_sb[:, h, ch * 128:(ch + 1) * 128],
                                og_sb[:, h, :],
                                start=(h == 0), stop=(h == c.HLOC - 1))
                        if ch % 2 == 0 and ch < 12:
                            nc.scalar.copy(f_sb[:, ch, :], pf[:, :])
                        else:
                            nc.vector.tensor_copy(f_sb[:, ch, :], pf[:, :])
                        if ch + 4 < nkt2:
                            gate_step(qt + 1, ch + 4)
                        if qt == c.QTN - 1 and ch % 2 == 1:
                            nc.sync.dma_start(
                                out=pout.ap().rearrange(
                                    "(ch p) q -> p ch q",
                                    p=128)[:, ch - 1:ch + 1, qsl],
                                in_=f_sb[:, ch - 1:ch + 1, :])
                        elif qt < c.QTN - 1 and ch % 4 == 3:
                            # batched output DMA per 4 chunks (0.5MB each):
                            # early chunks fly while later ones compute
                            nc.sync.dma_start(
                                out=pout.ap().rearrange(
                                    "(ch p) q -> p ch q",
                                    p=128)[:, ch - 3:ch + 1, qsl],
                                in_=f_sb[:, ch - 3:ch + 1, :])
                    if not c.use_wo:
                        for kt in range(nkt2):
                            gate_step(qt + 1, kt)

        if c.repeat > 1:
            with tc.For_i(0, c.repeat, 1,
                          hint_engines=(mybir.EngineType.PE,
                                        mybir.EngineType.DVE,
                                        mybir.EngineType.Activation,
                                        mybir.EngineType.Pool,
                                        mybir.EngineType.SP)):
                body()
        else:
            body()

    nc.compile()
    return nc


def make_core_inputs(inputs: dict, cfg: Cfg = FULL):
    """Host-side sharding: returns in_maps (one dict per core)."""
    c = cfg
    bf16 = ml_dtypes.bfloat16
    x = np.asarray(inputs["x"])
    mask = np.asarray(inputs["mask"])
    fc = np.asarray(inputs["freqs_cos"])
    fs = np.asarray(inputs["freqs_sin"])
    wq, wk, wv, wo = (np.asarray(inputs[k]) for k in ("wq", "wk", "wv", "wo"))
    wa_q, wa_k = np.asarray(inputs["wa_q"]), np.asarray(inputs["wa_k"])

    xTb = [np.ascontiguousarray(x[b].T).astype(bf16) for b in range(c.B)]
    waT = np.ascontiguousarray(
        np.concatenate([wa_q, wa_k], axis=0).T).astype(bf16)

    # rope tables in [d, tok] layout, split re/im: rows 0:64 = re lanes,
    # 64:128 = im lanes (wq/wk columns are permuted to match; the score
    # dot-product is invariant to a shared head-dim permutation)
    c2 = np.empty((c.HD, c.S), np.float32)
    s2 = np.empty((c.HD, c.S), np.float32)
    c2[0:64] = fc.T
    c2[64:128] = fc.T
    s2[0:64] = -fs.T
    s2[64:128] = fs.T
    c2 = c2.astype(bf16)
    s2 = s2.astype(bf16)

    # per-head column permutation: even (re) dims first, odd (im) second
    dperm = np.concatenate([np.arange(0, c.HD, 2), np.arange(1, c.HD, 2)])
    qkperm = np.concatenate([hb * c.HD + dperm
                             for hb in range(c.DIM // c.HD)])

    # diagonal-band mask patterns [j][k, q], extracted from the input mask
    qt_last = c.QTN - 1
    q0 = qt_last * c.QT
    maskd = np.empty((c.DIAG, c.KT, c.QT), np.float32)
    for j in range(c.DIAG):
        k0 = (c.DIAG * qt_last + j) * c.KT
        maskd[j] = (mask[0, 0, q0:q0 + c.QT, k0:k0 + c.KT].T == 0.0)
    maskd = maskd.astype(bf16)

    wslices = []
    for hs in range(c.CPG):
        rows = slice(hs * c.DH, (hs + 1) * c.DH)
        wslices.append({
            "wqT": np.ascontiguousarray(wq[qkperm][rows].T).astype(bf16),
            "wkT": np.ascontiguousarray(wk[qkperm][rows].T).astype(bf16),
            "wvT": np.ascontiguousarray(wv[rows].T).astype(bf16),
            "wocT": np.ascontiguousarray(wo[:, rows].T).astype(bf16),
        })

    in_maps = []
    for ci in range(c.NCORES):
        b = ci // c.CPG
        hs = ci % c.CPG
        in_maps.append({
            "xT": xTb[b],
            **wslices[hs],
            "waT": waT,
            "c2d": c2,
            "s2d": s2,
            "maskdd": maskd,
        })
    return in_maps


def assemble_output(results, cfg: Cfg = FULL) -> np.ndarray:
    c = cfg
    out = np.empty((c.B, c.S, c.DIM), np.float32)
    for b in range(c.B):
        total = np.zeros((c.DIM, c.S), np.float32)
        for hs in range(c.CPG):
            total += np.asarray(results[b * c.CPG + hs]["pout"]).astype(np.float32)
        out[b] = total.T
    return out


_NC_CACHE = {}


def run(nc, in_maps, trace=False, cfg: Cfg = FULL, **kw):
    return bass_utils.run_bass_kernel_spmd(
        nc, in_maps, core_ids=list(range(cfg.NCORES)), trace=trace, **kw)


def kernel(**inputs) -> np.ndarray:
    cfg = FULL
    if cfg not in _NC_CACHE:
        _NC_CACHE[cfg] = build_nc(cfg)
    nc = _NC_CACHE[cfg]
    in_maps = make_core_inputs(inputs, cfg)
    res = run(nc, in_maps, cfg=cfg)
    return assemble_output(res.results, cfg)


if __name__ == "__main__":
    nc = build_nc(FULL)
    print("built ok")


# revision 30
# speedup vs baseline: 1.0417x; 1.0065x over previous
"""Trainium2 Bass kernel for nn_Attention_10771777978404 (sparse_attention).

Sharding over 8 NeuronCores: 2 batch-groups x 4 cores (tensor parallel over
heads within each batch group).
  - core ci handles batch ci//4 and heads [4*(ci%4), 4*(ci%4)+4): it computes
    its q/k/v projections (columns of wq/wk/wv), RoPE, causal attention with
    the low-rank sigmoid gate, and a full-width partial of the output
    projection from its 4 heads (rows of wo).
  - the rank-32 adapter weights are replicated inside each batch group; the
    sigmoid gate is computed as 1/(1+exp(-A)) so the scalar engine only ever
    uses the Exp table (no Sigmoid-table reloads, no DRAM staging).
  - host sums the 4 partial output projections per batch (fp16 partials).

Everything on-device is bf16 with fp32 PSUM accumulation.

Schedule notes (v7, ~503us on HW vs 603us baseline):
  - diagonal-band tiles only compute the live query columns [128j:512]
    (causal wedge), cutting PE/ACT/DVE work on the band by ~37%; the causal
    mask is applied in place on the exp tile.
  - attention runs as ONE flat software pipeline over all (h, kt) tiles per
    query block: rowsum/AV drains lag the score matmuls by PIPE=8 steps
    ACROSS head boundaries, and normalize(h) is emitted inline after head
    h's last drain. Per-head denominators rotate through the pp PSUM pool
    (2 banks) so rowsum accumulation never WARs the previous head.
  - RoPE uses a split re/im head-dim layout (host permutes wq/wk columns
    and the rope tables; scores are invariant to a shared d-permutation),
    so the pair-swap is two contiguous partition-range DMAs instead of a
    PE permute matmul; the softmax 1/sum broadcast runs on the GPSIMD
    extended-ISA partition_broadcast instead of a ones-matmul. Both cuts
    matter because each matmul carries a ~55ns serial Ldweights cost.
  - the wo partial-projection PSUM tiles rotate over 4 banks (pp+po pools);
    PSUM->SBUF copies alternate ACT/DVE but the loop tail is DVE-only and
    gate generation leads by 4 chunks, so the ACT queue is drained when the
    next block's score exps arrive; output DMAs are batched 4 chunks per
    dma_start (HWDGE descriptor generation is a serial ~630ns/dma).
  - input DMAs are ordered by first use with the adapter matmuls leading
    the head matmuls by 2 kt steps, so the projection sweep starts on x
    alone while wq/wk stream in behind; the first four wv chunks are
    prefetched into a small persistent tile near the end of the q/k wave
    so the v loop's first token-block starts without a DMA wait.

self-contained: hardcodes the problem shapes; only needs `concourse` (on
PYTHONPATH in this container) + jax axon devices.
"""

import math
from contextlib import ExitStack
from dataclasses import dataclass

import numpy as np
import ml_dtypes

import concourse.tile as tile
from concourse import bacc
from concourse import mybir
from concourse import bass_utils

BF16 = mybir.dt.bfloat16
F16 = mybir.dt.float16
F32 = mybir.dt.float32
AF = mybir.ActivationFunctionType


@dataclass(frozen=True)
class Cfg:
    B: int = 2
    S: int = 2048
    DIM: int = 2048
    NH: int = 16
    HD: int = 128
    RANK: int = 32
    NCORES: int = 8
    GROUPS: int = 2     # batch groups of 4 cores
    QT: int = 512       # query block (free dim of score tiles)
    KT: int = 128       # key block (partition dim of score tiles)
    PIPE: int = 8       # score tiles in flight ahead of rowsum/AV drains
    repeat: int = 1     # hardware-loop repetitions of the whole body (timing)
    # ablation flags (profiling on hardware; all True for the real kernel)
    use_gate: bool = True
    use_rowsum: bool = True
    use_attn: bool = True
    use_wo: bool = True

    @property
    def CPG(self):
        return self.NCORES // self.GROUPS  # cores per batch group

    @property
    def HLOC(self):
        return self.NH // self.CPG  # heads per core (4)

    @property
    def DH(self):
        return self.HLOC * self.HD  # per-core head-dim span (512)

    @property
    def KTILES(self):
        return self.DIM // 128  # contraction tiles for projections

    @property
    def QTN(self):
        return self.S // self.QT

    @property
    def DIAG(self):
        return self.QT // self.KT  # k-tiles per diagonal band


FULL = Cfg()


def build_nc(cfg: Cfg = FULL):
    c = cfg
    assert c.HD == 128 and c.KT == 128
    nc = bacc.Bacc("TRN2", target_bir_lowering=False, debug=False,
                   num_devices=c.NCORES)

    # ---- kernel I/O (per core: one batch, HLOC heads) ----
    xT = nc.dram_tensor("xT", [c.DIM, c.S + 2 * c.RANK], BF16,
                        kind="ExternalInput")
    wqT = nc.dram_tensor("wqT", [c.DIM, c.DH], BF16, kind="ExternalInput")
    wkT = nc.dram_tensor("wkT", [c.DIM, c.DH], BF16, kind="ExternalInput")
    wvT = nc.dram_tensor("wvT", [c.DIM, c.DH], BF16, kind="ExternalInput")
    wocT = nc.dram_tensor("wocT", [c.DH, c.DIM], BF16, kind="ExternalInput")
    c2d = nc.dram_tensor("c2d", [c.HD, c.S], BF16, kind="ExternalInput")
    s2d = nc.dram_tensor("s2d", [c.HD, c.S], BF16, kind="ExternalInput")
    maskdd = nc.dram_tensor("maskdd", [c.DIAG, c.KT, c.QT], BF16, kind="ExternalInput")

    # partial output projection, transposed: pout[j, t] (fp16; host sums the
    # 4 partials of each batch group in fp32)
    pout = nc.dram_tensor("pout", [c.DIM, c.S], F16, kind="ExternalOutput")

    isqrt = 1.0 / math.sqrt(c.HD)
    sc_score = isqrt
    sc_gate = -1.0
    NKT = c.KTILES

    with ExitStack() as _ctx:
        tc = _ctx.enter_context(tile.TileContext(nc))
        # persistent pools (whole-iteration lifetime)
        cst = _ctx.enter_context(tc.tile_pool(name="const", bufs=1))
        adp = _ctx.enter_context(tc.tile_pool(name="ap", bufs=1))
        qkp = _ctx.enter_context(tc.tile_pool(name="qk", bufs=1))
        vp = _ctx.enter_context(tc.tile_pool(name="vp", bufs=1))
        rtp = _ctx.enter_context(tc.tile_pool(name="rope_t", bufs=1))
        pge = _ctx.enter_context(tc.tile_pool(name="pge", bufs=9))
        gwk = _ctx.enter_context(tc.tile_pool(name="gwk", bufs=2))
        ogp = _ctx.enter_context(tc.tile_pool(name="og", bufs=1))
        # PSUM pools (8 banks total): pp 2 + ps 4 + po 2; the per-head
        # softmax-denominator tiles and the norm-broadcast tiles share the
        # pp rotation so rowsum accumulation never WARs the previous head
        pp = _ctx.enter_context(tc.tile_pool(name="pp", bufs=2, space="PSUM"))
        psp = _ctx.enter_context(tc.tile_pool(name="ps", bufs=4, space="PSUM"))
        pop = _ctx.enter_context(tc.tile_pool(name="po", bufs=2, space="PSUM"))

        def body():
            # ---- constants ----
            c2_sb = cst.tile([128, c.S], BF16, name="c2_sb", tag="c2")
            s2_sb = cst.tile([128, c.S], BF16, name="s2_sb", tag="s2")
            ones_sb = cst.tile([128, 1], BF16, name="ones_sb", tag="ones")

            # packed adapter projections: one [64,512] matmul computes both
            # aq and ak rows (halves the adapter matmul count); ak is then
            # moved to partition base 0 with one SBUF->SBUF DMA (engines
            # cannot shift partitions, DMA can)
            aqk_sb = adp.tile([2 * c.RANK, c.S], BF16, name="aqk_sb", tag="aqk")
            ak_sb = adp.tile([c.RANK, c.S], BF16, name="ak_sb", tag="ak")
            q_sb = [qkp.tile([128, c.S], BF16, name=f"q{h}_sb", tag=f"q{h}")
                    for h in range(c.HLOC)]
            k_sb = [qkp.tile([128, c.S], BF16, name=f"k{h}_sb", tag=f"k{h}")
                    for h in range(c.HLOC)]
            v_sb = vp.tile([128, c.S // 128, c.DH], BF16, name="v_sb", tag="v")
            wv0_sb = vp.tile([128, 4, c.DH], BF16, name="wv0_sb", tag="wv0")

            with tc.tile_pool(name="xtp", bufs=1) as xtp:
                SW = c.S + 2 * c.RANK
                xt_sb = xtp.tile([128, c.KTILES, SW], BF16, name="xt_sb", tag="xt")
                xr = xT.ap().rearrange("(t p) n -> p t n", p=128)
                wa_sb = xt_sb[:, :, c.S:]  # adapter weight columns ride with x

                def xsl(j, sl):
                    return xt_sb[:, j, sl]

                nc.vector.memset(ones_sb, 1.0)

                # ---- adapter + q/k projections, kt-outer, 8 live psum accs ----
                apools = [pp, psp, pop, psp]
                acc = [apools[i].tile([2 * c.RANK, c.QT], F32, name=f"acc_a{i}",
                                      tag=("pp", "ps", "po", "ps")[i])
                       for i in range(c.QTN)]

                wvpool = None
                with tc.tile_pool(name="wqk", bufs=1) as wp:
                    wq_sb = wp.tile([128, c.KTILES, c.DH], BF16, name="wq_sb", tag="wq")
                    wk_sb = wp.tile([128, c.KTILES, c.DH], BF16, name="wk_sb", tag="wk")
                    # input DMAs ordered by first use; the adapter matmuls
                    # lead the head matmuls by 2 kt steps, so the sweep can
                    # start on x alone while wq/wk stream in behind
                    nc.sync.dma_start(out=xt_sb[:, 0, :], in_=xr[:, 0, :])
                    nc.sync.dma_start(out=xt_sb[:, 1, :], in_=xr[:, 1, :])
                    nc.sync.dma_start(out=xt_sb[:, 2, :], in_=xr[:, 2, :])
                    wqr = wqT.ap().rearrange("(t p) m -> p t m", p=128)
                    wkr = wkT.ap().rearrange("(t p) m -> p t m", p=128)
                    nc.scalar.dma_start(out=wq_sb[:, 0:2, :], in_=wqr[:, 0:2, :])
                    nc.scalar.dma_start(out=wk_sb[:, 0:2, :], in_=wkr[:, 0:2, :])
                    nc.sync.dma_start(out=xt_sb[:, 3:5, :], in_=xr[:, 3:5, :])
                    nc.scalar.dma_start(out=wq_sb[:, 2:6, :], in_=wqr[:, 2:6, :])
                    nc.scalar.dma_start(out=wk_sb[:, 2:6, :], in_=wkr[:, 2:6, :])
                    nc.sync.dma_start(out=xt_sb[:, 5:8, :], in_=xr[:, 5:8, :])
                    nc.scalar.dma_start(out=wq_sb[:, 6:, :], in_=wqr[:, 6:, :])
                    nc.scalar.dma_start(out=wk_sb[:, 6:, :], in_=wkr[:, 6:, :])
                    nc.sync.dma_start(out=xt_sb[:, 8:12, :], in_=xr[:, 8:12, :])
                    nc.sync.dma_start(out=xt_sb[:, 12:16, :], in_=xr[:, 12:16, :])

                    allqk = [(dst, w, h, qt)
                             for dst, w in ((q_sb, wq_sb), (k_sb, wk_sb))
                             for h in range(c.HLOC)
                             for qt in range(c.QTN)]
                    # first four groups run kt-outer interleaved with the
                    # adapter accumulation so the PE issues 8 matmuls per
                    # arriving x chunk instead of 4
                    head_pools = [(pp, "pp"), (psp, "ps"), (psp, "ps"),
                                  (pop, "po")]
                    headacc = [head_pools[i][0].tile([128, c.QT], F32,
                                                     name=f"acc_qk{i}",
                                                     tag=head_pools[i][1])
                               for i in range(4)]
                    LAG = 3
                    for s in range(NKT + LAG):
                        if s < NKT:
                            for qt in range(c.QTN):
                                nc.tensor.matmul(
                                    acc[qt][:, :],
                                    wa_sb[:, s, :],
                                    xsl(s, slice(qt * c.QT, (qt + 1) * c.QT)),
                                    start=(s == 0), stop=(s == NKT - 1))

                        if s >= LAG:
                            kt = s - LAG
                            for i in range(4):
                                dst, w, h, qt = allqk[i]
                                nc.tensor.matmul(
                                    headacc[i][:, :],
                                    w[:, kt, h * 128:(h + 1) * 128],
                                    xsl(kt, slice(qt * c.QT, (qt + 1) * c.QT)),
                                    start=(kt == 0), stop=(kt == NKT - 1))
                    for qt in range(c.QTN):
                        nc.vector.tensor_copy(
                            aqk_sb[:, qt * c.QT:(qt + 1) * c.QT], acc[qt][:, :])
                    nc.sync.dma_start(out=ak_sb[:, :],
                                      in_=aqk_sb[c.RANK:2 * c.RANK, :])
                    for i in range(4):
                        dst, w, h, qt = allqk[i]
                        nc.scalar.copy(dst[h][:, qt * c.QT:(qt + 1) * c.QT],
                                       headacc[i][:, :])
                    wr = wvT.ap().rearrange("(t p) m -> p t m", p=128)
                    for gi, (dst, w, h, qt) in enumerate(allqk[4:]):
                        if gi == 24:
                            # prefetch the first wv chunks so the v loop's
                            # first token-block starts without a DMA wait
                            nc.sync.dma_start(out=wv0_sb, in_=wr[:, 0:4, :])
                        psum = pp.tile([128, c.QT], F32, name="psum_qk", tag="pp")
                        for kt in range(NKT):
                            nc.tensor.matmul(
                                psum[:, :],
                                w[:, kt, h * 128:(h + 1) * 128],
                                xsl(kt, slice(qt * c.QT, (qt + 1) * c.QT)),
                                start=(kt == 0), stop=(kt == NKT - 1))
                        nc.scalar.copy(dst[h][:, qt * c.QT:(qt + 1) * c.QT],
                                       psum[:, :])

                # ---- v projection: [tok, d] natural, moving 512 wide ----
                # RoPE tiles are interleaved into the v loop: the rope chain
                # is DVE/GPSIMD-paced, the v matmuls keep the PE busy
                rope_tiles = [(tiles, h, qt)
                              for tiles in (q_sb, k_sb)
                              for h in range(c.HLOC)
                              for qt in range(c.QTN)]

                def rope_step(i):
                    # q/k head dims use a split re/im layout (host permutes
                    # wq/wk columns and the rope tables; scores are invariant
                    # to a shared d-permutation), so the pair-swap is a
                    # half-swap: two contiguous partition-range DMAs instead
                    # of a PE permute matmul.
                    tiles, h, qt = rope_tiles[i]
                    eng = nc.vector if i % 2 == 0 else nc.gpsimd
                    sl = slice(qt * c.QT, (qt + 1) * c.QT)
                    swp = rtp.tile([128, c.QT], BF16, name="rope_swp",
                                   tag=f"sw{i % 2}")
                    nc.sync.dma_start(out=swp[0:64, :], in_=tiles[h][64:128, sl])
                    nc.sync.dma_start(out=swp[64:128, :], in_=tiles[h][0:64, sl])
                    m1 = rtp.tile([128, c.QT], BF16, name="rope_m1",
                                  tag=f"m1{i % 2}")
                    nc.gpsimd.tensor_mul(m1[:, :], tiles[h][:, sl], c2_sb[:, sl])
                    # sin term lands in place over the swapped copy
                    nc.vector.tensor_mul(swp[:, :], swp[:, :], s2_sb[:, sl])
                    eng.tensor_add(tiles[h][:, sl], m1[:, :], swp[:, :])

                with tc.tile_pool(name="wvp", bufs=1) as wvpool:
                    wv_sb = wvpool.tile([128, c.KTILES - 4, c.DH], BF16,
                                        name="wv_sb", tag="wv")
                    nc.sync.dma_start(out=wv_sb[:, 0:4, :], in_=wr[:, 4:8, :])
                    nc.sync.dma_start(out=wv_sb[:, 4:12, :], in_=wr[:, 8:16, :])
                    # rope tables arrive while the first v token-blocks run
                    nc.sync.dma_start(out=c2_sb, in_=c2d.ap())
                    nc.sync.dma_start(out=s2_sb, in_=s2d.ap())
                    for tt in range(c.S // 128):
                        psum = pp.tile([128, c.DH], F32, name="psum_v", tag="pp")
                        for kt in range(NKT):
                            wsrc = (wv0_sb[:, kt, :] if kt < 4
                                    else wv_sb[:, kt - 4, :])
                            nc.tensor.matmul(
                                psum[:, :],
                                xsl(kt, slice(tt * 128, (tt + 1) * 128)),
                                wsrc,
                                start=(kt == 0), stop=(kt == NKT - 1))
                        nc.scalar.copy(v_sb[:, tt, :], psum[:, :])
                        rope_step(2 * tt)
                        rope_step(2 * tt + 1)

            # ---- attention + output projection, per 512-query block ----
            with tc.tile_pool(name="wog", bufs=1) as wog, \
                    tc.tile_pool(name="wo_out", bufs=2) as wop:
                mask_sb = wog.tile([128, c.DIAG, c.QT], BF16, name="mask_sb",
                                   tag="mask")
                nc.sync.dma_start(out=mask_sb,
                                  in_=maskdd.ap().rearrange("j p q -> p j q"))
                woc_sb = wog.tile([128, c.HLOC, c.DIM], BF16, name="woc_sb", tag="woc")
                wcr = wocT.ap().rearrange("(h p) j -> p h j", p=128)
                nc.sync.dma_start(out=woc_sb[:, 0:2, :], in_=wcr[:, 0:2, :])
                nc.sync.dma_start(out=woc_sb[:, 2:4, :], in_=wcr[:, 2:4, :])
                # double-buffered per-qt gate tiles r = sigmoid(A)[k, q]
                rg_sb = [wog.tile([128, c.DIAG * c.QTN, c.QT], BF16,
                                  name=f"rg{i}_sb", tag=f"rg{i}") for i in range(2)]

                def gate_step(qt, kt):
                    # r = 1/(1+exp(-A)) via the (shared) Exp table
                    qsl = slice(qt * c.QT, (qt + 1) * c.QT)
                    ksl = slice(kt * c.KT, (kt + 1) * c.KT)
                    pga = psp.tile([128, c.QT], F32, name="pga", tag="ps")
                    nc.tensor.matmul(pga[:, :], ak_sb[:, ksl],
                                     aqk_sb[0:c.RANK, qsl],
                                     start=True, stop=True)
                    ge = pge.tile([128, c.QT], BF16, name="ge", tag="p")
                    nc.scalar.activation(ge[:, :], pga[:, :], AF.Exp, scale=sc_gate)
                    gt = gwk.tile([128, c.QT], F32, name="gt", tag="gt")
                    nc.vector.tensor_scalar_add(gt[:, :], ge[:, :], 1.0)
                    gr = gwk.tile([128, c.QT], F32, name="gr", tag="gr")
                    nc.vector.reciprocal_approx_fast(out=gr[:, :], in_=gt[:, :])
                    nc.gpsimd.tensor_copy(rg_sb[qt % 2][:, kt, :], gr[:, :])

                if c.use_gate and c.use_attn:
                    for kt in range(c.DIAG):
                        gate_step(0, kt)

                for qt in range(c.QTN):
                    qsl = slice(qt * c.QT, (qt + 1) * c.QT)
                    nkt = c.DIAG * (qt + 1)  # causal k tiles
                    rg = rg_sb[qt % 2]

                    og_sb = ogp.tile([128, c.HLOC, c.QT], BF16, name="og_sb", tag="og")
                    if not c.use_attn:
                        nc.vector.memset(og_sb, 0.0)

                    def normalize(h, po, prs):
                        if not c.use_rowsum:
                            nc.vector.tensor_copy(og_sb[:, h, :], po[:, :])
                            return
                        rr = gwk.tile([1, c.QT], F32, name="rr", tag="rr", bufs=1)
                        nc.vector.reciprocal_approx_fast(
                            out=rr[:, :], in_=prs[0:1, :])
                        rrh = gwk.tile([1, c.QT], F16, name="rrh", tag="rrh", bufs=1)
                        nc.vector.tensor_copy(rrh[:, :], rr[:, :])
                        # GPSIMD extended-ISA broadcast: partition 0 -> all
                        rbs = gwk.tile([128, c.QT], F16, name="rbs", tag="rbs", bufs=1)
                        nc.gpsimd.partition_broadcast(rbs[:, :], rrh[0:1, :])
                        nc.vector.tensor_mul(og_sb[:, h, :], po[:, :],
                                             rbs[:, :])

                    # single software pipeline over all (h, kt) score tiles:
                    # drains lag PIPE steps behind scores ACROSS head
                    # boundaries, so head-end drain tails overlap the next
                    # head's score matmuls; normalize(h) is emitted inline
                    # right after head h's last drain
                    po_t, prs_t = {}, {}
                    stage = []  # (h, kt, col-slice, p_or_pm_tile, pgm_tile)

                    def score_step(h, kt):
                        ksl = slice(kt * c.KT, (kt + 1) * c.KT)
                        j = kt - c.DIAG * qt
                        qoff = 128 * j if j > 0 else 0
                        s = slice(qoff, c.QT)
                        qs = slice(qt * c.QT + qoff, (qt + 1) * c.QT)
                        ps = psp.tile([128, c.QT], F32, name="ps", tag="ps")
                        nc.tensor.matmul(ps[:, s], k_sb[h][:, ksl],
                                         q_sb[h][:, qs], start=True, stop=True)
                        p_sb = pge.tile([128, c.QT], BF16, name="p_sb", tag="p")
                        nc.scalar.activation(p_sb[:, s], ps[:, s], AF.Exp,
                                             scale=sc_score)
                        if j >= 0:
                            # diagonal band: 0/1 causal mask after exp,
                            # in place (rowsum needs the masked pre-gate sum)
                            nc.vector.tensor_mul(p_sb[:, s], p_sb[:, s],
                                                 mask_sb[:, j, s])
                        if c.use_gate:
                            pgm = pge.tile([128, c.QT], BF16, name="pgm",
                                           tag="pgm")
                            nc.vector.tensor_mul(pgm[:, s], p_sb[:, s],
                                                 rg[:, kt, s])
                        else:
                            pgm = p_sb
                        stage.append((h, kt, s, p_sb, pgm))

                    def drain_step():
                        h, kt, s, p_sb, pgm = stage.pop(0)
                        # pre-gate rowsum (softmax denominator)
                        if c.use_rowsum:
                            nc.tensor.matmul(prs_t[h][0:1, s],
                                             ones_sb[:, :], p_sb[:, s],
                                             start=(kt == 0),
                                             stop=(kt == nkt - 1),
                                             skip_group_check=True)
                        # out_h^T[d, q] += v[k, d].T @ p_gated[k, q]
                        nc.tensor.matmul(po_t[h][:, s],
                                         v_sb[:, kt, h * 128:(h + 1) * 128],
                                         pgm[:, s],
                                         start=(kt == 0), stop=(kt == nkt - 1),
                                         skip_group_check=True)
                        if kt == nkt - 1:
                            normalize(h, po_t[h], prs_t[h])

                    for h in range(c.HLOC if c.use_attn else 0):
                        po_t[h] = pop.tile([128, c.QT], F32, name="po", tag="po")
                        prs_t[h] = pp.tile([1, c.QT], F32, name="prs", tag="pp")
                        for kt in range(nkt):
                            score_step(h, kt)
                            if len(stage) > c.PIPE:
                                drain_step()
                    while stage:
                        drain_step()

                    # output-projection partial for this query block,
                    # interleaved with the NEXT block's gate generation so the
                    # PE fills the gate chain's latency with wo matmuls
                    nkt2 = (c.DIAG * (qt + 2)
                            if (qt + 1 < c.QTN and c.use_gate and c.use_attn)
                            else 0)
                    ncha = c.DIM // 128
                    f_sb = wop.tile([128, ncha, c.QT], F16, name="f_sb", tag="f")
                    pfpools = [(pp, "pp"), (pop, "po")]
                    # four gate steps lead the wo loop: their matmuls keep
                    # the PE fed while the last head's normalize completes,
                    # and no gate exp lands in the loop's tail where it would
                    # delay the next block's first score exps on ACT
                    for kt in range(min(4, nkt2)):
                        gate_step(qt + 1, kt)
                    for ch in range(ncha if c.use_wo else 0):
                        pfp, pft = pfpools[ch % 2]
                        pf = pfp.tile([128, c.QT], F32, name="pf", tag=pft)
                        for h in range(c.HLOC):
                            nc.tensor.matmul(
                                pf[:, :],
                                woc_sb[:, h, ch * 128:(ch + 1) * 128],
                                og_sb[:, h, :],
                                start=(h == 0), stop=(h == c.HLOC - 1))
                        if ch % 2 == 0 and ch < 12:
                            nc.scalar.copy(f_sb[:, ch, :], pf[:, :])
                        else:
                            nc.vector.tensor_copy(f_sb[:, ch, :], pf[:, :])
                        if ch + 4 < nkt2:
                            gate_step(qt + 1, ch + 4)
                        if qt == c.QTN - 1 and ch % 2 == 1:
                            nc.sync.dma_start(
                                out=pout.ap().rearrange(
                                    "(ch p) q -> p ch q",
                                    p=128)[:, ch - 1:ch + 1, qsl],
                                in_=f_sb[:, ch - 1:ch + 1, :])
                        elif qt < c.QTN - 1 and ch % 4 == 3:
                            # batched output DMA per 4 chunks (0.5MB each):
                            # early chunks fly while later ones compute
                            nc.sync.dma_start(
                                out=pout.ap().rearrange(
                                    "(ch p) q -> p ch q",
                                    p=128)[:, ch - 3:ch + 1, qsl],
                                in_=f_sb[:, ch - 3:ch + 1, :])
                    if not c.use_wo:
                        for kt in range(nkt2):
                            gate_step(qt + 1, kt)

        if c.repeat > 1:
            with tc.For_i(0, c.repeat, 1,
                          hint_engines=(mybir.EngineType.PE,
                                        mybir.EngineType.DVE,
                                        mybir.EngineType.Activation,
                                        mybir.EngineType.Pool,
                                        mybir.EngineType.SP)):
                body()
        else:
            body()

    nc.compile()
    return nc


def make_core_inputs(inputs: dict, cfg: Cfg = FULL):
    """Host-side sharding: returns in_maps (one dict per core)."""
    c = cfg
    bf16 = ml_dtypes.bfloat16
    x = np.asarray(inputs["x"])
    mask = np.asarray(inputs["mask"])
    fc = np.asarray(inputs["freqs_cos"])
    fs = np.asarray(inputs["freqs_sin"])
    wq, wk, wv, wo = (np.asarray(inputs[k]) for k in ("wq", "wk", "wv", "wo"))
    wa_q, wa_k = np.asarray(inputs["wa_q"]), np.asarray(inputs["wa_k"])

    waT = np.ascontiguousarray(
        np.concatenate([wa_q, wa_k], axis=0).T).astype(np.float32)
    xTb = [np.ascontiguousarray(
        np.concatenate([x[b].T, waT], axis=1)).astype(bf16)
        for b in range(c.B)]

    # rope tables in [d, tok] layout, split re/im: rows 0:64 = re lanes,
    # 64:128 = im lanes (wq/wk columns are permuted to match; the score
    # dot-product is invariant to a shared head-dim permutation)
    c2 = np.empty((c.HD, c.S), np.float32)
    s2 = np.empty((c.HD, c.S), np.float32)
    c2[0:64] = fc.T
    c2[64:128] = fc.T
    s2[0:64] = -fs.T
    s2[64:128] = fs.T
    c2 = c2.astype(bf16)
    s2 = s2.astype(bf16)

    # per-head column permutation: even (re) dims first, odd (im) second
    dperm = np.concatenate([np.arange(0, c.HD, 2), np.arange(1, c.HD, 2)])
    qkperm = np.concatenate([hb * c.HD + dperm
                             for hb in range(c.DIM // c.HD)])

    # diagonal-band mask patterns [j][k, q], extracted from the input mask
    qt_last = c.QTN - 1
    q0 = qt_last * c.QT
    maskd = np.empty((c.DIAG, c.KT, c.QT), np.float32)
    for j in range(c.DIAG):
        k0 = (c.DIAG * qt_last + j) * c.KT
        maskd[j] = (mask[0, 0, q0:q0 + c.QT, k0:k0 + c.KT].T == 0.0)
    maskd = maskd.astype(bf16)

    wslices = []
    for hs in range(c.CPG):
        rows = slice(hs * c.DH, (hs + 1) * c.DH)
        wslices.append({
            "wqT": np.ascontiguousarray(wq[qkperm][rows].T).astype(bf16),
            "wkT": np.ascontiguousarray(wk[qkperm][rows].T).astype(bf16),
            "wvT": np.ascontiguousarray(wv[rows].T).astype(bf16),
            "wocT": np.ascontiguousarray(wo[:, rows].T).astype(bf16),
        })

    in_maps = []
    for ci in range(c.NCORES):
        b = ci // c.CPG
        hs = ci % c.CPG
        in_maps.append({
            "xT": xTb[b],
            **wslices[hs],
            "c2d": c2,
            "s2d": s2,
            "maskdd": maskd,
        })
    return in_maps


def assemble_output(results, cfg: Cfg = FULL) -> np.ndarray:
    c = cfg
    out = np.empty((c.B, c.S, c.DIM), np.float32)
    for b in range(c.B):
        total = np.zeros((c.DIM, c.S), np.float32)
        for hs in range(c.CPG):
            total += np.asarray(results[b * c.CPG + hs]["pout"]).astype(np.float32)
        out[b] = total.T
    return out


_NC_CACHE = {}


def run(nc, in_maps, trace=False, cfg: Cfg = FULL, **kw):
    return bass_utils.run_bass_kernel_spmd(
        nc, in_maps, core_ids=list(range(cfg.NCORES)), trace=trace, **kw)


def kernel(**inputs) -> np.ndarray:
    cfg = FULL
    if cfg not in _NC_CACHE:
        _NC_CACHE[cfg] = build_nc(cfg)
    nc = _NC_CACHE[cfg]
    in_maps = make_core_inputs(inputs, cfg)
    res = run(nc, in_maps, cfg=cfg)
    return assemble_output(res.results, cfg)


if __name__ == "__main__":
    nc = build_nc(FULL)
    print("built ok")
